# revision 1
# baseline (speedup 1.0000x reference)
"""GCN (2-layer) on 8 Trainium2 NeuronCores.

Strategy (graph/data parallel, per the node-range sharding hint):
- Nodes are sharded by range (25k per core); edges live on the core that
  owns their *destination* node; tiny weights are replicated.
- All irregular graph routing is converted on the host into REGULAR
  device-side layouts:
    * scatter side: destination nodes are bucketed by in-degree class
      j=ceil(d/8); each node gets exactly 8j message slots, so segment-sum
      becomes plain `tensor_reduce` over the innermost axis.
    * gather side: source rows are bucketed by multiplicity class m
      (# edges this core pulls from the row); the per-core permuted table
      is expanded by stride-0 broadcast copies [P,b,f] -> [P,b,f,m].
- The per-edge routing between gather order and scatter order (the
  "all-to-all on gathered messages") is staged through the host between
  device programs; every floating-point operation on values happens on
  device.

Five small SPMD programs: deg/dis -> expand L1 -> reduce L1 + MLP ->
expand L2 -> reduce L2.
"""
import sys

sys.path.insert(0, "/opt/trn_rl_repo")

import numpy as np

import bass_rust
from concourse import bass, mybir
from concourse.bass_utils import run_bass_kernel_spmd
import concourse.tile as tile

import os as _os

PROGRAM_TIMES_NS = []   # (name, exec_time_ns) per device program of last kernel() call


def _enable_tracing():
    import types
    import antenv
    if 'antenv.axon_hooks' in sys.modules:
        return True
    try:
        from trn_agent_boot.trn_boot import _ntff_profile_via_ctypes
        hook = _ntff_profile_via_ctypes('/opt/axon/libaxon_pjrt.so')
    except Exception:
        return False
    mod = types.ModuleType('antenv.axon_hooks')
    mod.get_axon_ntff_profile_hook = lambda: hook
    mod.set_axon_ntff_profile_hook = lambda h: None
    sys.modules['antenv.axon_hooks'] = mod
    antenv.axon_hooks = mod
    import concourse.bass_utils as _bu
    _bu.upload_artifacts = lambda tmpdir: f"local://{tmpdir}"
    return True


def _run(nc, in_maps, name):
    trace = bool(_os.environ.get('GCN_TRACE')) and _enable_tracing()
    r = run_bass_kernel_spmd(nc, in_maps, core_ids=CORE_IDS, trace=trace)
    if trace:
        PROGRAM_TIMES_NS.append((name, r.exec_time_ns))
    return r.results

S = 8
N = 200000
NS = N // S
P = 128
F1 = 4
F2 = 7
CORE_IDS = list(range(S))
FP = mybir.dt.float32
MUL = mybir.AluOpType.mult
ADD = mybir.AluOpType.add


def _ceil(a, b):
    return -(-a // b)


# --------------------------------------------------------------------------
# walrus on this toolchain accepts at most ONE sync-wait per instruction;
# Tile emits several at DAG joins / kernel-tail drain. Hoist excess waits
# onto fresh same-engine NoOps inserted immediately before the violator.
def legalize_waits(nc):
    nop_idx = 0
    for f in nc.m.functions:
        for bb in f.blocks:
            il = bb.instructions
            if not any(
                inst.sync_info is not None
                and len(inst.sync_info.on_wait or []) > 1
                for inst in il
            ):
                continue
            new_il = []
            for inst in il:
                si = inst.sync_info
                w = list(si.on_wait or []) if si is not None else []
                if len(w) > 1:
                    for extra in w[:-1]:
                        nop = mybir.InstNoOp(
                            name=f"I-waitsplit-{nop_idx}", ins=[], outs=[]
                        )
                        nop_idx += 1
                        nop.engine = inst.engine
                        nop.sync_info = bass_rust.SyncInfo(
                            on_wait=[extra], on_update=[]
                        )
                        new_il.append(nop)
                    si.on_wait = [w[-1]]
                new_il.append(inst)
            bb.instructions = new_il


# --------------------------------------------------------------------------
# host-side structure building
class _O:
    pass


def build_structs(row, col, ew):
    row = row.astype(np.int64)
    col = col.astype(np.int64)
    cores = []
    for c in range(S):
        cs = _O()
        m = (col // NS) == c
        cs.erow = row[m]
        cs.ecol = (col[m] - c * NS).astype(np.int64)
        cs.eew = ew[m].astype(np.float32)
        cores.append(cs)

    for cs in cores:
        d = np.bincount(cs.ecol, minlength=NS)
        cs.indeg = d
        cs.jcls = np.maximum(1, _ceil(np.maximum(d, 1), 8))
    jmax = max(int(cs.jcls.max()) for cs in cores)
    nj = np.zeros(jmax + 1, np.int64)
    for j in range(1, jmax + 1):
        njc = max(int((cs.jcls == j).sum()) for cs in cores)
        nj[j] = _ceil(max(njc, 1), P) * P
    for cs in cores:
        nodepos = np.full(NS, -1, np.int64)
        pos = 0
        for j in range(1, jmax + 1):
            nodes = np.nonzero(cs.jcls == j)[0]
            nodepos[nodes] = pos + np.arange(len(nodes))
            pos += nj[j]
        cs.nodepos = nodepos
    ntot = int(nj[1:].sum())

    for cs in cores:
        rows_used, inv, cnt = np.unique(
            cs.erow, return_inverse=True, return_counts=True
        )
        cs.g_rows = rows_used
        cs.g_cnt = cnt
        cs.g_inv = inv
    mmax = max(int(cs.g_cnt.max()) for cs in cores)
    tm = np.zeros(mmax + 1, np.int64)
    for mcl in range(1, mmax + 1):
        tmc = max(int((cs.g_cnt == mcl).sum()) for cs in cores)
        tm[mcl] = _ceil(max(tmc, 1), P) * P
    for cs in cores:
        tabrows = np.full(int(tm[1:].sum()), -1, np.int64)
        pos_of_unique = np.empty(len(cs.g_rows), np.int64)
        pos = 0
        for mcl in range(1, mmax + 1):
            sel = cs.g_cnt == mcl
            rr = cs.g_rows[sel]
            tabrows[pos : pos + len(rr)] = rr
            pos_of_unique[sel] = pos - 0 + np.arange(len(rr))
            pos += tm[mcl]
        cs.tabrows = tabrows
        cs.g_tabpos = pos_of_unique[cs.g_inv]
    rtot = int(tm[1:].sum())

    st = _O()
    st.jmax, st.nj, st.ntot = jmax, nj, ntot
    st.mmax, st.tm, st.rtot = mmax, tm, rtot
    st.cores = cores
    base_tab = np.zeros(mmax + 2, np.int64)
    base_tab[1:] = np.cumsum(tm)[: mmax + 1]
    st.base_tab = base_tab

    for cs in cores:
        mm = cs.g_cnt[cs.g_inv]
        ord_ = np.argsort(cs.g_inv, kind="stable")
        inv_sorted = cs.g_inv[ord_]
        first = np.r_[True, inv_sorted[1:] != inv_sorted[:-1]]
        idx_of_first = np.maximum.accumulate(
            np.where(first, np.arange(len(ord_)), 0)
        )
        occ = np.empty(len(cs.erow), np.int64)
        occ[ord_] = np.arange(len(ord_)) - idx_of_first
        cs.g_m = mm
        cs.g_occ = occ
    return st


def gather_flat_index(st, cs, f):
    mm = cs.g_m
    q_local = cs.g_tabpos - st.base_tab[mm]
    tbm = st.tm[mm] // P
    p = q_local // tbm
    b = q_local % tbm
    base_free = np.zeros(st.mmax + 1, np.int64)
    acc = 0
    for mcl in range(1, st.mmax + 1):
        base_free[mcl] = acc
        acc += (st.tm[mcl] // P) * f * mcl
    gfree = base_free[mm] + b * (f * mm) + cs.g_occ
    return p, gfree, acc


def scatter_flat_index(st, cs, f):
    nodes = cs.ecol
    j = cs.jcls[nodes]
    q = cs.nodepos[nodes]
    base_node = np.zeros(st.jmax + 1, np.int64)
    accn = 0
    for jj in range(1, st.jmax + 1):
        base_node[jj] = accn
        accn += st.nj[jj]
    q_local = q - base_node[j]
    nbj = st.nj[j] // P
    p = q_local // nbj
    b = q_local % nbj
    base_free = np.zeros(st.jmax + 1, np.int64)
    acc = 0
    for jj in range(1, st.jmax + 1):
        base_free[jj] = acc
        acc += (st.nj[jj] // P) * f * 8 * jj
    ord_ = np.argsort(nodes, kind="stable")
    ns = nodes[ord_]
    first = np.r_[True, ns[1:] != ns[:-1]]
    idx_of_first = np.maximum.accumulate(np.where(first, np.arange(len(ord_)), 0))
    k = np.empty(len(nodes), np.int64)
    k[ord_] = np.arange(len(nodes)) - idx_of_first
    sfree = base_free[j] + b * (f * 8 * j) + k
    return p, sfree, acc


def own_perm(st, cs):
    """per local node: (partition, block) in the scatter/agg [P, ntot/P] grid"""
    base_node = np.zeros(st.jmax + 1, np.int64)
    accn = 0
    for jj in range(1, st.jmax + 1):
        base_node[jj] = accn
        accn += st.nj[jj]
    base_nb = np.zeros(st.jmax + 1, np.int64)
    accb = 0
    for jj in range(1, st.jmax + 1):
        base_nb[jj] = accb
        accb += st.nj[jj] // P
    j = cs.jcls
    ql = cs.nodepos - base_node[j]
    nbj = st.nj[j] // P
    return ql // nbj, base_nb[j] + ql % nbj


def tab_place(st):
    """table position q -> (p, block) in [P, rtot/P]."""
    pp = np.empty(st.rtot, np.int64)
    bb = np.empty(st.rtot, np.int64)
    accb = 0
    pos = 0
    for mcl in range(1, st.mmax + 1):
        tbm = st.tm[mcl] // P
        ql = np.arange(st.tm[mcl])
        pp[pos : pos + st.tm[mcl]] = ql // tbm
        bb[pos : pos + st.tm[mcl]] = accb + ql % tbm
        pos += st.tm[mcl]
        accb += tbm
    return pp, bb


# --------------------------------------------------------------------------
# device programs
_CHUNK = 6144  # free-size chunk budget (fp32 elems per partition) for streaming


def _chunks(total, step):
    out = []
    o = 0
    while o < total:
        out.append((o, min(step, total - o)))
        o += step
    return out


def build_PA(st):
    """ews [P, SF1] -> dis [P, ntot/P] ; deg = reduce + 1 ; dis = rsqrt."""
    nc = bass.Bass("TRN2", num_devices=S)
    SF1 = sum((int(st.nj[j]) // P) * 8 * j for j in range(1, st.jmax + 1))
    nb_all = st.ntot // P
    ews = nc.dram_tensor("ews", (P, SF1), FP, kind="ExternalInput")
    dis_o = nc.dram_tensor("dis", (P, nb_all), FP, kind="ExternalOutput")
    with tile.TileContext(nc) as tc:
        with tc.tile_pool(name="sb", bufs=2) as pool, tc.tile_pool(
            name="acc", bufs=1
        ) as apool:
            t_deg = apool.tile([P, nb_all], FP)
            accf = 0
            accb = 0
            for j in range(1, st.jmax + 1):
                nbj = int(st.nj[j]) // P
                L = 8 * j
                for b0, bl in _chunks(nbj, max(1, _CHUNK // L)):
                    t_in = pool.tile([P, bl * L], FP, tag="in")
                    nc.sync.dma_start(
                        out=t_in[:],
                        in_=ews[:, accf + b0 * L : accf + (b0 + bl) * L],
                    )
                    nc.vector.tensor_reduce(
                        out=t_deg[:, accb + b0 : accb + b0 + bl],
                        in_=t_in[:].rearrange("p (b l) -> p b l", l=L),
                        axis=mybir.AxisListType.X,
                        op=ADD,
                    )
                accf += nbj * L
                accb += nbj
            t_d1 = apool.tile([P, nb_all], FP)
            nc.vector.tensor_scalar_add(t_d1[:], t_deg[:], 1.0)
            t_sq = apool.tile([P, nb_all], FP)
            nc.scalar.sqrt(t_sq[:], t_d1[:])
            t_r = apool.tile([P, nb_all], FP)
            nc.vector.reciprocal(t_r[:], t_sq[:])
            # one Newton step: y <- y * (1.5 - 0.5 * d * y^2)
            t_y2 = apool.tile([P, nb_all], FP)
            nc.vector.tensor_tensor(t_y2[:], t_r[:], t_r[:], MUL)
            nc.vector.tensor_tensor(t_y2[:], t_y2[:], t_d1[:], MUL)
            nc.vector.tensor_scalar_mul(t_y2[:], t_y2[:], -0.5)
            nc.vector.tensor_scalar_add(t_y2[:], t_y2[:], 1.5)
            nc.vector.tensor_tensor(t_r[:], t_r[:], t_y2[:], MUL)
            nc.sync.dma_start(out=dis_o[:], in_=t_r[:])
    legalize_waits(nc)
    return nc


def build_expand(st, F, scale_dis):
    """x_tab [P, RB*F] (+ dis_tab [P, RB] if scale_dis), ewg [P, EWT]
    -> msgs_g [P, GF]."""
    nc = bass.Bass("TRN2", num_devices=S)
    RB = st.rtot // P
    EWT = sum((int(st.tm[m]) // P) * m for m in range(1, st.mmax + 1))
    GF = sum((int(st.tm[m]) // P) * F * m for m in range(1, st.mmax + 1))
    x_tab = nc.dram_tensor("x_tab", (P, RB * F), FP, kind="ExternalInput")
    ewg = nc.dram_tensor("ewg", (P, EWT), FP, kind="ExternalInput")
    if scale_dis:
        dis_tab = nc.dram_tensor("dis_tab", (P, RB), FP, kind="ExternalInput")
    msgs = nc.dram_tensor("msgs", (P, GF), FP, kind="ExternalOutput")
    with tile.TileContext(nc) as tc:
        with tc.tile_pool(name="tab", bufs=1) as tpool, tc.tile_pool(
            name="str", bufs=3
        ) as pool:
            t_tab = tpool.tile([P, RB * F], FP)
            nc.sync.dma_start(out=t_tab[:], in_=x_tab[:])
            if scale_dis:
                t_dis = tpool.tile([P, RB], FP)
                nc.sync.dma_start(out=t_dis[:], in_=dis_tab[:])
                nc.vector.tensor_tensor(
                    t_tab[:].rearrange("p (b f) -> p b f", f=F),
                    t_tab[:].rearrange("p (b f) -> p b f", f=F),
                    t_dis[:].unsqueeze(2).broadcast_to([P, RB, F]),
                    MUL,
                )
            t_ew = tpool.tile([P, EWT], FP)
            nc.sync.dma_start(out=t_ew[:], in_=ewg[:])
            accb = 0
            accw = 0
            accg = 0
            for m in range(1, st.mmax + 1):
                tbm = int(st.tm[m]) // P
                for b0, bl in _chunks(tbm, max(1, _CHUNK // (F * m))):
                    t_out = pool.tile([P, bl * F * m], FP, tag="out")
                    src = t_tab[:, (accb + b0) * F : (accb + b0 + bl) * F]
                    ew = t_ew[:, accw + b0 * m : accw + (b0 + bl) * m]
                    nc.vector.tensor_tensor(
                        t_out[:].rearrange("p (b f m) -> p b f m", f=F, m=m),
                        src.rearrange("p (b f) -> p b f", f=F)
                        .unsqueeze(3)
                        .broadcast_to([P, bl, F, m]),
                        ew.rearrange("p (b m) -> p b m", m=m)
                        .unsqueeze(2)
                        .broadcast_to([P, bl, F, m]),
                        MUL,
                    )
                    nc.sync.dma_start(
                        out=msgs[:, accg + b0 * F * m : accg + (b0 + bl) * F * m],
                        in_=t_out[:],
                    )
                accb += tbm
                accw += tbm * m
                accg += tbm * F * m
    legalize_waits(nc)
    return nc


def _reduce_classes(nc, tc, pool, apool, st, F, msgs_in):
    nb_all = st.ntot // P
    t_agg = apool.tile([P, nb_all * F], FP)
    accf = 0
    accb = 0
    for j in range(1, st.jmax + 1):
        nbj = int(st.nj[j]) // P
        L = 8 * j
        for b0, bl in _chunks(nbj, max(1, _CHUNK // (F * L))):
            t_in = pool.tile([P, bl * F * L], FP, tag="rin")
            nc.sync.dma_start(
                out=t_in[:],
                in_=msgs_in[:, accf + b0 * F * L : accf + (b0 + bl) * F * L],
            )
            nc.vector.tensor_reduce(
                out=t_agg[:, (accb + b0) * F : (accb + b0 + bl) * F],
                in_=t_in[:].rearrange("p (b f l) -> p b f l", f=F, l=L),
                axis=mybir.AxisListType.X,
                op=ADD,
            )
        accf += nbj * F * L
        accb += nbj
    return t_agg


def build_PC(st):
    """msgs_s + x_own + dis_own + weights -> ys [P, nb*F2] (col 7 zero)."""
    nc = bass.Bass("TRN2", num_devices=S)
    nb = st.ntot // P
    SF = sum((int(st.nj[j]) // P) * F1 * 8 * j for j in range(1, st.jmax + 1))
    msgs = nc.dram_tensor("msgs", (P, SF), FP, kind="ExternalInput")
    x_own = nc.dram_tensor("x_own", (P, nb * F1), FP, kind="ExternalInput")
    dis_own = nc.dram_tensor("dis_own", (P, nb), FP, kind="ExternalInput")
    w1b = nc.dram_tensor("w1b", (P, F1 * 16), FP, kind="ExternalInput")
    b1b = nc.dram_tensor("b1b", (P, 16), FP, kind="ExternalInput")
    w2b = nc.dram_tensor("w2b", (P, 16 * 7), FP, kind="ExternalInput")
    ys_o = nc.dram_tensor("ys", (P, nb * F2), FP, kind="ExternalOutput")
    with tile.TileContext(nc) as tc:
        with tc.tile_pool(name="sb", bufs=3) as pool, tc.tile_pool(
            name="acc", bufs=1
        ) as apool:
            t_agg = _reduce_classes(nc, tc, pool, apool, st, F1, msgs)
            t_xo = apool.tile([P, nb * F1], FP)
            nc.sync.dma_start(out=t_xo[:], in_=x_own[:])
            t_do = apool.tile([P, nb], FP)
            nc.sync.dma_start(out=t_do[:], in_=dis_own[:])
            t_w1 = apool.tile([P, F1 * 16], FP)
            nc.sync.dma_start(out=t_w1[:], in_=w1b[:])
            t_b1 = apool.tile([P, 16], FP)
            nc.sync.dma_start(out=t_b1[:], in_=b1b[:])
            t_w2 = apool.tile([P, 16 * 7], FP)
            nc.sync.dma_start(out=t_w2[:], in_=w2b[:])

            dis_b = t_do[:].unsqueeze(2).broadcast_to([P, nb, F1])
            agg_r = t_agg[:].rearrange("p (b f) -> p b f", f=F1)
            xo_r = t_xo[:].rearrange("p (b f) -> p b f", f=F1)
            # v = dis * (agg + dis * x_own)
            nc.vector.tensor_tensor(xo_r, xo_r, dis_b, MUL)
            nc.vector.tensor_tensor(agg_r, agg_r, xo_r, ADD)
            nc.vector.tensor_tensor(agg_r, agg_r, dis_b, MUL)
            # h = relu(v @ W1 + b1)   (v[...,3] is zero-padded; W1 row 3 = 0)
            t_h = apool.tile([P, nb * 16], FP)
            h_r = t_h[:].rearrange("p (b o) -> p b o", o=16)
            t_tmp = apool.tile([P, nb * 16], FP)
            tmp_r = t_tmp[:].rearrange("p (b o) -> p b o", o=16)
            for i in range(3):
                vi = (
                    t_agg[:]
                    .rearrange("p (b f) -> p b f", f=F1)[:, :, i : i + 1]
                    .broadcast_to([P, nb, 16])
                )
                wrow = (
                    t_w1[:, i * 16 : (i + 1) * 16]
                    .unsqueeze(1)
                    .broadcast_to([P, nb, 16])
                )
                if i == 0:
                    nc.vector.tensor_tensor(h_r, vi, wrow, MUL)
                else:
                    nc.vector.tensor_tensor(tmp_r, vi, wrow, MUL)
                    nc.vector.tensor_tensor(h_r, h_r, tmp_r, ADD)
            nc.vector.tensor_tensor(
                h_r, h_r, t_b1[:].unsqueeze(1).broadcast_to([P, nb, 16]), ADD
            )
            nc.vector.tensor_scalar(
                t_h[:], t_h[:], 0.0, None, mybir.AluOpType.max
            )
            # ys0 = h @ W2 ; ys = dis * ys0 ; pad col 7 with zeros
            t_ys = apool.tile([P, nb * F2], FP)
            nc.vector.memset(t_ys[:], 0.0)
            ys_r = t_ys[:].rearrange("p (b o) -> p b o", o=F2)[:, :, 0:7]
            t_t7 = apool.tile([P, nb * 7], FP)
            t7_r = t_t7[:].rearrange("p (b o) -> p b o", o=7)
            for k in range(16):
                hk = h_r[:, :, k : k + 1].broadcast_to([P, nb, 7])
                wrow = (
                    t_w2[:, k * 7 : (k + 1) * 7]
                    .unsqueeze(1)
                    .broadcast_to([P, nb, 7])
                )
                if k == 0:
                    nc.vector.tensor_tensor(ys_r, hk, wrow, MUL)
                else:
                    nc.vector.tensor_tensor(t7_r, hk, wrow, MUL)
                    nc.vector.tensor_tensor(ys_r, ys_r, t7_r, ADD)
            nc.vector.tensor_tensor(
                ys_r, ys_r, t_do[:].unsqueeze(2).broadcast_to([P, nb, 7]), MUL
            )
            nc.sync.dma_start(out=ys_o[:], in_=t_ys[:])
    legalize_waits(nc)
    return nc


def build_PE(st):
    """msgs2_s + ys_own + dis_own + b2 -> out2 [P, nb*F2]."""
    nc = bass.Bass("TRN2", num_devices=S)
    nb = st.ntot // P
    SF = sum((int(st.nj[j]) // P) * F2 * 8 * j for j in range(1, st.jmax + 1))
    msgs = nc.dram_tensor("msgs", (P, SF), FP, kind="ExternalInput")
    ys_own = nc.dram_tensor("ys_own", (P, nb * F2), FP, kind="ExternalInput")
    dis_own = nc.dram_tensor("dis_own", (P, nb), FP, kind="ExternalInput")
    b2b = nc.dram_tensor("b2b", (P, F2), FP, kind="ExternalInput")
    out_o = nc.dram_tensor("out", (P, nb * F2), FP, kind="ExternalOutput")
    with tile.TileContext(nc) as tc:
        with tc.tile_pool(name="sb", bufs=3) as pool, tc.tile_pool(
            name="acc", bufs=1
        ) as apool:
            t_agg = _reduce_classes(nc, tc, pool, apool, st, F2, msgs)
            t_yo = apool.tile([P, nb * F2], FP)
            nc.sync.dma_start(out=t_yo[:], in_=ys_own[:])
            t_do = apool.tile([P, nb], FP)
            nc.sync.dma_start(out=t_do[:], in_=dis_own[:])
            t_b2 = apool.tile([P, F2], FP)
            nc.sync.dma_start(out=t_b2[:], in_=b2b[:])
            agg_r = t_agg[:].rearrange("p (b f) -> p b f", f=F2)
            yo_r = t_yo[:].rearrange("p (b f) -> p b f", f=F2)
            dis_b = t_do[:].unsqueeze(2).broadcast_to([P, nb, F2])
            nc.vector.tensor_tensor(agg_r, agg_r, yo_r, ADD)
            nc.vector.tensor_tensor(agg_r, agg_r, dis_b, MUL)
            nc.vector.tensor_tensor(
                agg_r, agg_r, t_b2[:].unsqueeze(1).broadcast_to([P, nb, F2]), ADD
            )
            nc.sync.dma_start(out=out_o[:], in_=t_agg[:])
    legalize_waits(nc)
    return nc


# --------------------------------------------------------------------------
def kernel(x, edge_index, edge_weight, W1, b1, W2, b2):
    x = np.asarray(x, np.float32)
    ei = np.asarray(edge_index)
    ew = np.asarray(edge_weight, np.float32)
    W1 = np.asarray(W1, np.float32)
    b1 = np.asarray(b1, np.float32)
    W2 = np.asarray(W2, np.float32)
    b2 = np.asarray(b2, np.float32)

    PROGRAM_TIMES_NS.clear()
    st = build_structs(ei[0], ei[1], ew)
    nb = st.ntot // P
    RB = st.rtot // P
    tpp, tpb = tab_place(st)

    core_idx = []
    for c in range(S):
        cs = st.cores[c]
        gp, gfree, GF = gather_flat_index(st, cs, F1)
        sp, sfree, SF = scatter_flat_index(st, cs, F1)
        gp2, gfree2, GF2 = gather_flat_index(st, cs, F2)
        sp2, sfree2, SF2 = scatter_flat_index(st, cs, F2)
        _, sfree1, SF1 = scatter_flat_index(st, cs, 1)
        core_idx.append(
            (cs, gp, gfree, GF, sp, sfree, SF, gp2, gfree2, GF2, sp2, sfree2, SF2, sfree1, SF1)
        )

    # ---------------- P_A ----------------
    nc = build_PA(st)
    in_maps = []
    for c in range(S):
        cs = core_idx[c][0]
        sp = core_idx[c][4]
        sfree1 = core_idx[c][13]
        SF1 = core_idx[c][14]
        ews = np.zeros((P, SF1), np.float32)
        ews[sp, sfree1] = cs.eew
        in_maps.append({"ews": ews})
    res = _run(nc, in_maps, "PA_deg")
    dis_shard = [res[c]["dis"] for c in range(S)]

    dis_can = np.zeros(N, np.float32)
    owns = []
    for c in range(S):
        cs = core_idx[c][0]
        pown, bown = own_perm(st, cs)
        owns.append((pown, bown))
        dis_can[c * NS + np.arange(NS)] = dis_shard[c][pown, bown]

    # ---------------- P_B (expand L1) ----------------
    nc = build_expand(st, F1, scale_dis=True)
    in_maps = []
    for c in range(S):
        cs = core_idx[c][0]
        gp, gfree = core_idx[c][1], core_idx[c][2]
        x_tab = np.zeros((P, RB, F1), np.float32)
        dis_tab = np.zeros((P, RB), np.float32)
        valid = cs.tabrows >= 0
        rr = cs.tabrows[valid]
        x_tab[tpp[valid], tpb[valid], :3] = x[rr]
        dis_tab[tpp[valid], tpb[valid]] = dis_can[rr]
        EWT = sum((int(st.tm[m]) // P) * m for m in range(1, st.mmax + 1))
        ewg = np.zeros((P, EWT), np.float32)
        # ew slot (no feature axis): per class base/  b*m + occ
        base_w = np.zeros(st.mmax + 1, np.int64)
        accw = 0
        for m in range(1, st.mmax + 1):
            base_w[m] = accw
            accw += (int(st.tm[m]) // P) * m
        mm = cs.g_m
        q_local = cs.g_tabpos - st.base_tab[mm]
        tbm = st.tm[mm] // P
        wfree = base_w[mm] + (q_local % tbm) * mm + cs.g_occ
        ewg[q_local // tbm, wfree] = cs.eew
        in_maps.append(
            {
                "x_tab": x_tab.reshape(P, RB * F1),
                "dis_tab": dis_tab,
                "ewg": ewg,
            }
        )
        core_idx[c] = core_idx[c] + (wfree, ewg)
    res = _run(nc, in_maps, "PB_expand1")
    msgs_g = [res[c]["msgs"] for c in range(S)]

    # ---------------- host route L1 ----------------
    nc = build_PC(st)
    w1b = np.zeros((P, F1 * 16), np.float32)
    w1b[:, : 3 * 16] = np.broadcast_to(W1.reshape(1, 48), (P, 48))
    b1b = np.broadcast_to(b1.reshape(1, 16), (P, 16)).copy()
    w2b = np.broadcast_to(W2.reshape(1, 112), (P, 112)).copy()
    in_maps = []
    for c in range(S):
        cs = core_idx[c][0]
        gp, gfree = core_idx[c][1], core_idx[c][2]
        sp, sfree, SF = core_idx[c][4], core_idx[c][5], core_idx[c][6]
        msgs_s = np.zeros((P, SF), np.float32)
        jL = 8 * cs.jcls[cs.ecol]
        for fi in range(F1):
            msgs_s[sp, sfree + fi * jL] = msgs_g[c][gp, gfree + fi * cs.g_m]
        pown, bown = owns[c]
        x_own = np.zeros((P, nb, F1), np.float32)
        x_own[pown, bown, :3] = x[c * NS + np.arange(NS)]
        dis_own = np.zeros((P, nb), np.float32)
        dis_own[pown, bown] = dis_can[c * NS + np.arange(NS)]
        in_maps.append(
            {
                "msgs": msgs_s,
                "x_own": x_own.reshape(P, nb * F1),
                "dis_own": dis_own,
                "w1b": w1b,
                "b1b": b1b,
                "w2b": w2b,
            }
        )
        core_idx[c] = core_idx[c] + (dis_own,)
    res = _run(nc, in_maps, "PC_reduce1_mlp")
    ys_shard = [res[c]["ys"] for c in range(S)]

    ys_can = np.zeros((N, F2), np.float32)
    for c in range(S):
        pown, bown = owns[c]
        ys_can[c * NS + np.arange(NS)] = ys_shard[c].reshape(P, nb, F2)[pown, bown]

    # ---------------- P_D (expand L2) ----------------
    nc = build_expand(st, F2, scale_dis=False)
    in_maps = []
    for c in range(S):
        cs = core_idx[c][0]
        ewg = core_idx[c][16]
        ys_tab = np.zeros((P, RB, F2), np.float32)
        valid = cs.tabrows >= 0
        rr = cs.tabrows[valid]
        ys_tab[tpp[valid], tpb[valid]] = ys_can[rr]
        in_maps.append({"x_tab": ys_tab.reshape(P, RB * F2), "ewg": ewg})
    res = _run(nc, in_maps, "PD_expand2")
    msgs2_g = [res[c]["msgs"] for c in range(S)]

    # ---------------- host route L2 + P_E ----------------
    nc = build_PE(st)
    b2b = np.zeros((P, F2), np.float32)
    b2b[:, :] = b2
    in_maps = []
    for c in range(S):
        cs = core_idx[c][0]
        gp2, gfree2 = core_idx[c][7], core_idx[c][8]
        sp2, sfree2, SF2 = core_idx[c][10], core_idx[c][11], core_idx[c][12]
        msgs2_s = np.zeros((P, SF2), np.float32)
        jL = 8 * cs.jcls[cs.ecol]
        for fi in range(F2):
            msgs2_s[sp2, sfree2 + fi * jL] = msgs2_g[c][gp2, gfree2 + fi * cs.g_m]
        pown, bown = owns[c]
        ys_own = np.zeros((P, nb, F2), np.float32)
        ys_own[pown, bown] = ys_can[c * NS + np.arange(NS)]
        dis_own = core_idx[c][17]
        in_maps.append(
            {
                "msgs": msgs2_s,
                "ys_own": ys_own.reshape(P, nb * F2),
                "dis_own": dis_own,
                "b2b": b2b,
            }
        )
    res = _run(nc, in_maps, "PE_reduce2")

    out = np.zeros((N, 7), np.float32)
    for c in range(S):
        o = res[c]["out"].reshape(P, nb, F2)
        pown, bown = owns[c]
        out[c * NS + np.arange(NS)] = o[pown, bown, :7]
    return out



# revision 2
# speedup vs baseline: 1.0993x; 1.0993x over previous
"""GCN (2-layer) on 8 Trainium2 NeuronCores — v4 (3 device programs).

Graph/data parallel per the node-range sharding hint: nodes sharded by
range, edges live on the destination core, weights replicated.  All
irregular routing happens on the HOST as pure copies/permutations;
every FP arithmetic op on values runs on device.

- Destination nodes bucketed by in-degree class j=ceil(d/4); each node
  gets exactly 4j slots so segment-sum becomes a slot-axis reduction.
- Big chunks use a SLOT-MAJOR [l, f, b] layout: the ew multiply and a
  tree of tensor_tensor adds are then fully contiguous fp16 APs, which
  is what the DVE 2x packed mode requires on hardware.  Small chunks
  stay node-major with one 1x tensor_reduce (fewer instructions).
- Node values feeding edges are pre-scaled on device (x'=dis*x,
  ys=dis*relu(vW1+b1)W2) so the per-edge device math is one multiply.
- PB's whole MLP runs on the Tensor engine: v is transposed via the PE,
  W1 is applied as a block-diagonal [32,128] stationary (8 nodes per
  matmul), bias+relu ride the scalar-engine PSUM eviction, W2 likewise
  block-diagonal [32,14]; dis scaling fuses into the final eviction.
"""
import sys

sys.path.insert(0, "/opt/trn_rl_repo")

import numpy as np

import bass_rust
from concourse import bass, mybir
from concourse.bass_utils import run_bass_kernel_spmd
from concourse.masks import make_identity
import concourse.tile as tile

import os as _os

PROGRAM_TIMES_NS = []   # (name, exec_time_ns) per device program of last kernel() call


def _enable_tracing():
    import types
    import antenv
    if 'antenv.axon_hooks' in sys.modules:
        return True
    try:
        from trn_agent_boot.trn_boot import _ntff_profile_via_ctypes
        hook = _ntff_profile_via_ctypes('/opt/axon/libaxon_pjrt.so')
    except Exception:
        return False
    mod = types.ModuleType('antenv.axon_hooks')
    mod.get_axon_ntff_profile_hook = lambda: hook
    mod.set_axon_ntff_profile_hook = lambda h: None
    sys.modules['antenv.axon_hooks'] = mod
    antenv.axon_hooks = mod
    import concourse.bass_utils as _bu
    _bu.upload_artifacts = lambda tmpdir: f"local://{tmpdir}"
    return True


def _run(nc, in_maps, name):
    trace = bool(_os.environ.get('GCN_TRACE')) and _enable_tracing()
    r = run_bass_kernel_spmd(nc, in_maps, core_ids=CORE_IDS, trace=trace)
    if trace:
        PROGRAM_TIMES_NS.append((name, r.exec_time_ns))
    return r.results

S = 8
N = 200000
NS = N // S
P = 128
G = 4            # degree-class granularity: slots per node = G*ceil(d/G)
F1 = 3
F2 = 7
CORE_IDS = list(range(S))
FP = mybir.dt.float32
F16 = mybir.dt.float16
MUL = mybir.AluOpType.mult
ADD = mybir.AluOpType.add
AX = mybir.AxisListType.X
RELU = mybir.ActivationFunctionType.Relu

_CHUNK7 = 12288   # chunk budget in F2-elems per partition
_SM_MIN = 1000    # below this (F2-elems), node-major + tensor_reduce


def _ceil(a, b):
    return -(-a // b)


# --------------------------------------------------------------------------
# walrus on this toolchain accepts at most ONE sync-wait per instruction;
# Tile emits several at DAG joins / kernel-tail drain. Hoist excess waits
# onto fresh same-engine NoOps inserted immediately before the violator.
def legalize_waits(nc):
    nop_idx = 0
    for f in nc.m.functions:
        for bb in f.blocks:
            il = bb.instructions
            if not any(
                inst.sync_info is not None
                and len(inst.sync_info.on_wait or []) > 1
                for inst in il
            ):
                continue
            new_il = []
            for inst in il:
                si = inst.sync_info
                w = list(si.on_wait or []) if si is not None else []
                if len(w) > 1:
                    for extra in w[:-1]:
                        nop = mybir.InstNoOp(
                            name=f"I-waitsplit-{nop_idx}", ins=[], outs=[]
                        )
                        nop_idx += 1
                        nop.engine = inst.engine
                        nop.sync_info = bass_rust.SyncInfo(
                            on_wait=[extra], on_update=[]
                        )
                        new_il.append(nop)
                    si.on_wait = [w[-1]]
                new_il.append(inst)
            bb.instructions = new_il


# --------------------------------------------------------------------------
# host-side structure building (integer routing only)
class _O:
    pass


def build_structs(row, col, ew):
    row = row.astype(np.int64)
    col = col.astype(np.int64)
    cores = []
    for c in range(S):
        cs = _O()
        m = (col // NS) == c
        cs.erow = row[m]
        cs.ecol = (col[m] - c * NS).astype(np.int64)
        cs.eew = ew[m].astype(np.float32)
        cores.append(cs)

    for cs in cores:
        d = np.bincount(cs.ecol, minlength=NS)
        cs.jcls = np.maximum(1, _ceil(np.maximum(d, 1), G))
    jmax = max(int(cs.jcls.max()) for cs in cores)
    nj = np.zeros(jmax + 1, np.int64)
    for j in range(1, jmax + 1):
        njc = max(int((cs.jcls == j).sum()) for cs in cores)
        nj[j] = _ceil(max(njc, 1), P) * P
    for cs in cores:
        nodepos = np.full(NS, -1, np.int64)
        pos = 0
        for j in range(1, jmax + 1):
            nodes = np.nonzero(cs.jcls == j)[0]
            nodepos[nodes] = pos + np.arange(len(nodes))
            pos += nj[j]
        cs.nodepos = nodepos
    ntot = int(nj[1:].sum())

    st = _O()
    st.jmax, st.nj, st.ntot = jmax, nj, ntot
    st.cores = cores

    # per-edge occurrence index among edges sharing a destination
    for cs in cores:
        nodes = cs.ecol
        ord_ = np.argsort(nodes, kind="stable")
        ns = nodes[ord_]
        first = np.r_[True, ns[1:] != ns[:-1]]
        idx_of_first = np.maximum.accumulate(
            np.where(first, np.arange(len(ord_)), 0)
        )
        k = np.empty(len(nodes), np.int64)
        k[ord_] = np.arange(len(ord_)) - idx_of_first
        cs.kocc = k

    # class bases
    st.base_node = np.zeros(jmax + 2, np.int64)
    st.base_blk = np.zeros(jmax + 2, np.int64)
    for j in range(1, jmax + 1):
        st.base_node[j + 1] = st.base_node[j] + nj[j]
        st.base_blk[j + 1] = st.base_blk[j] + nj[j] // P

    # shared chunk plan (same block ranges for all three per-edge tensors);
    # carve a small GpSimd-sized chunk off the three biggest classes
    sizes = [(int(nj[j]) // P * G * j, j) for j in range(1, jmax + 1)]
    gp_classes = {j for _, j in sorted(sizes, reverse=True)[:3]}
    chunks = []
    off1 = off3 = off7 = agg7 = 0
    for j in range(1, jmax + 1):
        L = G * j
        nbj = int(nj[j]) // P
        blmax = max(2, (_CHUNK7 // (F2 * L)) & ~1)
        gp_left = 14 if (j in gp_classes and nbj >= 40) else 0
        b0 = 0
        while b0 < nbj:
            if gp_left:
                bl, gp = gp_left, True
                gp_left = 0
            else:
                bl, gp = min(blmax, nbj - b0), False
                if bl % 2 == 1 and bl > 1 and bl * F2 * L >= _SM_MIN:
                    bl -= 1   # keep the big chunk even (slot-major capable)
            sm = (bl % 2 == 0) and (gp or bl * F2 * L >= _SM_MIN)
            ck = _O()
            ck.j, ck.L, ck.b0, ck.bl, ck.sm, ck.gp = j, L, b0, bl, sm, gp
            ck.off1, ck.off3, ck.off7, ck.agg7 = off1, off3, off7, agg7
            chunks.append(ck)
            off1 += bl * L
            off3 += bl * F1 * L
            off7 += bl * F2 * L
            agg7 += bl * F2
            b0 += bl
    st.chunks = chunks
    st.SF1, st.SF3, st.SF7 = off1, off3, off7
    assert agg7 == (ntot // P) * F2

    # per-class chunk lookup tables (by block index)
    st.cmap = {}
    for j in range(1, jmax + 1):
        nbj = int(nj[j]) // P
        cid = np.zeros(nbj, np.int64)
        for i, ck in enumerate(chunks):
            if ck.j == j:
                cid[ck.b0 : ck.b0 + ck.bl] = i
        st.cmap[j] = cid
    return st


def edge_slots(st, cs):
    """per edge: partition p, and for each tensor the flat offset of the
    (slot k, feature 0) element plus the per-feature stride."""
    nodes = cs.ecol
    j = cs.jcls[nodes]
    q_local = cs.nodepos[nodes] - st.base_node[j]
    nbj = st.nj[j] // P
    p = q_local // nbj
    b = q_local % nbj
    k = cs.kocc
    ne = len(nodes)
    off1 = np.empty(ne, np.int64)
    off3 = np.empty(ne, np.int64)
    off7 = np.empty(ne, np.int64)
    fs3 = np.empty(ne, np.int64)
    fs7 = np.empty(ne, np.int64)
    for jj in range(1, st.jmax + 1):
        sel = j == jj
        if not sel.any():
            continue
        L = G * jj
        ci = st.cmap[jj][b[sel]]
        cb0 = np.array([st.chunks[i].b0 for i in range(len(st.chunks))])
        cbl = np.array([st.chunks[i].bl for i in range(len(st.chunks))])
        csm = np.array([st.chunks[i].sm for i in range(len(st.chunks))])
        c1 = np.array([st.chunks[i].off1 for i in range(len(st.chunks))])
        c3 = np.array([st.chunks[i].off3 for i in range(len(st.chunks))])
        c7 = np.array([st.chunks[i].off7 for i in range(len(st.chunks))])
        bo = b[sel] - cb0[ci]
        bl = cbl[ci]
        sm = csm[ci]
        kk = k[sel]
        # slot-major: base + k*(F*bl) + f*bl + bo ; node-major: bo*(F*L)+f*L+k
        off1[sel] = np.where(sm, c1[ci] + kk * bl + bo,
                             c1[ci] + bo * L + kk)
        off3[sel] = np.where(sm, c3[ci] + kk * (F1 * bl) + bo,
                             c3[ci] + bo * (F1 * L) + kk)
        off7[sel] = np.where(sm, c7[ci] + kk * (F2 * bl) + bo,
                             c7[ci] + bo * (F2 * L) + kk)
        fs3[sel] = np.where(sm, bl, L)
        fs7[sel] = np.where(sm, bl, L)
    return p, off1, off3, fs3, off7, fs7


def node_slots(st, cs):
    """per local node: partition p, global block gb, and the (base, fstride)
    of its features in the chunk-major agg7 layout."""
    j = cs.jcls
    q_local = cs.nodepos - st.base_node[j]
    nbj = st.nj[j] // P
    p = q_local // nbj
    b = q_local % nbj
    gb = st.base_blk[j] + b
    ns = len(j)
    base = np.empty(ns, np.int64)
    fstr = np.empty(ns, np.int64)
    cb0 = np.array([c.b0 for c in st.chunks])
    cbl = np.array([c.bl for c in st.chunks])
    csm = np.array([c.sm for c in st.chunks])
    ca7 = np.array([c.agg7 for c in st.chunks])
    for jj in range(1, st.jmax + 1):
        sel = j == jj
        if not sel.any():
            continue
        ci = st.cmap[jj][b[sel]]
        bo = b[sel] - cb0[ci]
        base[sel] = np.where(csm[ci], ca7[ci] + bo,
                             ca7[ci] + bo * F2)
        fstr[sel] = np.where(csm[ci], cbl[ci], 1)
    return p, gb, base, fstr


# --------------------------------------------------------------------------
# device-program helpers
def _fold_flat(nc, eng, t_in, FB, L, out_ap, fshape=None):
    """tree-fold the leading slot axis of a slot-major chunk (viewed as
    [P, L*FB] with l outermost): contiguous halves, all 2x; the final add
    writes out_ap (free size FB; if fshape=(F, bl) the inputs are viewed
    [P, F, bl] to match a shaped/strided out_ap)."""
    tv = t_in[:]
    cur = L
    while cur > 2:
        if cur % 2 == 1:
            eng.tensor_tensor(
                tv[:, 0:FB], tv[:, 0:FB],
                tv[:, (cur - 1) * FB : cur * FB], ADD,
            )
            cur -= 1
            if cur == 2:
                break
        h = cur // 2
        eng.tensor_tensor(
            tv[:, 0 : h * FB], tv[:, 0 : h * FB],
            tv[:, h * FB : cur * FB], ADD,
        )
        cur = h
    i0, i1 = tv[:, 0:FB], tv[:, FB : 2 * FB]
    if fshape is not None:
        F, bl = fshape
        i0 = i0.rearrange("p (f b) -> p f b", f=F)
        i1 = i1.rearrange("p (f b) -> p f b", f=F)
    eng.tensor_tensor(out_ap, i0, i1, ADD)


def _edge_stream(nc, pool, st, F, xs, t_ew, agg_of, tag, shaped_fold=False,
                 use_gp=True, subset=None):
    """Stream per-edge fp16 payload chunks, multiply by the ew slots and
    reduce the slot axis.  agg_of(ck) -> output AP (free size bl*F) in the
    chosen agg layout for that chunk.  Chunks marked gp run on GpSimd."""
    for ck in (subset if subset is not None else st.chunks):
        L, bl = ck.L, ck.bl
        eng = nc.gpsimd if (ck.gp and use_gp) else nc.vector
        offF = ck.off1 if F == 1 else (ck.off3 if F == F1 else ck.off7)
        n = bl * F * L
        t_in = pool.tile([P, n], F16, tag=tag + ("g" if ck.gp else ""))
        nc.sync.dma_start(out=t_in[:], in_=xs[:, offF : offF + n])
        if ck.sm:
            FB = F * bl
            if t_ew is not None:
                eng.tensor_tensor(
                    t_in[:].rearrange("p (l f b) -> p l f b", l=L, f=F),
                    t_in[:].rearrange("p (l f b) -> p l f b", l=L, f=F),
                    t_ew[:, ck.off1 : ck.off1 + bl * L]
                    .rearrange("p (l b) -> p l b", l=L)
                    .unsqueeze(2)
                    .broadcast_to([P, L, F, bl]),
                    MUL,
                )
            _fold_flat(nc, eng, t_in, FB, L, agg_of(ck),
                       fshape=(F, bl) if shaped_fold else None)
        else:
            if t_ew is not None:
                eng.tensor_tensor(
                    t_in[:].rearrange("p (b f l) -> p b f l", f=F, l=L),
                    t_in[:].rearrange("p (b f l) -> p b f l", f=F, l=L),
                    t_ew[:, ck.off1 : ck.off1 + bl * L]
                    .rearrange("p (b l) -> p b l", l=L)
                    .unsqueeze(2)
                    .broadcast_to([P, bl, F, L]),
                    MUL,
                )
            with nc.allow_low_precision(reason="fp16 agg within tolerance"):
                eng.tensor_reduce(
                    out=agg_of(ck),
                    in_=t_in[:].rearrange("p (b f l) -> p b f l", f=F, l=L),
                    axis=AX,
                    op=ADD,
                )


def build_PA(st):
    """ews_pa (node-major, per class) + x_own -> dis (fp16), xp = dis*x."""
    nc = bass.Bass("TRN2", num_devices=S)
    nb = st.ntot // P
    SFA = sum((int(st.nj[j]) // P) * G * j for j in range(1, st.jmax + 1))
    ews = nc.dram_tensor("ews_pa", (P, SFA), F16, kind="ExternalInput")
    x_own = nc.dram_tensor("x_own", (P, nb * F1), F16, kind="ExternalInput")
    dis_o = nc.dram_tensor("dis", (P, nb), F16, kind="ExternalOutput")
    xp_o = nc.dram_tensor("xp", (P, nb * F1), F16, kind="ExternalOutput")
    with tile.TileContext(nc) as tc:
        with tc.tile_pool(name="acc", bufs=1) as apool:
            # whole ews resident; two DMA pieces so reduces start early
            t_ews = apool.tile([P, SFA], F16)
            half = 0
            accf = 0
            for j in range(1, st.jmax + 1):
                if accf >= SFA // 2:
                    half = accf
                    break
                accf += (int(st.nj[j]) // P) * G * j
            if not half:
                half = SFA
            nc.sync.dma_start(out=t_ews[:, :half], in_=ews[:, :half])
            if half < SFA:
                nc.sync.dma_start(out=t_ews[:, half:], in_=ews[:, half:])
            t_xo = apool.tile([P, nb * F1], F16)
            nc.sync.dma_start(out=t_xo[:], in_=x_own[:])
            t_deg = apool.tile([P, nb], F16)
            accf = 0
            accb = 0
            for j in range(1, st.jmax + 1):
                L = G * j
                nbj = int(st.nj[j]) // P
                with nc.allow_low_precision(
                    reason="fp16 deg within tolerance"
                ):
                    nc.vector.tensor_reduce(
                        out=t_deg[:, accb : accb + nbj],
                        in_=t_ews[:, accf : accf + nbj * L].rearrange(
                            "p (b l) -> p b l", l=L
                        ),
                        axis=AX,
                        op=ADD,
                    )
                accf += nbj * L
                accb += nbj
            t_d32 = apool.tile([P, nb], FP)
            nc.scalar.copy(t_d32[:], t_deg[:])
            nc.vector.tensor_scalar_add(t_d32[:], t_d32[:], 1.0)
            t_sq = apool.tile([P, nb], FP)
            nc.scalar.sqrt(t_sq[:], t_d32[:])
            t_r = apool.tile([P, nb], FP)
            nc.vector.reciprocal(t_r[:], t_sq[:])
            t_d16 = apool.tile([P, nb], F16)
            nc.scalar.copy(t_d16[:], t_r[:])
            nc.sync.dma_start(out=dis_o[:], in_=t_d16[:])
            t_xp = apool.tile([P, nb * F1], F16)
            nc.vector.tensor_tensor(
                t_xp[:].rearrange("p (b f) -> p b f", f=F1),
                t_xo[:].rearrange("p (b f) -> p b f", f=F1),
                t_d16[:].unsqueeze(2).broadcast_to([P, nb, F1]),
                MUL,
            )
            nc.sync.dma_start(out=xp_o[:], in_=t_xp[:])
    legalize_waits(nc)
    return nc


def _pb_geom(st):
    nb = st.ntot // P
    nbp = _ceil(nb, 128) * 128   # v4 node blocks padded (32 nodes/tile, 4-col)
    ntiles = nbp // 32           # 128-col transpose tiles of 4-stride v
    ncc = ntiles // 4            # 512-col W1 column chunks
    ne = ncc * 4                 # W1 matmuls / ht 512-col chunks
    nw2 = ne * 4                 # W2 matmuls
    npy = ncc * 6                # y PSUM tiles (3 outs at 0/32/64, per cc)
    return nb, nbp, ntiles, ncc, ne, nw2, npy


def build_PB(st):
    """xs + ews + xp_own + dis -> yst: packed dis*relu(vW1+b1)@W2 (fp16).

    v is kept 4-strided per node [b*4+f]; PE transposes 128-col tiles; W1
    is a block-diagonal [32,128] stationary (8 nodes per matmul, K row
    groups at partition bases 0/32/64 and a widened K=64 for the base-96
    group); bias+relu fuse into the scalar-engine PSUM eviction; W2 is
    block-diagonal [32,14]; dis scaling fuses into the final eviction."""
    nc = bass.Bass("TRN2", num_devices=S)
    nb, nbp, ntiles, ncc, ne, nw2, npy = _pb_geom(st)
    ews = nc.dram_tensor("ews", (P, st.SF1), F16, kind="ExternalInput")
    xs = nc.dram_tensor("xs", (P, st.SF3), F16, kind="ExternalInput")
    xpo4 = nc.dram_tensor("xpo4", (P, nbp * 4), F16, kind="ExternalInput")
    dis4 = nc.dram_tensor("dis4", (P, nbp * 4), F16, kind="ExternalInput")
    w1bd = nc.dram_tensor("w1bd", (P, 128), F16, kind="ExternalInput")
    w1bdh = nc.dram_tensor("w1bdh", (P, 128), F16, kind="ExternalInput")
    w2bd = nc.dram_tensor("w2bd", (P, 14), F16, kind="ExternalInput")
    w2bdh = nc.dram_tensor("w2bdh", (P, 14), F16, kind="ExternalInput")
    b1v = nc.dram_tensor("b1v", (P, 1), FP, kind="ExternalInput")
    disr = nc.dram_tensor("disr", (P, npy * 512), F16, kind="ExternalInput")
    yst_o = nc.dram_tensor("yst", (P, npy * 512), F16, kind="ExternalOutput")
    with tile.TileContext(nc) as tc:
        with tc.tile_pool(name="sb", bufs=3) as pool, tc.tile_pool(
            name="acc", bufs=1
        ) as apool, tc.tile_pool(
            name="ptr", bufs=2, space="PSUM"
        ) as tpool, tc.tile_pool(
            name="ph", bufs=2, space="PSUM"
        ) as hpool, tc.tile_pool(
            name="py", bufs=2, space="PSUM"
        ) as ypool:
            t_v4 = apool.tile([P, nbp * 4], F16)
            nc.gpsimd.memset(t_v4[:], 0.0)
            t_id = apool.tile([P, P], F16)
            make_identity(nc, t_id[:])
            t_ew = apool.tile([P, st.SF1], F16)
            nc.sync.dma_start(out=t_ew[:], in_=ews[:])
            t_xpo = apool.tile([P, nbp * 4], F16)
            nc.sync.dma_start(out=t_xpo[:], in_=xpo4[:])
            t_dis = apool.tile([P, nbp * 4], F16)
            nc.sync.dma_start(out=t_dis[:], in_=dis4[:])

            def agg_of(ck):
                gb0 = int(st.base_blk[ck.j]) + ck.b0
                view = t_v4[:, gb0 * 4 : (gb0 + ck.bl) * 4].rearrange(
                    "p (b f) -> p b f", f=4
                )[:, :, 0:F1]
                if ck.sm:
                    # fold's final add iterates (f, b)
                    return view.rearrange("p b f -> p f b")
                return view

            t_vt = apool.tile([P, ntiles * 128], F16)
            t_ht = apool.tile([P, ne * 512], F16)
            t_yst = apool.tile([P, npy * 512], F16)
            t_w1 = apool.tile([P, 128], F16)
            t_w1h = apool.tile([P, 128], F16)
            t_w2 = apool.tile([P, 14], F16)
            t_w2h = apool.tile([P, 14], F16)
            t_b1v = apool.tile([P, 1], FP)
            t_disr = apool.tile([P, npy * 512], F16)

            def mlp_part(cc):
                # v-prep for this 512-col slice, then the PE pipeline
                sl = slice(cc * 512, (cc + 1) * 512)
                nc.vector.tensor_tensor(
                    t_v4[:, sl], t_v4[:, sl], t_xpo[:, sl], ADD
                )
                nc.vector.tensor_tensor(
                    t_v4[:, sl], t_v4[:, sl], t_dis[:, sl], MUL
                )
                t_tr = tpool.tile([P, 512], F16, tag="tr")
                for gi in range(4):
                    tt = cc * 4 + gi
                    nc.tensor.transpose(
                        t_tr[:, gi * 128 : (gi + 1) * 128],
                        t_v4[:, tt * 128 : (tt + 1) * 128],
                        t_id[:],
                    )
                nc.scalar.copy(t_vt[:, sl], t_tr[:])
                for Q in range(4):
                    e = cc * 4 + Q
                    t_hp = hpool.tile([P, 512], FP, tag="h")
                    if Q < 3:
                        nc.tensor.matmul(
                            t_hp[:, :],
                            t_w1[32 * Q : 32 * Q + 32, :],
                            t_vt[32 * Q : 32 * Q + 32, sl],
                        )
                    else:
                        nc.tensor.matmul(
                            t_hp[:, :],
                            t_w1h[64:128, :],
                            t_vt[64:128, sl],
                        )
                    nc.scalar.activation(
                        t_ht[:, e * 512 : (e + 1) * 512],
                        t_hp[:, :],
                        RELU,
                        bias=t_b1v[:, 0:1],
                    )
                t_y = None
                for Q in range(4):
                    e = cc * 4 + Q
                    for R in range(4):
                        wl = Q * 4 + R
                        s = wl % 3
                        if s == 0:
                            t_y = ypool.tile([P, 512], FP, tag="y")
                        if R < 3:
                            nc.tensor.matmul(
                                t_y[32 * s : 32 * s + 14, :],
                                t_w2[32 * R : 32 * R + 32, :],
                                t_ht[32 * R : 32 * R + 32,
                                     e * 512 : (e + 1) * 512],
                            )
                        else:
                            nc.tensor.matmul(
                                t_y[32 * s : 32 * s + 14, :],
                                t_w2h[64:128, :],
                                t_ht[64:128, e * 512 : (e + 1) * 512],
                            )
                        if s == 2 or wl == 15:
                            pt = cc * 6 + wl // 3
                            nc.vector.tensor_tensor(
                                t_yst[:, pt * 512 : (pt + 1) * 512],
                                t_y[:, :],
                                t_disr[:, pt * 512 : (pt + 1) * 512],
                                MUL,
                            )

            # interleave: edge chunks needed by column-chunk cc, then its
            # MLP part, so the Tensor/Scalar pipeline overlaps the stream
            gs = [int(st.base_blk[ck.j]) + ck.b0 for ck in st.chunks]
            done = 0
            for cc in range(ncc):
                need = 128 * (cc + 1)
                hi = len(st.chunks)
                if cc < ncc - 1:
                    hi = next(
                        (i for i, g in enumerate(gs) if g >= need),
                        len(st.chunks),
                    )
                _edge_stream(nc, pool, st, F1, xs, t_ew, agg_of, "x",
                             shaped_fold=True,
                             subset=st.chunks[done:hi])
                if cc == 0:
                    # weights arrive while the first folds run
                    nc.sync.dma_start(out=t_w1[:], in_=w1bd[:])
                    nc.sync.dma_start(out=t_w1h[:], in_=w1bdh[:])
                    nc.sync.dma_start(out=t_w2[:], in_=w2bd[:])
                    nc.sync.dma_start(out=t_w2h[:], in_=w2bdh[:])
                    nc.sync.dma_start(out=t_b1v[:], in_=b1v[:])
                    nc.sync.dma_start(out=t_disr[:], in_=disr[:])
                done = hi
                mlp_part(cc)
            nc.sync.dma_start(out=yst_o[:], in_=t_yst[:])
    legalize_waits(nc)
    return nc


def build_PE(st):
    """yss + ews + yso + dis7 + b2e -> out = dis*(agg2 + ys_own) + b2,
    everything in the chunk-major agg layout (host unscrambles)."""
    nc = bass.Bass("TRN2", num_devices=S)
    nb = st.ntot // P
    ews = nc.dram_tensor("ews", (P, st.SF1), F16, kind="ExternalInput")
    yss = nc.dram_tensor("yss", (P, st.SF7), F16, kind="ExternalInput")
    yso = nc.dram_tensor("yso", (P, nb * F2), F16, kind="ExternalInput")
    dis7 = nc.dram_tensor("dis7", (P, nb * F2), F16, kind="ExternalInput")
    b2e = nc.dram_tensor("b2e", (P, nb * F2), F16, kind="ExternalInput")
    out_o = nc.dram_tensor("out", (P, nb * F2), F16, kind="ExternalOutput")
    with tile.TileContext(nc) as tc:
        with tc.tile_pool(name="sb", bufs=3) as pool, tc.tile_pool(
            name="acc", bufs=1
        ) as apool:
            t_ew = apool.tile([P, st.SF1], F16)
            nc.sync.dma_start(out=t_ew[:], in_=ews[:])
            t_yso = apool.tile([P, nb * F2], F16)
            t_dis7 = apool.tile([P, nb * F2], F16)
            t_b2e = apool.tile([P, nb * F2], F16)

            t_agg = apool.tile([P, nb * F2], F16)

            def agg_of(ck):
                sl = t_agg[:, ck.agg7 : ck.agg7 + ck.bl * F2]
                if ck.sm:
                    return sl
                return sl.rearrange("p (b f) -> p b f", f=F2)

            _edge_stream(nc, pool, st, F2, yss, t_ew, agg_of, "y",
                         subset=st.chunks[:4])
            # node-level operands load while the stream runs
            nc.sync.dma_start(out=t_yso[:], in_=yso[:])
            nc.sync.dma_start(out=t_dis7[:], in_=dis7[:])
            nc.sync.dma_start(out=t_b2e[:], in_=b2e[:])
            _edge_stream(nc, pool, st, F2, yss, t_ew, agg_of, "y",
                         subset=st.chunks[4:])

            nc.vector.tensor_tensor(t_agg[:], t_agg[:], t_yso[:], ADD)
            nc.vector.tensor_tensor(t_agg[:], t_agg[:], t_dis7[:], MUL)
            nc.vector.tensor_tensor(t_agg[:], t_agg[:], t_b2e[:], ADD)
            nc.sync.dma_start(out=out_o[:], in_=t_agg[:])
    legalize_waits(nc)
    return nc


# --------------------------------------------------------------------------
def kernel(x, edge_index, edge_weight, W1, b1, W2, b2):
    x = np.asarray(x, np.float32)
    ei = np.asarray(edge_index)
    ew = np.asarray(edge_weight, np.float32)
    W1 = np.asarray(W1, np.float32)
    b1 = np.asarray(b1, np.float32)
    W2 = np.asarray(W2, np.float32)
    b2 = np.asarray(b2, np.float32)

    PROGRAM_TIMES_NS.clear()
    st = build_structs(ei[0], ei[1], ew)
    nb = st.ntot // P
    _, nbp, ntiles, ncc, ne, nw2, npy = _pb_geom(st)
    ar = np.arange(NS)

    core_idx = []
    for c in range(S):
        cs = st.cores[c]
        p_e, off1, off3, fs3, off7, fs7 = edge_slots(st, cs)
        p_n, gb, nbase, nfstr = node_slots(st, cs)
        core_idx.append((cs, p_e, off1, off3, fs3, off7, fs7,
                         p_n, gb, nbase, nfstr))

    # node (p,gb) -> (row base before feature, column) in packed yst
    def yst_pos(p_n, gb):
        tc_ = gb // 32
        r32 = gb % 32
        Q = r32 // 8
        g = r32 % 8
        cc = tc_ // 4
        ci = (tc_ % 4) * 128 + p_n
        R = g // 2
        gp = g % 2
        wl = Q * 4 + R
        return 32 * (wl % 3) + 7 * gp, (cc * 6 + wl // 3) * 512 + ci

    # ---------------- P_A ----------------
    nc = build_PA(st)
    SFA = sum((int(st.nj[j]) // P) * G * j for j in range(1, st.jmax + 1))
    in_maps = []
    for c in range(S):
        cs = st.cores[c]
        p_e = core_idx[c][1]
        p_n, gb = core_idx[c][7], core_idx[c][8]
        # node-major per-class layout for PA's degree reduce
        j = cs.jcls[cs.ecol]
        q_local = cs.nodepos[cs.ecol] - st.base_node[j]
        nbj = st.nj[j] // P
        b = q_local % nbj
        base_free = np.zeros(st.jmax + 2, np.int64)
        for jj in range(1, st.jmax + 1):
            base_free[jj + 1] = base_free[jj] + (st.nj[jj] // P) * G * jj
        offa = base_free[j] + b * (G * j) + cs.kocc
        ews_pa = np.zeros((P, SFA), np.float16)
        ews_pa[p_e, offa] = cs.eew.astype(np.float16)
        x_own = np.zeros((P, nb, F1), np.float16)
        x_own[p_n, gb] = x[c * NS + ar].astype(np.float16)
        in_maps.append({"ews_pa": ews_pa, "x_own": x_own.reshape(P, nb * F1)})
    res = _run(nc, in_maps, "PA_deg")
    dis_l = [res[c]["dis"] for c in range(S)]
    xp_l = [res[c]["xp"] for c in range(S)]

    xp_can = np.zeros((N, F1), np.float16)
    dis_can = np.zeros(N, np.float16)
    for c in range(S):
        p_n, gb = core_idx[c][7], core_idx[c][8]
        xp_can[c * NS + ar] = xp_l[c].reshape(P, nb, F1)[p_n, gb]
        dis_can[c * NS + ar] = dis_l[c][p_n, gb]

    # ---------------- P_B (layer 1 + MLP) ----------------
    nc = build_PB(st)
    W1h = W1.astype(np.float16)
    W2h = W2.astype(np.float16)
    w1bdb = np.zeros((P, 128), np.float16)
    for Q in range(3):
        for gl in range(8):
            w1bdb[32 * Q + 4 * gl : 32 * Q + 4 * gl + 3,
                  16 * gl : 16 * gl + 16] = W1h
    w1bdhb = np.zeros((P, 128), np.float16)
    for gl in range(8):
        w1bdhb[96 + 4 * gl : 96 + 4 * gl + 3, 16 * gl : 16 * gl + 16] = W1h
    w2bdb = np.zeros((P, 14), np.float16)
    for R in range(3):
        for gp in range(2):
            w2bdb[32 * R + 16 * gp : 32 * R + 16 * gp + 16,
                  7 * gp : 7 * gp + 7] = W2h
    w2bdhb = np.zeros((P, 14), np.float16)
    for gp in range(2):
        w2bdhb[96 + 16 * gp : 96 + 16 * gp + 16, 7 * gp : 7 * gp + 7] = W2h
    b1vb = b1.astype(np.float32)[np.arange(P) % 16].reshape(P, 1)

    in_maps = []
    ews_l = []
    for c in range(S):
        cs = st.cores[c]
        p_e, off1, off3, fs3 = (core_idx[c][1], core_idx[c][2],
                                core_idx[c][3], core_idx[c][4])
        p_n, gb = core_idx[c][7], core_idx[c][8]
        ews = np.zeros((P, st.SF1), np.float16)
        ews[p_e, off1] = cs.eew.astype(np.float16)
        ews_l.append(ews)
        xs = np.zeros((P, st.SF3), np.float16)
        for fi in range(F1):
            xs[p_e, off3 + fi * fs3] = xp_can[cs.erow, fi]
        xpo4 = np.zeros((P, nbp, 4), np.float16)
        xpo4[p_n, gb, 0:F1] = xp_l[c].reshape(P, nb, F1)[p_n, gb]
        dis4 = np.zeros((P, nbp, 4), np.float16)
        dis4[p_n, gb, 0:F1] = dis_l[c][p_n, gb][:, None]
        rbase, col_ = yst_pos(p_n, gb)
        disr = np.zeros((P, npy * 512), np.float16)
        for r in range(F2):
            disr[rbase + r, col_] = dis_l[c][p_n, gb]
        in_maps.append(
            {
                "ews": ews,
                "xs": xs,
                "xpo4": xpo4.reshape(P, nbp * 4),
                "dis4": dis4.reshape(P, nbp * 4),
                "w1bd": w1bdb,
                "w1bdh": w1bdhb,
                "w2bd": w2bdb,
                "w2bdh": w2bdhb,
                "b1v": b1vb,
                "disr": disr,
            }
        )
    res = _run(nc, in_maps, "PB_layer1")
    yst_l = [res[c]["yst"] for c in range(S)]

    ys_can = np.zeros((N, F2), np.float16)
    for c in range(S):
        p_n, gb = core_idx[c][7], core_idx[c][8]
        rbase, col_ = yst_pos(p_n, gb)
        v = np.empty((NS, F2), np.float16)
        for r in range(F2):
            v[:, r] = yst_l[c][rbase + r, col_]
        ys_can[c * NS + ar] = v

    # ---------------- P_E (layer 2) ----------------
    nc = build_PE(st)
    b2h = b2.astype(np.float16)
    in_maps = []
    for c in range(S):
        cs = st.cores[c]
        p_e, off7, fs7 = core_idx[c][1], core_idx[c][5], core_idx[c][6]
        p_n, nbase, nfstr = core_idx[c][7], core_idx[c][9], core_idx[c][10]
        yss = np.zeros((P, st.SF7), np.float16)
        for fi in range(F2):
            yss[p_e, off7 + fi * fs7] = ys_can[cs.erow, fi]
        yso = np.zeros((P, nb * F2), np.float16)
        dis7 = np.zeros((P, nb * F2), np.float16)
        b2e = np.zeros((P, nb * F2), np.float16)
        ysl = ys_can[c * NS + ar]
        disl = dis_can[c * NS + ar]
        for fi in range(F2):
            yso[p_n, nbase + fi * nfstr] = ysl[:, fi]
            dis7[p_n, nbase + fi * nfstr] = disl
            b2e[p_n, nbase + fi * nfstr] = b2h[fi]
        in_maps.append(
            {
                "ews": ews_l[c],
                "yss": yss,
                "yso": yso,
                "dis7": dis7,
                "b2e": b2e,
            }
        )
    res = _run(nc, in_maps, "PE_layer2")

    out = np.zeros((N, F2), np.float32)
    for c in range(S):
        o = res[c]["out"]
        p_n, nbase, nfstr = core_idx[c][7], core_idx[c][9], core_idx[c][10]
        for fi in range(F2):
            out[c * NS + ar, fi] = o[p_n, nbase + fi * nfstr].astype(
                np.float32
            )
    return out


# revision 3
# speedup vs baseline: 1.1170x; 1.0161x over previous
"""GCN (2-layer) on 8 Trainium2 NeuronCores — v4 (3 device programs).

Graph/data parallel per the node-range sharding hint: nodes sharded by
range, edges live on the destination core, weights replicated.  All
irregular routing happens on the HOST as pure copies/permutations;
every FP arithmetic op on values runs on device.

- Destination nodes bucketed by in-degree class j=ceil(d/4); each node
  gets exactly 4j slots so segment-sum becomes a slot-axis reduction.
- Big chunks use a SLOT-MAJOR [l, f, b] layout: the ew multiply and a
  tree of tensor_tensor adds are then fully contiguous fp16 APs, which
  is what the DVE 2x packed mode requires on hardware.  Small chunks
  stay node-major with one 1x tensor_reduce (fewer instructions).
- Node values feeding edges are pre-scaled on device (x'=dis*x,
  ys=dis*relu(vW1+b1)W2) so the per-edge device math is one multiply.
- PB's whole MLP runs on the Tensor engine: v is transposed via the PE,
  W1 is applied as a block-diagonal [32,128] stationary (8 nodes per
  matmul), bias+relu ride the scalar-engine PSUM eviction, W2 likewise
  block-diagonal [32,14]; dis scaling fuses into the final eviction.
"""
import sys

sys.path.insert(0, "/opt/trn_rl_repo")

import numpy as np

import bass_rust
from concourse import bass, mybir
from concourse.bass_utils import run_bass_kernel_spmd
from concourse.masks import make_identity
import concourse.tile as tile

import os as _os

PROGRAM_TIMES_NS = []   # (name, exec_time_ns) per device program of last kernel() call


def _enable_tracing():
    import types
    import antenv
    if 'antenv.axon_hooks' in sys.modules:
        return True
    try:
        from trn_agent_boot.trn_boot import _ntff_profile_via_ctypes
        hook = _ntff_profile_via_ctypes('/opt/axon/libaxon_pjrt.so')
    except Exception:
        return False
    mod = types.ModuleType('antenv.axon_hooks')
    mod.get_axon_ntff_profile_hook = lambda: hook
    mod.set_axon_ntff_profile_hook = lambda h: None
    sys.modules['antenv.axon_hooks'] = mod
    antenv.axon_hooks = mod
    import concourse.bass_utils as _bu
    _bu.upload_artifacts = lambda tmpdir: f"local://{tmpdir}"
    return True


def _run(nc, in_maps, name):
    trace = bool(_os.environ.get('GCN_TRACE')) and _enable_tracing()
    r = run_bass_kernel_spmd(nc, in_maps, core_ids=CORE_IDS, trace=trace)
    if trace:
        PROGRAM_TIMES_NS.append((name, r.exec_time_ns))
    return r.results

S = 8
N = 200000
NS = N // S
P = 128
G = 4            # degree-class granularity: slots per node = G*ceil(d/G)
F1 = 3
F2 = 7
CORE_IDS = list(range(S))
FP = mybir.dt.float32
F16 = mybir.dt.float16
MUL = mybir.AluOpType.mult
ADD = mybir.AluOpType.add
AX = mybir.AxisListType.X
RELU = mybir.ActivationFunctionType.Relu

_CHUNK7 = 12288   # chunk budget in F2-elems per partition
_SM_MIN = 1000    # below this (F2-elems), node-major + tensor_reduce


def _ceil(a, b):
    return -(-a // b)


# --------------------------------------------------------------------------
# walrus on this toolchain accepts at most ONE sync-wait per instruction;
# Tile emits several at DAG joins / kernel-tail drain. Hoist excess waits
# onto fresh same-engine NoOps inserted immediately before the violator.
def legalize_waits(nc):
    nop_idx = 0
    for f in nc.m.functions:
        for bb in f.blocks:
            il = bb.instructions
            if not any(
                inst.sync_info is not None
                and len(inst.sync_info.on_wait or []) > 1
                for inst in il
            ):
                continue
            new_il = []
            for inst in il:
                si = inst.sync_info
                w = list(si.on_wait or []) if si is not None else []
                if len(w) > 1:
                    for extra in w[:-1]:
                        nop = mybir.InstNoOp(
                            name=f"I-waitsplit-{nop_idx}", ins=[], outs=[]
                        )
                        nop_idx += 1
                        nop.engine = inst.engine
                        nop.sync_info = bass_rust.SyncInfo(
                            on_wait=[extra], on_update=[]
                        )
                        new_il.append(nop)
                    si.on_wait = [w[-1]]
                new_il.append(inst)
            bb.instructions = new_il


# --------------------------------------------------------------------------
# host-side structure building (integer routing only)
class _O:
    pass


def build_structs(row, col, ew):
    row = row.astype(np.int64)
    col = col.astype(np.int64)
    cores = []
    for c in range(S):
        cs = _O()
        m = (col // NS) == c
        cs.erow = row[m]
        cs.ecol = (col[m] - c * NS).astype(np.int64)
        cs.eew = ew[m].astype(np.float32)
        cores.append(cs)

    for cs in cores:
        d = np.bincount(cs.ecol, minlength=NS)
        cs.jcls = np.maximum(1, _ceil(np.maximum(d, 1), G))
    jmax = max(int(cs.jcls.max()) for cs in cores)
    nj = np.zeros(jmax + 1, np.int64)
    for j in range(1, jmax + 1):
        njc = max(int((cs.jcls == j).sum()) for cs in cores)
        nj[j] = _ceil(max(njc, 1), P) * P
    for cs in cores:
        nodepos = np.full(NS, -1, np.int64)
        pos = 0
        for j in range(1, jmax + 1):
            nodes = np.nonzero(cs.jcls == j)[0]
            nodepos[nodes] = pos + np.arange(len(nodes))
            pos += nj[j]
        cs.nodepos = nodepos
    ntot = int(nj[1:].sum())

    st = _O()
    st.jmax, st.nj, st.ntot = jmax, nj, ntot
    st.cores = cores

    # per-edge occurrence index among edges sharing a destination
    for cs in cores:
        nodes = cs.ecol
        ord_ = np.argsort(nodes, kind="stable")
        ns = nodes[ord_]
        first = np.r_[True, ns[1:] != ns[:-1]]
        idx_of_first = np.maximum.accumulate(
            np.where(first, np.arange(len(ord_)), 0)
        )
        k = np.empty(len(nodes), np.int64)
        k[ord_] = np.arange(len(ord_)) - idx_of_first
        cs.kocc = k

    # class bases
    st.base_node = np.zeros(jmax + 2, np.int64)
    st.base_blk = np.zeros(jmax + 2, np.int64)
    for j in range(1, jmax + 1):
        st.base_node[j + 1] = st.base_node[j] + nj[j]
        st.base_blk[j + 1] = st.base_blk[j] + nj[j] // P

    # shared chunk plan (same block ranges for all three per-edge tensors);
    # carve a small GpSimd-sized chunk off the three biggest classes
    # (GpSimd offload measured net-negative: concurrent GpSimd SBUF traffic
    # halves the DVE 2x packed-mode rate, so no chunks are carved for it)
    gp_classes = set()
    chunks = []
    off1 = off3 = off7 = agg7 = 0
    for j in range(1, jmax + 1):
        L = G * j
        nbj = int(nj[j]) // P
        blmax = max(2, (_CHUNK7 // (F2 * L)) & ~1)
        gp_left = 14 if (j in gp_classes and nbj >= 40) else 0
        b0 = 0
        while b0 < nbj:
            if gp_left:
                bl, gp = gp_left, True
                gp_left = 0
            else:
                bl, gp = min(blmax, nbj - b0), False
                if bl % 2 == 1 and bl > 1 and bl * F2 * L >= _SM_MIN:
                    bl -= 1   # keep the big chunk even (slot-major capable)
            sm = (bl % 2 == 0) and (gp or bl * F2 * L >= _SM_MIN)
            ck = _O()
            ck.j, ck.L, ck.b0, ck.bl, ck.sm, ck.gp = j, L, b0, bl, sm, gp
            ck.off1, ck.off3, ck.off7, ck.agg7 = off1, off3, off7, agg7
            chunks.append(ck)
            off1 += bl * L
            off3 += bl * F1 * L
            off7 += bl * F2 * L
            agg7 += bl * F2
            b0 += bl
    st.chunks = chunks
    st.SF1, st.SF3, st.SF7 = off1, off3, off7
    assert agg7 == (ntot // P) * F2

    # per-class chunk lookup tables (by block index)
    st.cmap = {}
    for j in range(1, jmax + 1):
        nbj = int(nj[j]) // P
        cid = np.zeros(nbj, np.int64)
        for i, ck in enumerate(chunks):
            if ck.j == j:
                cid[ck.b0 : ck.b0 + ck.bl] = i
        st.cmap[j] = cid
    return st


def edge_slots(st, cs):
    """per edge: partition p, and for each tensor the flat offset of the
    (slot k, feature 0) element plus the per-feature stride."""
    nodes = cs.ecol
    j = cs.jcls[nodes]
    q_local = cs.nodepos[nodes] - st.base_node[j]
    nbj = st.nj[j] // P
    p = q_local // nbj
    b = q_local % nbj
    k = cs.kocc
    ne = len(nodes)
    off1 = np.empty(ne, np.int64)
    off3 = np.empty(ne, np.int64)
    off7 = np.empty(ne, np.int64)
    fs3 = np.empty(ne, np.int64)
    fs7 = np.empty(ne, np.int64)
    for jj in range(1, st.jmax + 1):
        sel = j == jj
        if not sel.any():
            continue
        L = G * jj
        ci = st.cmap[jj][b[sel]]
        cb0 = np.array([st.chunks[i].b0 for i in range(len(st.chunks))])
        cbl = np.array([st.chunks[i].bl for i in range(len(st.chunks))])
        csm = np.array([st.chunks[i].sm for i in range(len(st.chunks))])
        c1 = np.array([st.chunks[i].off1 for i in range(len(st.chunks))])
        c3 = np.array([st.chunks[i].off3 for i in range(len(st.chunks))])
        c7 = np.array([st.chunks[i].off7 for i in range(len(st.chunks))])
        bo = b[sel] - cb0[ci]
        bl = cbl[ci]
        sm = csm[ci]
        kk = k[sel]
        # slot-major: base + k*(F*bl) + f*bl + bo ; node-major: bo*(F*L)+f*L+k
        off1[sel] = np.where(sm, c1[ci] + kk * bl + bo,
                             c1[ci] + bo * L + kk)
        off3[sel] = np.where(sm, c3[ci] + kk * (F1 * bl) + bo,
                             c3[ci] + bo * (F1 * L) + kk)
        off7[sel] = np.where(sm, c7[ci] + kk * (F2 * bl) + bo,
                             c7[ci] + bo * (F2 * L) + kk)
        fs3[sel] = np.where(sm, bl, L)
        fs7[sel] = np.where(sm, bl, L)
    return p, off1, off3, fs3, off7, fs7


def node_slots(st, cs):
    """per local node: partition p, global block gb, and the (base, fstride)
    of its features in the chunk-major agg7 layout."""
    j = cs.jcls
    q_local = cs.nodepos - st.base_node[j]
    nbj = st.nj[j] // P
    p = q_local // nbj
    b = q_local % nbj
    gb = st.base_blk[j] + b
    ns = len(j)
    base = np.empty(ns, np.int64)
    fstr = np.empty(ns, np.int64)
    cb0 = np.array([c.b0 for c in st.chunks])
    cbl = np.array([c.bl for c in st.chunks])
    csm = np.array([c.sm for c in st.chunks])
    ca7 = np.array([c.agg7 for c in st.chunks])
    for jj in range(1, st.jmax + 1):
        sel = j == jj
        if not sel.any():
            continue
        ci = st.cmap[jj][b[sel]]
        bo = b[sel] - cb0[ci]
        base[sel] = np.where(csm[ci], ca7[ci] + bo,
                             ca7[ci] + bo * F2)
        fstr[sel] = np.where(csm[ci], cbl[ci], 1)
    return p, gb, base, fstr


# --------------------------------------------------------------------------
# device-program helpers
def _fold_flat(nc, eng, t_in, FB, L, out_ap, fshape=None):
    """tree-fold the leading slot axis of a slot-major chunk (viewed as
    [P, L*FB] with l outermost): contiguous halves, all 2x; the final add
    writes out_ap (free size FB; if fshape=(F, bl) the inputs are viewed
    [P, F, bl] to match a shaped/strided out_ap)."""
    tv = t_in[:]
    cur = L
    while cur > 2:
        if cur % 2 == 1:
            eng.tensor_tensor(
                tv[:, 0:FB], tv[:, 0:FB],
                tv[:, (cur - 1) * FB : cur * FB], ADD,
            )
            cur -= 1
            if cur == 2:
                break
        h = cur // 2
        eng.tensor_tensor(
            tv[:, 0 : h * FB], tv[:, 0 : h * FB],
            tv[:, h * FB : cur * FB], ADD,
        )
        cur = h
    i0, i1 = tv[:, 0:FB], tv[:, FB : 2 * FB]
    if fshape is not None:
        F, bl = fshape
        i0 = i0.rearrange("p (f b) -> p f b", f=F)
        i1 = i1.rearrange("p (f b) -> p f b", f=F)
    eng.tensor_tensor(out_ap, i0, i1, ADD)


def _edge_stream(nc, pool, st, F, xs, t_ew, agg_of, tag, shaped_fold=False,
                 subset=None, ews_dram=None, ew_state=None):
    """Stream per-edge fp16 payload chunks, multiply by the ew slots and
    reduce the slot axis.  agg_of(ck) -> output AP (free size bl*F) in the
    chosen agg layout for that chunk.  If ews_dram is given, the ew slots
    are DMA'd just-in-time in pieces right before the chunks needing them."""
    for ck in (subset if subset is not None else st.chunks):
        L, bl = ck.L, ck.bl
        eng = nc.vector
        if ews_dram is not None:
            need = ck.off1 + bl * L
            if need > ew_state["done"]:
                end = max(need, min(st.SF1, ew_state["done"] + 2048))
                nc.sync.dma_start(
                    out=t_ew[:, ew_state["done"] : end],
                    in_=ews_dram[:, ew_state["done"] : end],
                )
                ew_state["done"] = end
        offF = ck.off1 if F == 1 else (ck.off3 if F == F1 else ck.off7)
        n = bl * F * L
        t_in = pool.tile([P, n], F16, tag=tag)
        nc.sync.dma_start(out=t_in[:], in_=xs[:, offF : offF + n])
        if ck.sm:
            FB = F * bl
            if t_ew is not None:
                eng.tensor_tensor(
                    t_in[:].rearrange("p (l f b) -> p l f b", l=L, f=F),
                    t_in[:].rearrange("p (l f b) -> p l f b", l=L, f=F),
                    t_ew[:, ck.off1 : ck.off1 + bl * L]
                    .rearrange("p (l b) -> p l b", l=L)
                    .unsqueeze(2)
                    .broadcast_to([P, L, F, bl]),
                    MUL,
                )
            _fold_flat(nc, eng, t_in, FB, L, agg_of(ck),
                       fshape=(F, bl) if shaped_fold else None)
        else:
            if t_ew is not None:
                eng.tensor_tensor(
                    t_in[:].rearrange("p (b f l) -> p b f l", f=F, l=L),
                    t_in[:].rearrange("p (b f l) -> p b f l", f=F, l=L),
                    t_ew[:, ck.off1 : ck.off1 + bl * L]
                    .rearrange("p (b l) -> p b l", l=L)
                    .unsqueeze(2)
                    .broadcast_to([P, bl, F, L]),
                    MUL,
                )
            with nc.allow_low_precision(reason="fp16 agg within tolerance"):
                eng.tensor_reduce(
                    out=agg_of(ck),
                    in_=t_in[:].rearrange("p (b f l) -> p b f l", f=F, l=L),
                    axis=AX,
                    op=ADD,
                )


def build_PA(st):
    """ews_pa (node-major, per class) + x_own -> dis (fp16), xp = dis*x."""
    nc = bass.Bass("TRN2", num_devices=S)
    nb = st.ntot // P
    SFA = sum((int(st.nj[j]) // P) * G * j for j in range(1, st.jmax + 1))
    ews = nc.dram_tensor("ews_pa", (P, SFA), F16, kind="ExternalInput")
    x_own = nc.dram_tensor("x_own", (P, nb * F1), F16, kind="ExternalInput")
    dis_o = nc.dram_tensor("dis", (P, nb), F16, kind="ExternalOutput")
    xp_o = nc.dram_tensor("xp", (P, nb * F1), F16, kind="ExternalOutput")
    with tile.TileContext(nc) as tc:
        with tc.tile_pool(name="acc", bufs=1) as apool:
            # whole ews resident; two DMA pieces so reduces start early
            t_ews = apool.tile([P, SFA], F16)
            half = 0
            accf = 0
            for j in range(1, st.jmax + 1):
                if accf >= SFA // 2:
                    half = accf
                    break
                accf += (int(st.nj[j]) // P) * G * j
            if not half:
                half = SFA
            nc.sync.dma_start(out=t_ews[:, :half], in_=ews[:, :half])
            if half < SFA:
                nc.sync.dma_start(out=t_ews[:, half:], in_=ews[:, half:])
            t_xo = apool.tile([P, nb * F1], F16)
            nc.sync.dma_start(out=t_xo[:], in_=x_own[:])
            t_deg = apool.tile([P, nb], F16)
            accf = 0
            accb = 0
            for j in range(1, st.jmax + 1):
                L = G * j
                nbj = int(st.nj[j]) // P
                with nc.allow_low_precision(
                    reason="fp16 deg within tolerance"
                ):
                    nc.vector.tensor_reduce(
                        out=t_deg[:, accb : accb + nbj],
                        in_=t_ews[:, accf : accf + nbj * L].rearrange(
                            "p (b l) -> p b l", l=L
                        ),
                        axis=AX,
                        op=ADD,
                    )
                accf += nbj * L
                accb += nbj
            t_d32 = apool.tile([P, nb], FP)
            nc.scalar.copy(t_d32[:], t_deg[:])
            nc.vector.tensor_scalar_add(t_d32[:], t_d32[:], 1.0)
            t_sq = apool.tile([P, nb], FP)
            nc.scalar.sqrt(t_sq[:], t_d32[:])
            t_r = apool.tile([P, nb], FP)
            nc.vector.reciprocal(t_r[:], t_sq[:])
            t_d16 = apool.tile([P, nb], F16)
            nc.scalar.copy(t_d16[:], t_r[:])
            nc.sync.dma_start(out=dis_o[:], in_=t_d16[:])
            t_xp = apool.tile([P, nb * F1], F16)
            nc.vector.tensor_tensor(
                t_xp[:].rearrange("p (b f) -> p b f", f=F1),
                t_xo[:].rearrange("p (b f) -> p b f", f=F1),
                t_d16[:].unsqueeze(2).broadcast_to([P, nb, F1]),
                MUL,
            )
            nc.sync.dma_start(out=xp_o[:], in_=t_xp[:])
    legalize_waits(nc)
    return nc


def _pb_geom(st):
    nb = st.ntot // P
    nbp = _ceil(nb, 128) * 128   # v4 node blocks padded (32 nodes/tile, 4-col)
    ntiles = nbp // 32           # 128-col transpose tiles of 4-stride v
    ncc = ntiles // 4            # 512-col W1 column chunks
    ne = ncc * 4                 # W1 matmuls / ht 512-col chunks
    nw2 = ne * 4                 # W2 matmuls
    npy = ncc * 6                # y PSUM tiles (3 outs at 0/32/64, per cc)
    return nb, nbp, ntiles, ncc, ne, nw2, npy


def build_PB(st):
    """xs + ews + xp_own + dis -> yst: packed dis*relu(vW1+b1)@W2 (fp16).

    v is kept 4-strided per node [b*4+f]; PE transposes 128-col tiles; W1
    is a block-diagonal [32,128] stationary (8 nodes per matmul, K row
    groups at partition bases 0/32/64 and a widened K=64 for the base-96
    group); bias+relu fuse into the scalar-engine PSUM eviction; W2 is
    block-diagonal [32,14]; dis scaling fuses into the final eviction."""
    nc = bass.Bass("TRN2", num_devices=S)
    nb, nbp, ntiles, ncc, ne, nw2, npy = _pb_geom(st)
    ews = nc.dram_tensor("ews", (P, st.SF1), F16, kind="ExternalInput")
    xs = nc.dram_tensor("xs", (P, st.SF3), F16, kind="ExternalInput")
    xpo4 = nc.dram_tensor("xpo4", (P, nbp * 4), F16, kind="ExternalInput")
    dis4 = nc.dram_tensor("dis4", (P, nbp * 4), F16, kind="ExternalInput")
    w1bd = nc.dram_tensor("w1bd", (P, 128), F16, kind="ExternalInput")
    w1bdh = nc.dram_tensor("w1bdh", (P, 128), F16, kind="ExternalInput")
    w2bd = nc.dram_tensor("w2bd", (P, 14), F16, kind="ExternalInput")
    w2bdh = nc.dram_tensor("w2bdh", (P, 14), F16, kind="ExternalInput")
    b1v = nc.dram_tensor("b1v", (P, 1), FP, kind="ExternalInput")
    disr = nc.dram_tensor("disr", (P, npy * 512), F16, kind="ExternalInput")
    yst_o = nc.dram_tensor("yst", (P, npy * 512), F16, kind="ExternalOutput")
    with tile.TileContext(nc) as tc:
        with tc.tile_pool(name="sb", bufs=3) as pool, tc.tile_pool(
            name="acc", bufs=1
        ) as apool, tc.tile_pool(
            name="ptr", bufs=2, space="PSUM"
        ) as tpool, tc.tile_pool(
            name="ph", bufs=2, space="PSUM"
        ) as hpool, tc.tile_pool(
            name="py", bufs=2, space="PSUM"
        ) as ypool:
            t_v4 = apool.tile([P, nbp * 4], F16)
            nc.gpsimd.memset(t_v4[:], 0.0)
            t_id = apool.tile([P, P], F16)
            make_identity(nc, t_id[:])
            t_ew = apool.tile([P, st.SF1], F16)
            t_xpo = apool.tile([P, nbp * 4], F16)
            nc.sync.dma_start(out=t_xpo[:], in_=xpo4[:])
            t_dis = apool.tile([P, nbp * 4], F16)
            nc.sync.dma_start(out=t_dis[:], in_=dis4[:])

            def agg_of(ck):
                gb0 = int(st.base_blk[ck.j]) + ck.b0
                view = t_v4[:, gb0 * 4 : (gb0 + ck.bl) * 4].rearrange(
                    "p (b f) -> p b f", f=4
                )[:, :, 0:F1]
                if ck.sm:
                    # fold's final add iterates (f, b)
                    return view.rearrange("p b f -> p f b")
                return view

            t_vt = apool.tile([P, ntiles * 128], F16)
            t_ht = apool.tile([P, ne * 512], F16)
            t_yst = apool.tile([P, npy * 512], F16)
            t_w1 = apool.tile([P, 128], F16)
            t_w1h = apool.tile([P, 128], F16)
            t_w2 = apool.tile([P, 14], F16)
            t_w2h = apool.tile([P, 14], F16)
            t_b1v = apool.tile([P, 1], FP)
            t_disr = apool.tile([P, npy * 512], F16)

            def mlp_part(cc):
                # v-prep for this 512-col slice, then the PE pipeline
                sl = slice(cc * 512, (cc + 1) * 512)
                nc.vector.tensor_tensor(
                    t_v4[:, sl], t_v4[:, sl], t_xpo[:, sl], ADD
                )
                nc.vector.tensor_tensor(
                    t_v4[:, sl], t_v4[:, sl], t_dis[:, sl], MUL
                )
                t_tr = tpool.tile([P, 512], F16, tag="tr")
                for gi in range(4):
                    tt = cc * 4 + gi
                    nc.tensor.transpose(
                        t_tr[:, gi * 128 : (gi + 1) * 128],
                        t_v4[:, tt * 128 : (tt + 1) * 128],
                        t_id[:],
                    )
                nc.scalar.copy(t_vt[:, sl], t_tr[:])
                for Q in range(4):
                    e = cc * 4 + Q
                    t_hp = hpool.tile([P, 512], FP, tag="h")
                    if Q < 3:
                        nc.tensor.matmul(
                            t_hp[:, :],
                            t_w1[32 * Q : 32 * Q + 32, :],
                            t_vt[32 * Q : 32 * Q + 32, sl],
                        )
                    else:
                        nc.tensor.matmul(
                            t_hp[:, :],
                            t_w1h[64:128, :],
                            t_vt[64:128, sl],
                        )
                    nc.scalar.activation(
                        t_ht[:, e * 512 : (e + 1) * 512],
                        t_hp[:, :],
                        RELU,
                        bias=t_b1v[:, 0:1],
                    )
                t_y = None
                for Q in range(4):
                    e = cc * 4 + Q
                    for R in range(4):
                        wl = Q * 4 + R
                        s = wl % 3
                        if s == 0:
                            t_y = ypool.tile([P, 512], FP, tag="y")
                        if R < 3:
                            nc.tensor.matmul(
                                t_y[32 * s : 32 * s + 14, :],
                                t_w2[32 * R : 32 * R + 32, :],
                                t_ht[32 * R : 32 * R + 32,
                                     e * 512 : (e + 1) * 512],
                            )
                        else:
                            nc.tensor.matmul(
                                t_y[32 * s : 32 * s + 14, :],
                                t_w2h[64:128, :],
                                t_ht[64:128, e * 512 : (e + 1) * 512],
                            )
                        if s == 2 or wl == 15:
                            pt = cc * 6 + wl // 3
                            nc.vector.tensor_tensor(
                                t_yst[:, pt * 512 : (pt + 1) * 512],
                                t_y[:, :],
                                t_disr[:, pt * 512 : (pt + 1) * 512],
                                MUL,
                            )

            # interleave: edge chunks needed by column-chunk cc, then its
            # MLP part, so the Tensor/Scalar pipeline overlaps the stream
            gs = [int(st.base_blk[ck.j]) + ck.b0 for ck in st.chunks]
            done = 0
            ew_state = {"done": 0}
            for cc in range(ncc):
                need = 128 * (cc + 1)
                hi = len(st.chunks)
                if cc < ncc - 1:
                    hi = next(
                        (i for i, g in enumerate(gs) if g >= need),
                        len(st.chunks),
                    )
                _edge_stream(nc, pool, st, F1, xs, t_ew, agg_of, "x",
                             shaped_fold=True,
                             subset=st.chunks[done:hi],
                             ews_dram=ews, ew_state=ew_state)
                if cc == 0:
                    # weights arrive while the first folds run
                    nc.sync.dma_start(out=t_w1[:], in_=w1bd[:])
                    nc.sync.dma_start(out=t_w1h[:], in_=w1bdh[:])
                    nc.sync.dma_start(out=t_w2[:], in_=w2bd[:])
                    nc.sync.dma_start(out=t_w2h[:], in_=w2bdh[:])
                    nc.sync.dma_start(out=t_b1v[:], in_=b1v[:])
                    nc.sync.dma_start(out=t_disr[:], in_=disr[:])
                done = hi
                mlp_part(cc)
            nc.sync.dma_start(out=yst_o[:], in_=t_yst[:])
    legalize_waits(nc)
    return nc


def build_PE(st):
    """yss + ews + yso + dis7 + b2e -> out = dis*(agg2 + ys_own) + b2,
    everything in the chunk-major agg layout (host unscrambles)."""
    nc = bass.Bass("TRN2", num_devices=S)
    nb = st.ntot // P
    ews = nc.dram_tensor("ews", (P, st.SF1), F16, kind="ExternalInput")
    yss = nc.dram_tensor("yss", (P, st.SF7), F16, kind="ExternalInput")
    yso = nc.dram_tensor("yso", (P, nb * F2), F16, kind="ExternalInput")
    dis7 = nc.dram_tensor("dis7", (P, nb * F2), F16, kind="ExternalInput")
    if st.b2_nonzero:
        b2e = nc.dram_tensor("b2e", (P, nb * F2), F16, kind="ExternalInput")
    out_o = nc.dram_tensor("out", (P, nb * F2), F16, kind="ExternalOutput")
    with tile.TileContext(nc) as tc:
        with tc.tile_pool(name="sb", bufs=3) as pool, tc.tile_pool(
            name="acc", bufs=1
        ) as apool:
            t_ew = apool.tile([P, st.SF1], F16)
            t_yso = apool.tile([P, nb * F2], F16)
            t_dis7 = apool.tile([P, nb * F2], F16)
            t_b2e = apool.tile([P, nb * F2], F16)

            t_agg = apool.tile([P, nb * F2], F16)

            def agg_of(ck):
                sl = t_agg[:, ck.agg7 : ck.agg7 + ck.bl * F2]
                if ck.sm:
                    return sl
                return sl.rearrange("p (b f) -> p b f", f=F2)

            ew_state = {"done": 0}
            _edge_stream(nc, pool, st, F2, yss, t_ew, agg_of, "y",
                         subset=st.chunks[:4],
                         ews_dram=ews, ew_state=ew_state)
            # node-level operands load while the stream runs
            nc.sync.dma_start(out=t_yso[:], in_=yso[:])
            nc.sync.dma_start(out=t_dis7[:], in_=dis7[:])
            if st.b2_nonzero:
                nc.sync.dma_start(out=t_b2e[:], in_=b2e[:])
            _edge_stream(nc, pool, st, F2, yss, t_ew, agg_of, "y",
                         subset=st.chunks[4:],
                         ews_dram=ews, ew_state=ew_state)

            nc.vector.tensor_tensor(t_agg[:], t_agg[:], t_yso[:], ADD)
            nc.vector.tensor_tensor(t_agg[:], t_agg[:], t_dis7[:], MUL)
            if st.b2_nonzero:
                nc.vector.tensor_tensor(t_agg[:], t_agg[:], t_b2e[:], ADD)
            nc.sync.dma_start(out=out_o[:], in_=t_agg[:])
    legalize_waits(nc)
    return nc


# --------------------------------------------------------------------------
def kernel(x, edge_index, edge_weight, W1, b1, W2, b2):
    x = np.asarray(x, np.float32)
    ei = np.asarray(edge_index)
    ew = np.asarray(edge_weight, np.float32)
    W1 = np.asarray(W1, np.float32)
    b1 = np.asarray(b1, np.float32)
    W2 = np.asarray(W2, np.float32)
    b2 = np.asarray(b2, np.float32)

    PROGRAM_TIMES_NS.clear()
    st = build_structs(ei[0], ei[1], ew)
    st.b2_nonzero = bool(np.any(b2))
    nb = st.ntot // P
    _, nbp, ntiles, ncc, ne, nw2, npy = _pb_geom(st)
    ar = np.arange(NS)

    core_idx = []
    for c in range(S):
        cs = st.cores[c]
        p_e, off1, off3, fs3, off7, fs7 = edge_slots(st, cs)
        p_n, gb, nbase, nfstr = node_slots(st, cs)
        core_idx.append((cs, p_e, off1, off3, fs3, off7, fs7,
                         p_n, gb, nbase, nfstr))

    # node (p,gb) -> (row base before feature, column) in packed yst
    def yst_pos(p_n, gb):
        tc_ = gb // 32
        r32 = gb % 32
        Q = r32 // 8
        g = r32 % 8
        cc = tc_ // 4
        ci = (tc_ % 4) * 128 + p_n
        R = g // 2
        gp = g % 2
        wl = Q * 4 + R
        return 32 * (wl % 3) + 7 * gp, (cc * 6 + wl // 3) * 512 + ci

    # ---------------- P_A ----------------
    nc = build_PA(st)
    SFA = sum((int(st.nj[j]) // P) * G * j for j in range(1, st.jmax + 1))
    in_maps = []
    for c in range(S):
        cs = st.cores[c]
        p_e = core_idx[c][1]
        p_n, gb = core_idx[c][7], core_idx[c][8]
        # node-major per-class layout for PA's degree reduce
        j = cs.jcls[cs.ecol]
        q_local = cs.nodepos[cs.ecol] - st.base_node[j]
        nbj = st.nj[j] // P
        b = q_local % nbj
        base_free = np.zeros(st.jmax + 2, np.int64)
        for jj in range(1, st.jmax + 1):
            base_free[jj + 1] = base_free[jj] + (st.nj[jj] // P) * G * jj
        offa = base_free[j] + b * (G * j) + cs.kocc
        ews_pa = np.zeros((P, SFA), np.float16)
        ews_pa[p_e, offa] = cs.eew.astype(np.float16)
        x_own = np.zeros((P, nb, F1), np.float16)
        x_own[p_n, gb] = x[c * NS + ar].astype(np.float16)
        in_maps.append({"ews_pa": ews_pa, "x_own": x_own.reshape(P, nb * F1)})
    res = _run(nc, in_maps, "PA_deg")
    dis_l = [res[c]["dis"] for c in range(S)]
    xp_l = [res[c]["xp"] for c in range(S)]

    xp_can = np.zeros((N, F1), np.float16)
    dis_can = np.zeros(N, np.float16)
    for c in range(S):
        p_n, gb = core_idx[c][7], core_idx[c][8]
        xp_can[c * NS + ar] = xp_l[c].reshape(P, nb, F1)[p_n, gb]
        dis_can[c * NS + ar] = dis_l[c][p_n, gb]

    # ---------------- P_B (layer 1 + MLP) ----------------
    nc = build_PB(st)
    W1h = W1.astype(np.float16)
    W2h = W2.astype(np.float16)
    w1bdb = np.zeros((P, 128), np.float16)
    for Q in range(3):
        for gl in range(8):
            w1bdb[32 * Q + 4 * gl : 32 * Q + 4 * gl + 3,
                  16 * gl : 16 * gl + 16] = W1h
    w1bdhb = np.zeros((P, 128), np.float16)
    for gl in range(8):
        w1bdhb[96 + 4 * gl : 96 + 4 * gl + 3, 16 * gl : 16 * gl + 16] = W1h
    w2bdb = np.zeros((P, 14), np.float16)
    for R in range(3):
        for gp in range(2):
            w2bdb[32 * R + 16 * gp : 32 * R + 16 * gp + 16,
                  7 * gp : 7 * gp + 7] = W2h
    w2bdhb = np.zeros((P, 14), np.float16)
    for gp in range(2):
        w2bdhb[96 + 16 * gp : 96 + 16 * gp + 16, 7 * gp : 7 * gp + 7] = W2h
    b1vb = b1.astype(np.float32)[np.arange(P) % 16].reshape(P, 1)

    in_maps = []
    ews_l = []
    for c in range(S):
        cs = st.cores[c]
        p_e, off1, off3, fs3 = (core_idx[c][1], core_idx[c][2],
                                core_idx[c][3], core_idx[c][4])
        p_n, gb = core_idx[c][7], core_idx[c][8]
        ews = np.zeros((P, st.SF1), np.float16)
        ews[p_e, off1] = cs.eew.astype(np.float16)
        ews_l.append(ews)
        xs = np.zeros((P, st.SF3), np.float16)
        for fi in range(F1):
            xs[p_e, off3 + fi * fs3] = xp_can[cs.erow, fi]
        xpo4 = np.zeros((P, nbp, 4), np.float16)
        xpo4[p_n, gb, 0:F1] = xp_l[c].reshape(P, nb, F1)[p_n, gb]
        dis4 = np.zeros((P, nbp, 4), np.float16)
        dis4[p_n, gb, 0:F1] = dis_l[c][p_n, gb][:, None]
        rbase, col_ = yst_pos(p_n, gb)
        disr = np.zeros((P, npy * 512), np.float16)
        for r in range(F2):
            disr[rbase + r, col_] = dis_l[c][p_n, gb]
        in_maps.append(
            {
                "ews": ews,
                "xs": xs,
                "xpo4": xpo4.reshape(P, nbp * 4),
                "dis4": dis4.reshape(P, nbp * 4),
                "w1bd": w1bdb,
                "w1bdh": w1bdhb,
                "w2bd": w2bdb,
                "w2bdh": w2bdhb,
                "b1v": b1vb,
                "disr": disr,
            }
        )
    res = _run(nc, in_maps, "PB_layer1")
    yst_l = [res[c]["yst"] for c in range(S)]

    ys_can = np.zeros((N, F2), np.float16)
    for c in range(S):
        p_n, gb = core_idx[c][7], core_idx[c][8]
        rbase, col_ = yst_pos(p_n, gb)
        v = np.empty((NS, F2), np.float16)
        for r in range(F2):
            v[:, r] = yst_l[c][rbase + r, col_]
        ys_can[c * NS + ar] = v

    # ---------------- P_E (layer 2) ----------------
    nc = build_PE(st)
    b2h = b2.astype(np.float16)
    in_maps = []
    for c in range(S):
        cs = st.cores[c]
        p_e, off7, fs7 = core_idx[c][1], core_idx[c][5], core_idx[c][6]
        p_n, nbase, nfstr = core_idx[c][7], core_idx[c][9], core_idx[c][10]
        yss = np.zeros((P, st.SF7), np.float16)
        for fi in range(F2):
            yss[p_e, off7 + fi * fs7] = ys_can[cs.erow, fi]
        yso = np.zeros((P, nb * F2), np.float16)
        dis7 = np.zeros((P, nb * F2), np.float16)
        ysl = ys_can[c * NS + ar]
        disl = dis_can[c * NS + ar]
        for fi in range(F2):
            yso[p_n, nbase + fi * nfstr] = ysl[:, fi]
            dis7[p_n, nbase + fi * nfstr] = disl
        im = {"ews": ews_l[c], "yss": yss, "yso": yso, "dis7": dis7}
        if st.b2_nonzero:
            b2e = np.zeros((P, nb * F2), np.float16)
            for fi in range(F2):
                b2e[p_n, nbase + fi * nfstr] = b2h[fi]
            im["b2e"] = b2e
        in_maps.append(im)
    res = _run(nc, in_maps, "PE_layer2")

    out = np.zeros((N, F2), np.float32)
    for c in range(S):
        o = res[c]["out"]
        p_n, nbase, nfstr = core_idx[c][7], core_idx[c][9], core_idx[c][10]
        for fi in range(F2):
            out[c * NS + ar, fi] = o[p_n, nbase + fi * nfstr].astype(
                np.float32
            )
    return out


# revision 4
# speedup vs baseline: 1.1382x; 1.0190x over previous
"""GCN (2-layer) on 8 Trainium2 NeuronCores — v4 (3 device programs).

Graph/data parallel per the node-range sharding hint: nodes sharded by
range, edges live on the destination core, weights replicated.  All
irregular routing happens on the HOST as pure copies/permutations;
every FP arithmetic op on values runs on device.

- Destination nodes bucketed by in-degree class j=ceil(d/4); each node
  gets exactly 4j slots so segment-sum becomes a slot-axis reduction.
- Big chunks use a SLOT-MAJOR [l, f, b] layout: the ew multiply and a
  tree of tensor_tensor adds are then fully contiguous fp16 APs, which
  is what the DVE 2x packed mode requires on hardware.  Small chunks
  stay node-major with one 1x tensor_reduce (fewer instructions).
- Node values feeding edges are pre-scaled on device (x'=dis*x,
  ys=dis*relu(vW1+b1)W2) so the per-edge device math is one multiply.
- PB's whole MLP runs on the Tensor engine: v is transposed via the PE,
  W1 is applied as a block-diagonal [32,128] stationary (8 nodes per
  matmul), bias+relu ride the scalar-engine PSUM eviction, W2 likewise
  block-diagonal [32,14]; dis scaling fuses into the final eviction.
"""
import sys

sys.path.insert(0, "/opt/trn_rl_repo")

import numpy as np

import bass_rust
from concourse import bass, mybir
from concourse.bass_utils import run_bass_kernel_spmd
from concourse.masks import make_identity
import concourse.tile as tile

import os as _os

PROGRAM_TIMES_NS = []   # (name, exec_time_ns) per device program of last kernel() call


def _enable_tracing():
    import types
    import antenv
    if 'antenv.axon_hooks' in sys.modules:
        return True
    try:
        from trn_agent_boot.trn_boot import _ntff_profile_via_ctypes
        hook = _ntff_profile_via_ctypes('/opt/axon/libaxon_pjrt.so')
    except Exception:
        return False
    mod = types.ModuleType('antenv.axon_hooks')
    mod.get_axon_ntff_profile_hook = lambda: hook
    mod.set_axon_ntff_profile_hook = lambda h: None
    sys.modules['antenv.axon_hooks'] = mod
    antenv.axon_hooks = mod
    import concourse.bass_utils as _bu
    _bu.upload_artifacts = lambda tmpdir: f"local://{tmpdir}"
    return True


def _run(nc, in_maps, name):
    trace = bool(_os.environ.get('GCN_TRACE')) and _enable_tracing()
    r = run_bass_kernel_spmd(nc, in_maps, core_ids=CORE_IDS, trace=trace)
    if trace:
        PROGRAM_TIMES_NS.append((name, r.exec_time_ns))
    return r.results

S = 8
N = 200000
NS = N // S
P = 128
G = 4            # degree-class granularity: slots per node = G*ceil(d/G)
F1 = 3
F2 = 7
CORE_IDS = list(range(S))
FP = mybir.dt.float32
F16 = mybir.dt.float16
MUL = mybir.AluOpType.mult
ADD = mybir.AluOpType.add
AX = mybir.AxisListType.X
RELU = mybir.ActivationFunctionType.Relu

_CHUNK7 = 16384   # chunk budget in F2-elems per partition
_SM_MIN = 1000    # below this (F2-elems), node-major + tensor_reduce


def _ceil(a, b):
    return -(-a // b)


# --------------------------------------------------------------------------
# walrus on this toolchain accepts at most ONE sync-wait per instruction;
# Tile emits several at DAG joins / kernel-tail drain. Hoist excess waits
# onto fresh same-engine NoOps inserted immediately before the violator.
def legalize_waits(nc):
    nop_idx = 0
    for f in nc.m.functions:
        for bb in f.blocks:
            il = bb.instructions
            if not any(
                inst.sync_info is not None
                and len(inst.sync_info.on_wait or []) > 1
                for inst in il
            ):
                continue
            new_il = []
            for inst in il:
                si = inst.sync_info
                w = list(si.on_wait or []) if si is not None else []
                if len(w) > 1:
                    for extra in w[:-1]:
                        nop = mybir.InstNoOp(
                            name=f"I-waitsplit-{nop_idx}", ins=[], outs=[]
                        )
                        nop_idx += 1
                        nop.engine = inst.engine
                        nop.sync_info = bass_rust.SyncInfo(
                            on_wait=[extra], on_update=[]
                        )
                        new_il.append(nop)
                    si.on_wait = [w[-1]]
                new_il.append(inst)
            bb.instructions = new_il


# --------------------------------------------------------------------------
# host-side structure building (integer routing only)
class _O:
    pass


def build_structs(row, col, ew):
    row = row.astype(np.int64)
    col = col.astype(np.int64)
    cores = []
    for c in range(S):
        cs = _O()
        m = (col // NS) == c
        cs.erow = row[m]
        cs.ecol = (col[m] - c * NS).astype(np.int64)
        cs.eew = ew[m].astype(np.float32)
        cores.append(cs)

    for cs in cores:
        d = np.bincount(cs.ecol, minlength=NS)
        cs.jcls = np.maximum(1, _ceil(np.maximum(d, 1), G))
    jmax = max(int(cs.jcls.max()) for cs in cores)
    nj = np.zeros(jmax + 1, np.int64)
    for j in range(1, jmax + 1):
        njc = max(int((cs.jcls == j).sum()) for cs in cores)
        nj[j] = _ceil(max(njc, 1), P) * P
    for cs in cores:
        nodepos = np.full(NS, -1, np.int64)
        pos = 0
        for j in range(1, jmax + 1):
            nodes = np.nonzero(cs.jcls == j)[0]
            nodepos[nodes] = pos + np.arange(len(nodes))
            pos += nj[j]
        cs.nodepos = nodepos
    ntot = int(nj[1:].sum())

    st = _O()
    st.jmax, st.nj, st.ntot = jmax, nj, ntot
    st.cores = cores

    # per-edge occurrence index among edges sharing a destination
    for cs in cores:
        nodes = cs.ecol
        ord_ = np.argsort(nodes, kind="stable")
        ns = nodes[ord_]
        first = np.r_[True, ns[1:] != ns[:-1]]
        idx_of_first = np.maximum.accumulate(
            np.where(first, np.arange(len(ord_)), 0)
        )
        k = np.empty(len(nodes), np.int64)
        k[ord_] = np.arange(len(ord_)) - idx_of_first
        cs.kocc = k

    # class bases
    st.base_node = np.zeros(jmax + 2, np.int64)
    st.base_blk = np.zeros(jmax + 2, np.int64)
    for j in range(1, jmax + 1):
        st.base_node[j + 1] = st.base_node[j] + nj[j]
        st.base_blk[j + 1] = st.base_blk[j] + nj[j] // P

    # shared chunk plan (same block ranges for all three per-edge tensors);
    # carve a small GpSimd-sized chunk off the three biggest classes
    # (GpSimd offload measured net-negative: concurrent GpSimd SBUF traffic
    # halves the DVE 2x packed-mode rate, so no chunks are carved for it)
    gp_classes = set()
    chunks = []
    off1 = off3 = off7 = agg7 = 0
    for j in range(1, jmax + 1):
        L = G * j
        nbj = int(nj[j]) // P
        blmax = max(2, (_CHUNK7 // (F2 * L)) & ~1)
        gp_left = 14 if (j in gp_classes and nbj >= 40) else 0
        b0 = 0
        while b0 < nbj:
            if gp_left:
                bl, gp = gp_left, True
                gp_left = 0
            else:
                bl, gp = min(blmax, nbj - b0), False
                if bl % 2 == 1 and bl > 1 and bl * F2 * L >= _SM_MIN:
                    bl -= 1   # keep the big chunk even (slot-major capable)
            sm = (bl % 2 == 0) and (gp or bl * F2 * L >= _SM_MIN)
            ck = _O()
            ck.j, ck.L, ck.b0, ck.bl, ck.sm, ck.gp = j, L, b0, bl, sm, gp
            ck.off1, ck.off3, ck.off7, ck.agg7 = off1, off3, off7, agg7
            chunks.append(ck)
            off1 += bl * L
            off3 += bl * F1 * L
            off7 += bl * F2 * L
            agg7 += bl * F2
            b0 += bl
    st.chunks = chunks
    st.SF1, st.SF3, st.SF7 = off1, off3, off7
    assert agg7 == (ntot // P) * F2

    # per-class chunk lookup tables (by block index)
    st.cmap = {}
    for j in range(1, jmax + 1):
        nbj = int(nj[j]) // P
        cid = np.zeros(nbj, np.int64)
        for i, ck in enumerate(chunks):
            if ck.j == j:
                cid[ck.b0 : ck.b0 + ck.bl] = i
        st.cmap[j] = cid
    return st


def edge_slots(st, cs):
    """per edge: partition p, and for each tensor the flat offset of the
    (slot k, feature 0) element plus the per-feature stride."""
    nodes = cs.ecol
    j = cs.jcls[nodes]
    q_local = cs.nodepos[nodes] - st.base_node[j]
    nbj = st.nj[j] // P
    p = q_local // nbj
    b = q_local % nbj
    k = cs.kocc
    ne = len(nodes)
    off1 = np.empty(ne, np.int64)
    off3 = np.empty(ne, np.int64)
    off7 = np.empty(ne, np.int64)
    fs3 = np.empty(ne, np.int64)
    fs7 = np.empty(ne, np.int64)
    for jj in range(1, st.jmax + 1):
        sel = j == jj
        if not sel.any():
            continue
        L = G * jj
        ci = st.cmap[jj][b[sel]]
        cb0 = np.array([st.chunks[i].b0 for i in range(len(st.chunks))])
        cbl = np.array([st.chunks[i].bl for i in range(len(st.chunks))])
        csm = np.array([st.chunks[i].sm for i in range(len(st.chunks))])
        c1 = np.array([st.chunks[i].off1 for i in range(len(st.chunks))])
        c3 = np.array([st.chunks[i].off3 for i in range(len(st.chunks))])
        c7 = np.array([st.chunks[i].off7 for i in range(len(st.chunks))])
        bo = b[sel] - cb0[ci]
        bl = cbl[ci]
        sm = csm[ci]
        kk = k[sel]
        # slot-major: base + k*(F*bl) + f*bl + bo ; node-major: bo*(F*L)+f*L+k
        off1[sel] = np.where(sm, c1[ci] + kk * bl + bo,
                             c1[ci] + bo * L + kk)
        off3[sel] = np.where(sm, c3[ci] + kk * (F1 * bl) + bo,
                             c3[ci] + bo * (F1 * L) + kk)
        off7[sel] = np.where(sm, c7[ci] + kk * (F2 * bl) + bo,
                             c7[ci] + bo * (F2 * L) + kk)
        fs3[sel] = np.where(sm, bl, L)
        fs7[sel] = np.where(sm, bl, L)
    return p, off1, off3, fs3, off7, fs7


def node_slots(st, cs):
    """per local node: partition p, global block gb, and the (base, fstride)
    of its features in the chunk-major agg7 layout."""
    j = cs.jcls
    q_local = cs.nodepos - st.base_node[j]
    nbj = st.nj[j] // P
    p = q_local // nbj
    b = q_local % nbj
    gb = st.base_blk[j] + b
    ns = len(j)
    base = np.empty(ns, np.int64)
    fstr = np.empty(ns, np.int64)
    cb0 = np.array([c.b0 for c in st.chunks])
    cbl = np.array([c.bl for c in st.chunks])
    csm = np.array([c.sm for c in st.chunks])
    ca7 = np.array([c.agg7 for c in st.chunks])
    for jj in range(1, st.jmax + 1):
        sel = j == jj
        if not sel.any():
            continue
        ci = st.cmap[jj][b[sel]]
        bo = b[sel] - cb0[ci]
        base[sel] = np.where(csm[ci], ca7[ci] + bo,
                             ca7[ci] + bo * F2)
        fstr[sel] = np.where(csm[ci], cbl[ci], 1)
    return p, gb, base, fstr


# --------------------------------------------------------------------------
# device-program helpers
def _fold_flat(nc, eng, t_in, FB, L, out_ap, fshape=None):
    """tree-fold the leading slot axis of a slot-major chunk (viewed as
    [P, L*FB] with l outermost): contiguous halves, all 2x; the final add
    writes out_ap (free size FB; if fshape=(F, bl) the inputs are viewed
    [P, F, bl] to match a shaped/strided out_ap)."""
    tv = t_in[:]
    cur = L
    while cur > 2:
        if cur % 2 == 1:
            eng.tensor_tensor(
                tv[:, 0:FB], tv[:, 0:FB],
                tv[:, (cur - 1) * FB : cur * FB], ADD,
            )
            cur -= 1
            if cur == 2:
                break
        h = cur // 2
        eng.tensor_tensor(
            tv[:, 0 : h * FB], tv[:, 0 : h * FB],
            tv[:, h * FB : cur * FB], ADD,
        )
        cur = h
    i0, i1 = tv[:, 0:FB], tv[:, FB : 2 * FB]
    if fshape is not None:
        F, bl = fshape
        i0 = i0.rearrange("p (f b) -> p f b", f=F)
        i1 = i1.rearrange("p (f b) -> p f b", f=F)
    eng.tensor_tensor(out_ap, i0, i1, ADD)


def _edge_stream(nc, pool, st, F, xs, t_ew, agg_of, tag, shaped_fold=False,
                 subset=None, ews_dram=None, ew_state=None):
    """Stream per-edge fp16 payload chunks, multiply by the ew slots and
    reduce the slot axis.  agg_of(ck) -> output AP (free size bl*F) in the
    chosen agg layout for that chunk.  If ews_dram is given, the ew slots
    are DMA'd just-in-time in pieces right before the chunks needing them."""
    for ck in (subset if subset is not None else st.chunks):
        L, bl = ck.L, ck.bl
        eng = nc.vector
        if ews_dram is not None:
            need = ck.off1 + bl * L
            if need > ew_state["done"]:
                end = max(need, min(st.SF1, ew_state["done"] + 2048))
                nc.sync.dma_start(
                    out=t_ew[:, ew_state["done"] : end],
                    in_=ews_dram[:, ew_state["done"] : end],
                )
                ew_state["done"] = end
        offF = ck.off1 if F == 1 else (ck.off3 if F == F1 else ck.off7)
        n = bl * F * L
        t_in = pool.tile([P, n], F16, tag=tag)
        nc.sync.dma_start(out=t_in[:], in_=xs[:, offF : offF + n])
        if ck.sm:
            FB = F * bl
            if t_ew is not None:
                eng.tensor_tensor(
                    t_in[:].rearrange("p (l f b) -> p l f b", l=L, f=F),
                    t_in[:].rearrange("p (l f b) -> p l f b", l=L, f=F),
                    t_ew[:, ck.off1 : ck.off1 + bl * L]
                    .rearrange("p (l b) -> p l b", l=L)
                    .unsqueeze(2)
                    .broadcast_to([P, L, F, bl]),
                    MUL,
                )
            _fold_flat(nc, eng, t_in, FB, L, agg_of(ck),
                       fshape=(F, bl) if shaped_fold else None)
        else:
            if t_ew is not None:
                eng.tensor_tensor(
                    t_in[:].rearrange("p (b f l) -> p b f l", f=F, l=L),
                    t_in[:].rearrange("p (b f l) -> p b f l", f=F, l=L),
                    t_ew[:, ck.off1 : ck.off1 + bl * L]
                    .rearrange("p (b l) -> p b l", l=L)
                    .unsqueeze(2)
                    .broadcast_to([P, bl, F, L]),
                    MUL,
                )
            with nc.allow_low_precision(reason="fp16 agg within tolerance"):
                eng.tensor_reduce(
                    out=agg_of(ck),
                    in_=t_in[:].rearrange("p (b f l) -> p b f l", f=F, l=L),
                    axis=AX,
                    op=ADD,
                )


def build_PA(st):
    """ews_pa (node-major, per class) + x_own -> dis (fp16), xp = dis*x."""
    nc = bass.Bass("TRN2", num_devices=S)
    nb = st.ntot // P
    SFA = sum((int(st.nj[j]) // P) * G * j for j in range(1, st.jmax + 1))
    ews = nc.dram_tensor("ews_pa", (P, SFA), F16, kind="ExternalInput")
    x_own = nc.dram_tensor("x_own", (P, nb * F1), F16, kind="ExternalInput")
    dis_o = nc.dram_tensor("dis", (P, nb), F16, kind="ExternalOutput")
    xp_o = nc.dram_tensor("xp", (P, nb * F1), F16, kind="ExternalOutput")
    with tile.TileContext(nc) as tc:
        with tc.tile_pool(name="acc", bufs=1) as apool:
            # whole ews resident; two DMA pieces so reduces start early
            t_ews = apool.tile([P, SFA], F16)
            half = 0
            accf = 0
            for j in range(1, st.jmax + 1):
                if accf >= SFA // 2:
                    half = accf
                    break
                accf += (int(st.nj[j]) // P) * G * j
            if not half:
                half = SFA
            nc.sync.dma_start(out=t_ews[:, :half], in_=ews[:, :half])
            if half < SFA:
                nc.sync.dma_start(out=t_ews[:, half:], in_=ews[:, half:])
            t_xo = apool.tile([P, nb * F1], F16)
            nc.sync.dma_start(out=t_xo[:], in_=x_own[:])
            t_deg = apool.tile([P, nb], F16)
            accf = 0
            accb = 0
            for j in range(1, st.jmax + 1):
                L = G * j
                nbj = int(st.nj[j]) // P
                with nc.allow_low_precision(
                    reason="fp16 deg within tolerance"
                ):
                    nc.vector.tensor_reduce(
                        out=t_deg[:, accb : accb + nbj],
                        in_=t_ews[:, accf : accf + nbj * L].rearrange(
                            "p (b l) -> p b l", l=L
                        ),
                        axis=AX,
                        op=ADD,
                    )
                accf += nbj * L
                accb += nbj
            t_d32 = apool.tile([P, nb], FP)
            nc.scalar.copy(t_d32[:], t_deg[:])
            nc.vector.tensor_scalar_add(t_d32[:], t_d32[:], 1.0)
            t_sq = apool.tile([P, nb], FP)
            nc.scalar.sqrt(t_sq[:], t_d32[:])
            t_r = apool.tile([P, nb], FP)
            nc.vector.reciprocal(t_r[:], t_sq[:])
            t_d16 = apool.tile([P, nb], F16)
            nc.scalar.copy(t_d16[:], t_r[:])
            nc.sync.dma_start(out=dis_o[:], in_=t_d16[:])
            t_xp = apool.tile([P, nb * F1], F16)
            nc.vector.tensor_tensor(
                t_xp[:].rearrange("p (b f) -> p b f", f=F1),
                t_xo[:].rearrange("p (b f) -> p b f", f=F1),
                t_d16[:].unsqueeze(2).broadcast_to([P, nb, F1]),
                MUL,
            )
            nc.sync.dma_start(out=xp_o[:], in_=t_xp[:])
    legalize_waits(nc)
    return nc


def _pb_geom(st):
    nb = st.ntot // P
    nbp = _ceil(nb, 128) * 128   # v4 node blocks padded (32 nodes/tile, 4-col)
    ntiles = nbp // 32           # 128-col transpose tiles of 4-stride v
    ncc = ntiles // 4            # 512-col W1 column chunks
    ne = ncc * 4                 # W1 matmuls / ht 512-col chunks
    nw2 = ne * 4                 # W2 matmuls
    npy = ncc * 6                # y PSUM tiles (3 outs at 0/32/64, per cc)
    return nb, nbp, ntiles, ncc, ne, nw2, npy


def build_PB(st):
    """xs + ews + xp_own + dis -> yst: packed dis*relu(vW1+b1)@W2 (fp16).

    v is kept 4-strided per node [b*4+f]; PE transposes 128-col tiles; W1
    is a block-diagonal [32,128] stationary (8 nodes per matmul, K row
    groups at partition bases 0/32/64 and a widened K=64 for the base-96
    group); bias+relu fuse into the scalar-engine PSUM eviction; W2 is
    block-diagonal [32,14]; dis scaling fuses into the final eviction."""
    nc = bass.Bass("TRN2", num_devices=S)
    nb, nbp, ntiles, ncc, ne, nw2, npy = _pb_geom(st)
    ews = nc.dram_tensor("ews", (P, st.SF1), F16, kind="ExternalInput")
    xs = nc.dram_tensor("xs", (P, st.SF3), F16, kind="ExternalInput")
    xpo4 = nc.dram_tensor("xpo4", (P, nbp * 4), F16, kind="ExternalInput")
    dis4 = nc.dram_tensor("dis4", (P, nbp * 4), F16, kind="ExternalInput")
    w1bd = nc.dram_tensor("w1bd", (P, 128), F16, kind="ExternalInput")
    w1bdh = nc.dram_tensor("w1bdh", (P, 128), F16, kind="ExternalInput")
    w2bd = nc.dram_tensor("w2bd", (P, 14), F16, kind="ExternalInput")
    w2bdh = nc.dram_tensor("w2bdh", (P, 14), F16, kind="ExternalInput")
    b1v = nc.dram_tensor("b1v", (P, 1), FP, kind="ExternalInput")
    disr = nc.dram_tensor("disr", (P, npy * 512), F16, kind="ExternalInput")
    yst_o = nc.dram_tensor("yst", (P, npy * 512), F16, kind="ExternalOutput")
    with tile.TileContext(nc) as tc:
        with tc.tile_pool(name="sb", bufs=3) as pool, tc.tile_pool(
            name="acc", bufs=1
        ) as apool, tc.tile_pool(
            name="ptr", bufs=2, space="PSUM"
        ) as tpool, tc.tile_pool(
            name="ph", bufs=2, space="PSUM"
        ) as hpool, tc.tile_pool(
            name="py", bufs=2, space="PSUM"
        ) as ypool:
            t_v4 = apool.tile([P, nbp * 4], F16)
            nc.gpsimd.memset(t_v4[:], 0.0)
            t_id = apool.tile([P, P], F16)
            make_identity(nc, t_id[:])
            t_ew = apool.tile([P, st.SF1], F16)
            t_xpo = apool.tile([P, nbp * 4], F16)
            nc.sync.dma_start(out=t_xpo[:], in_=xpo4[:])
            t_dis = apool.tile([P, nbp * 4], F16)
            nc.sync.dma_start(out=t_dis[:], in_=dis4[:])

            def agg_of(ck):
                gb0 = int(st.base_blk[ck.j]) + ck.b0
                view = t_v4[:, gb0 * 4 : (gb0 + ck.bl) * 4].rearrange(
                    "p (b f) -> p b f", f=4
                )[:, :, 0:F1]
                if ck.sm:
                    # fold's final add iterates (f, b)
                    return view.rearrange("p b f -> p f b")
                return view

            t_vt = apool.tile([P, ntiles * 128], F16)
            t_ht = apool.tile([P, ne * 512], F16)
            t_yst = apool.tile([P, npy * 512], F16)
            t_w1 = apool.tile([P, 128], F16)
            t_w1h = apool.tile([P, 128], F16)
            t_w2 = apool.tile([P, 14], F16)
            t_w2h = apool.tile([P, 14], F16)
            t_b1v = apool.tile([P, 1], FP)
            t_disr = apool.tile([P, npy * 512], F16)

            def mlp_part(cc):
                # v-prep for this 512-col slice, then the PE pipeline
                sl = slice(cc * 512, (cc + 1) * 512)
                nc.vector.tensor_tensor(
                    t_v4[:, sl], t_v4[:, sl], t_xpo[:, sl], ADD
                )
                nc.vector.tensor_tensor(
                    t_v4[:, sl], t_v4[:, sl], t_dis[:, sl], MUL
                )
                t_tr = tpool.tile([P, 512], F16, tag="tr")
                for gi in range(4):
                    tt = cc * 4 + gi
                    nc.tensor.transpose(
                        t_tr[:, gi * 128 : (gi + 1) * 128],
                        t_v4[:, tt * 128 : (tt + 1) * 128],
                        t_id[:],
                    )
                nc.scalar.copy(t_vt[:, sl], t_tr[:])
                for Q in range(4):
                    e = cc * 4 + Q
                    t_hp = hpool.tile([P, 512], FP, tag="h")
                    if Q < 3:
                        nc.tensor.matmul(
                            t_hp[:, :],
                            t_w1[32 * Q : 32 * Q + 32, :],
                            t_vt[32 * Q : 32 * Q + 32, sl],
                        )
                    else:
                        nc.tensor.matmul(
                            t_hp[:, :],
                            t_w1h[64:128, :],
                            t_vt[64:128, sl],
                        )
                    nc.scalar.activation(
                        t_ht[:, e * 512 : (e + 1) * 512],
                        t_hp[:, :],
                        RELU,
                        bias=t_b1v[:, 0:1],
                    )
                t_y = None
                for Q in range(4):
                    e = cc * 4 + Q
                    for R in range(4):
                        wl = Q * 4 + R
                        s = wl % 3
                        if s == 0:
                            t_y = ypool.tile([P, 512], FP, tag="y")
                        if R < 3:
                            nc.tensor.matmul(
                                t_y[32 * s : 32 * s + 14, :],
                                t_w2[32 * R : 32 * R + 32, :],
                                t_ht[32 * R : 32 * R + 32,
                                     e * 512 : (e + 1) * 512],
                            )
                        else:
                            nc.tensor.matmul(
                                t_y[32 * s : 32 * s + 14, :],
                                t_w2h[64:128, :],
                                t_ht[64:128, e * 512 : (e + 1) * 512],
                            )
                        if s == 2 or wl == 15:
                            pt = cc * 6 + wl // 3
                            nc.vector.tensor_tensor(
                                t_yst[:, pt * 512 : (pt + 1) * 512],
                                t_y[:, :],
                                t_disr[:, pt * 512 : (pt + 1) * 512],
                                MUL,
                            )

            # interleave: edge chunks needed by column-chunk cc, then its
            # MLP part, so the Tensor/Scalar pipeline overlaps the stream
            gs = [int(st.base_blk[ck.j]) + ck.b0 for ck in st.chunks]
            done = 0
            ew_state = {"done": 0}
            for cc in range(ncc):
                need = 128 * (cc + 1)
                hi = len(st.chunks)
                if cc < ncc - 1:
                    hi = next(
                        (i for i, g in enumerate(gs) if g >= need),
                        len(st.chunks),
                    )
                _edge_stream(nc, pool, st, F1, xs, t_ew, agg_of, "x",
                             shaped_fold=True,
                             subset=st.chunks[done:hi],
                             ews_dram=ews, ew_state=ew_state)
                if cc == 0:
                    # weights arrive while the first folds run
                    nc.sync.dma_start(out=t_w1[:], in_=w1bd[:])
                    nc.sync.dma_start(out=t_w1h[:], in_=w1bdh[:])
                    nc.sync.dma_start(out=t_w2[:], in_=w2bd[:])
                    nc.sync.dma_start(out=t_w2h[:], in_=w2bdh[:])
                    nc.sync.dma_start(out=t_b1v[:], in_=b1v[:])
                    nc.sync.dma_start(out=t_disr[:], in_=disr[:])
                done = hi
                mlp_part(cc)
                # ship this part's packed outputs while the next streams
                nc.sync.dma_start(
                    out=yst_o[:, cc * 6 * 512 : (cc + 1) * 6 * 512],
                    in_=t_yst[:, cc * 6 * 512 : (cc + 1) * 6 * 512],
                )
    legalize_waits(nc)
    return nc


def build_PE(st):
    """yss + ews + yso + dis7 + b2e -> out = dis*(agg2 + ys_own) + b2,
    everything in the chunk-major agg layout (host unscrambles)."""
    nc = bass.Bass("TRN2", num_devices=S)
    nb = st.ntot // P
    ews = nc.dram_tensor("ews", (P, st.SF1), F16, kind="ExternalInput")
    yss = nc.dram_tensor("yss", (P, st.SF7), F16, kind="ExternalInput")
    yso = nc.dram_tensor("yso", (P, nb * F2), F16, kind="ExternalInput")
    dis7 = nc.dram_tensor("dis7", (P, nb * F2), F16, kind="ExternalInput")
    if st.b2_nonzero:
        b2e = nc.dram_tensor("b2e", (P, nb * F2), F16, kind="ExternalInput")
    out_o = nc.dram_tensor("out", (P, nb * F2), F16, kind="ExternalOutput")
    with tile.TileContext(nc) as tc:
        with tc.tile_pool(name="sb", bufs=3) as pool, tc.tile_pool(
            name="acc", bufs=1
        ) as apool:
            t_ew = apool.tile([P, st.SF1], F16)
            t_yso = apool.tile([P, nb * F2], F16)
            t_dis7 = apool.tile([P, nb * F2], F16)
            t_b2e = apool.tile([P, nb * F2], F16)

            t_agg = apool.tile([P, nb * F2], F16)

            def agg_of(ck):
                sl = t_agg[:, ck.agg7 : ck.agg7 + ck.bl * F2]
                if ck.sm:
                    return sl
                return sl.rearrange("p (b f) -> p b f", f=F2)

            ew_state = {"done": 0}
            _edge_stream(nc, pool, st, F2, yss, t_ew, agg_of, "y",
                         subset=st.chunks[:4],
                         ews_dram=ews, ew_state=ew_state)
            # node-level operands load while the stream runs
            nc.sync.dma_start(out=t_yso[:], in_=yso[:])
            nc.sync.dma_start(out=t_dis7[:], in_=dis7[:])
            if st.b2_nonzero:
                nc.sync.dma_start(out=t_b2e[:], in_=b2e[:])
            _edge_stream(nc, pool, st, F2, yss, t_ew, agg_of, "y",
                         subset=st.chunks[4:],
                         ews_dram=ews, ew_state=ew_state)

            nc.vector.tensor_tensor(t_agg[:], t_agg[:], t_yso[:], ADD)
            nc.vector.tensor_tensor(t_agg[:], t_agg[:], t_dis7[:], MUL)
            if st.b2_nonzero:
                nc.vector.tensor_tensor(t_agg[:], t_agg[:], t_b2e[:], ADD)
            nc.sync.dma_start(out=out_o[:], in_=t_agg[:])
    legalize_waits(nc)
    return nc


# --------------------------------------------------------------------------
def kernel(x, edge_index, edge_weight, W1, b1, W2, b2):
    x = np.asarray(x, np.float32)
    ei = np.asarray(edge_index)
    ew = np.asarray(edge_weight, np.float32)
    W1 = np.asarray(W1, np.float32)
    b1 = np.asarray(b1, np.float32)
    W2 = np.asarray(W2, np.float32)
    b2 = np.asarray(b2, np.float32)

    PROGRAM_TIMES_NS.clear()
    st = build_structs(ei[0], ei[1], ew)
    st.b2_nonzero = bool(np.any(b2))
    nb = st.ntot // P
    _, nbp, ntiles, ncc, ne, nw2, npy = _pb_geom(st)
    ar = np.arange(NS)

    core_idx = []
    for c in range(S):
        cs = st.cores[c]
        p_e, off1, off3, fs3, off7, fs7 = edge_slots(st, cs)
        p_n, gb, nbase, nfstr = node_slots(st, cs)
        core_idx.append((cs, p_e, off1, off3, fs3, off7, fs7,
                         p_n, gb, nbase, nfstr))

    # node (p,gb) -> (row base before feature, column) in packed yst
    def yst_pos(p_n, gb):
        tc_ = gb // 32
        r32 = gb % 32
        Q = r32 // 8
        g = r32 % 8
        cc = tc_ // 4
        ci = (tc_ % 4) * 128 + p_n
        R = g // 2
        gp = g % 2
        wl = Q * 4 + R
        return 32 * (wl % 3) + 7 * gp, (cc * 6 + wl // 3) * 512 + ci

    # ---------------- P_A ----------------
    nc = build_PA(st)
    SFA = sum((int(st.nj[j]) // P) * G * j for j in range(1, st.jmax + 1))
    in_maps = []
    for c in range(S):
        cs = st.cores[c]
        p_e = core_idx[c][1]
        p_n, gb = core_idx[c][7], core_idx[c][8]
        # node-major per-class layout for PA's degree reduce
        j = cs.jcls[cs.ecol]
        q_local = cs.nodepos[cs.ecol] - st.base_node[j]
        nbj = st.nj[j] // P
        b = q_local % nbj
        base_free = np.zeros(st.jmax + 2, np.int64)
        for jj in range(1, st.jmax + 1):
            base_free[jj + 1] = base_free[jj] + (st.nj[jj] // P) * G * jj
        offa = base_free[j] + b * (G * j) + cs.kocc
        ews_pa = np.zeros((P, SFA), np.float16)
        ews_pa[p_e, offa] = cs.eew.astype(np.float16)
        x_own = np.zeros((P, nb, F1), np.float16)
        x_own[p_n, gb] = x[c * NS + ar].astype(np.float16)
        in_maps.append({"ews_pa": ews_pa, "x_own": x_own.reshape(P, nb * F1)})
    res = _run(nc, in_maps, "PA_deg")
    dis_l = [res[c]["dis"] for c in range(S)]
    xp_l = [res[c]["xp"] for c in range(S)]

    xp_can = np.zeros((N, F1), np.float16)
    dis_can = np.zeros(N, np.float16)
    for c in range(S):
        p_n, gb = core_idx[c][7], core_idx[c][8]
        xp_can[c * NS + ar] = xp_l[c].reshape(P, nb, F1)[p_n, gb]
        dis_can[c * NS + ar] = dis_l[c][p_n, gb]

    # ---------------- P_B (layer 1 + MLP) ----------------
    nc = build_PB(st)
    W1h = W1.astype(np.float16)
    W2h = W2.astype(np.float16)
    w1bdb = np.zeros((P, 128), np.float16)
    for Q in range(3):
        for gl in range(8):
            w1bdb[32 * Q + 4 * gl : 32 * Q + 4 * gl + 3,
                  16 * gl : 16 * gl + 16] = W1h
    w1bdhb = np.zeros((P, 128), np.float16)
    for gl in range(8):
        w1bdhb[96 + 4 * gl : 96 + 4 * gl + 3, 16 * gl : 16 * gl + 16] = W1h
    w2bdb = np.zeros((P, 14), np.float16)
    for R in range(3):
        for gp in range(2):
            w2bdb[32 * R + 16 * gp : 32 * R + 16 * gp + 16,
                  7 * gp : 7 * gp + 7] = W2h
    w2bdhb = np.zeros((P, 14), np.float16)
    for gp in range(2):
        w2bdhb[96 + 16 * gp : 96 + 16 * gp + 16, 7 * gp : 7 * gp + 7] = W2h
    b1vb = b1.astype(np.float32)[np.arange(P) % 16].reshape(P, 1)

    in_maps = []
    ews_l = []
    for c in range(S):
        cs = st.cores[c]
        p_e, off1, off3, fs3 = (core_idx[c][1], core_idx[c][2],
                                core_idx[c][3], core_idx[c][4])
        p_n, gb = core_idx[c][7], core_idx[c][8]
        ews = np.zeros((P, st.SF1), np.float16)
        ews[p_e, off1] = cs.eew.astype(np.float16)
        ews_l.append(ews)
        xs = np.zeros((P, st.SF3), np.float16)
        for fi in range(F1):
            xs[p_e, off3 + fi * fs3] = xp_can[cs.erow, fi]
        xpo4 = np.zeros((P, nbp, 4), np.float16)
        xpo4[p_n, gb, 0:F1] = xp_l[c].reshape(P, nb, F1)[p_n, gb]
        dis4 = np.zeros((P, nbp, 4), np.float16)
        dis4[p_n, gb, 0:F1] = dis_l[c][p_n, gb][:, None]
        rbase, col_ = yst_pos(p_n, gb)
        disr = np.zeros((P, npy * 512), np.float16)
        for r in range(F2):
            disr[rbase + r, col_] = dis_l[c][p_n, gb]
        in_maps.append(
            {
                "ews": ews,
                "xs": xs,
                "xpo4": xpo4.reshape(P, nbp * 4),
                "dis4": dis4.reshape(P, nbp * 4),
                "w1bd": w1bdb,
                "w1bdh": w1bdhb,
                "w2bd": w2bdb,
                "w2bdh": w2bdhb,
                "b1v": b1vb,
                "disr": disr,
            }
        )
    res = _run(nc, in_maps, "PB_layer1")
    yst_l = [res[c]["yst"] for c in range(S)]

    ys_can = np.zeros((N, F2), np.float16)
    for c in range(S):
        p_n, gb = core_idx[c][7], core_idx[c][8]
        rbase, col_ = yst_pos(p_n, gb)
        v = np.empty((NS, F2), np.float16)
        for r in range(F2):
            v[:, r] = yst_l[c][rbase + r, col_]
        ys_can[c * NS + ar] = v

    # ---------------- P_E (layer 2) ----------------
    nc = build_PE(st)
    b2h = b2.astype(np.float16)
    in_maps = []
    for c in range(S):
        cs = st.cores[c]
        p_e, off7, fs7 = core_idx[c][1], core_idx[c][5], core_idx[c][6]
        p_n, nbase, nfstr = core_idx[c][7], core_idx[c][9], core_idx[c][10]
        yss = np.zeros((P, st.SF7), np.float16)
        for fi in range(F2):
            yss[p_e, off7 + fi * fs7] = ys_can[cs.erow, fi]
        yso = np.zeros((P, nb * F2), np.float16)
        dis7 = np.zeros((P, nb * F2), np.float16)
        ysl = ys_can[c * NS + ar]
        disl = dis_can[c * NS + ar]
        for fi in range(F2):
            yso[p_n, nbase + fi * nfstr] = ysl[:, fi]
            dis7[p_n, nbase + fi * nfstr] = disl
        im = {"ews": ews_l[c], "yss": yss, "yso": yso, "dis7": dis7}
        if st.b2_nonzero:
            b2e = np.zeros((P, nb * F2), np.float16)
            for fi in range(F2):
                b2e[p_n, nbase + fi * nfstr] = b2h[fi]
            im["b2e"] = b2e
        in_maps.append(im)
    res = _run(nc, in_maps, "PE_layer2")

    out = np.zeros((N, F2), np.float32)
    for c in range(S):
        o = res[c]["out"]
        p_n, nbase, nfstr = core_idx[c][7], core_idx[c][9], core_idx[c][10]
        for fi in range(F2):
            out[c * NS + ar, fi] = o[p_n, nbase + fi * nfstr].astype(
                np.float32
            )
    return out


# revision 5
# speedup vs baseline: 1.1513x; 1.0115x over previous
"""GCN (2-layer) on 8 Trainium2 NeuronCores — v4 (3 device programs).

Graph/data parallel per the node-range sharding hint: nodes sharded by
range, edges live on the destination core, weights replicated.  All
irregular routing happens on the HOST as pure copies/permutations;
every FP arithmetic op on values runs on device.

- Destination nodes bucketed by in-degree class j=ceil(d/4); each node
  gets exactly 4j slots so segment-sum becomes a slot-axis reduction.
- Big chunks use a SLOT-MAJOR [l, f, b] layout: the ew multiply and a
  tree of tensor_tensor adds are then fully contiguous fp16 APs, which
  is what the DVE 2x packed mode requires on hardware.  Small chunks
  stay node-major with one 1x tensor_reduce (fewer instructions).
- Node values feeding edges are pre-scaled on device (x'=dis*x,
  ys=dis*relu(vW1+b1)W2) so the per-edge device math is one multiply.
- PB's whole MLP runs on the Tensor engine: v is transposed via the PE,
  W1 is applied as a block-diagonal [32,128] stationary (8 nodes per
  matmul), bias+relu ride the scalar-engine PSUM eviction, W2 likewise
  block-diagonal [32,14]; dis scaling fuses into the final eviction.
"""
import sys

sys.path.insert(0, "/opt/trn_rl_repo")

import numpy as np

import bass_rust
from concourse import bass, mybir
from concourse.bass_utils import run_bass_kernel_spmd
from concourse.masks import make_identity
import concourse.tile as tile

import os as _os

PROGRAM_TIMES_NS = []   # (name, exec_time_ns) per device program of last kernel() call


def _enable_tracing():
    import types
    import antenv
    if 'antenv.axon_hooks' in sys.modules:
        return True
    try:
        from trn_agent_boot.trn_boot import _ntff_profile_via_ctypes
        hook = _ntff_profile_via_ctypes('/opt/axon/libaxon_pjrt.so')
    except Exception:
        return False
    mod = types.ModuleType('antenv.axon_hooks')
    mod.get_axon_ntff_profile_hook = lambda: hook
    mod.set_axon_ntff_profile_hook = lambda h: None
    sys.modules['antenv.axon_hooks'] = mod
    antenv.axon_hooks = mod
    import concourse.bass_utils as _bu
    _bu.upload_artifacts = lambda tmpdir: f"local://{tmpdir}"
    return True


def _run(nc, in_maps, name):
    trace = bool(_os.environ.get('GCN_TRACE')) and _enable_tracing()
    r = run_bass_kernel_spmd(nc, in_maps, core_ids=CORE_IDS, trace=trace)
    if trace:
        PROGRAM_TIMES_NS.append((name, r.exec_time_ns))
    return r.results

S = 8
N = 200000
NS = N // S
P = 128
G = 4            # degree-class granularity: slots per node = G*ceil(d/G)
F1 = 3
F2 = 7
CORE_IDS = list(range(S))
FP = mybir.dt.float32
F16 = mybir.dt.float16
MUL = mybir.AluOpType.mult
ADD = mybir.AluOpType.add
AX = mybir.AxisListType.X
RELU = mybir.ActivationFunctionType.Relu

_CHUNK7 = 16384   # chunk budget in F2-elems per partition
_SM_MIN = 1000    # below this (F2-elems), node-major + tensor_reduce


def _ceil(a, b):
    return -(-a // b)


# --------------------------------------------------------------------------
# walrus on this toolchain accepts at most ONE sync-wait per instruction;
# Tile emits several at DAG joins / kernel-tail drain. Hoist excess waits
# onto fresh same-engine NoOps inserted immediately before the violator.
def legalize_waits(nc):
    nop_idx = 0
    for f in nc.m.functions:
        for bb in f.blocks:
            il = bb.instructions
            if not any(
                inst.sync_info is not None
                and len(inst.sync_info.on_wait or []) > 1
                for inst in il
            ):
                continue
            new_il = []
            for inst in il:
                si = inst.sync_info
                w = list(si.on_wait or []) if si is not None else []
                if len(w) > 1:
                    for extra in w[:-1]:
                        nop = mybir.InstNoOp(
                            name=f"I-waitsplit-{nop_idx}", ins=[], outs=[]
                        )
                        nop_idx += 1
                        nop.engine = inst.engine
                        nop.sync_info = bass_rust.SyncInfo(
                            on_wait=[extra], on_update=[]
                        )
                        new_il.append(nop)
                    si.on_wait = [w[-1]]
                new_il.append(inst)
            bb.instructions = new_il


# --------------------------------------------------------------------------
# host-side structure building (integer routing only)
class _O:
    pass


def build_structs(row, col, ew):
    row = row.astype(np.int64)
    col = col.astype(np.int64)
    cores = []
    for c in range(S):
        cs = _O()
        m = (col // NS) == c
        cs.erow = row[m]
        cs.ecol = (col[m] - c * NS).astype(np.int64)
        cs.eew = ew[m].astype(np.float32)
        cores.append(cs)

    for cs in cores:
        d = np.bincount(cs.ecol, minlength=NS)
        cs.jcls = np.maximum(1, _ceil(np.maximum(d, 1), G))
    jmax = max(int(cs.jcls.max()) for cs in cores)
    nj = np.zeros(jmax + 1, np.int64)
    for j in range(1, jmax + 1):
        njc = max(int((cs.jcls == j).sum()) for cs in cores)
        nj[j] = _ceil(max(njc, 1), P) * P
    for cs in cores:
        nodepos = np.full(NS, -1, np.int64)
        pos = 0
        for j in range(1, jmax + 1):
            nodes = np.nonzero(cs.jcls == j)[0]
            nodepos[nodes] = pos + np.arange(len(nodes))
            pos += nj[j]
        cs.nodepos = nodepos
    ntot = int(nj[1:].sum())

    st = _O()
    st.jmax, st.nj, st.ntot = jmax, nj, ntot
    st.cores = cores

    # per-edge occurrence index among edges sharing a destination
    for cs in cores:
        nodes = cs.ecol
        ord_ = np.argsort(nodes, kind="stable")
        ns = nodes[ord_]
        first = np.r_[True, ns[1:] != ns[:-1]]
        idx_of_first = np.maximum.accumulate(
            np.where(first, np.arange(len(ord_)), 0)
        )
        k = np.empty(len(nodes), np.int64)
        k[ord_] = np.arange(len(ord_)) - idx_of_first
        cs.kocc = k

    # class bases
    st.base_node = np.zeros(jmax + 2, np.int64)
    st.base_blk = np.zeros(jmax + 2, np.int64)
    for j in range(1, jmax + 1):
        st.base_node[j + 1] = st.base_node[j] + nj[j]
        st.base_blk[j + 1] = st.base_blk[j] + nj[j] // P

    # shared chunk plan (same block ranges for all three per-edge tensors);
    # carve a small GpSimd-sized chunk off the three biggest classes
    # (GpSimd offload measured net-negative: concurrent GpSimd SBUF traffic
    # halves the DVE 2x packed-mode rate, so no chunks are carved for it)
    gp_classes = set()
    chunks = []
    off1 = off3 = off7 = agg7 = 0
    for j in range(1, jmax + 1):
        L = G * j
        nbj = int(nj[j]) // P
        blmax = max(2, (_CHUNK7 // (F2 * L)) & ~1)
        gp_left = 14 if (j in gp_classes and nbj >= 40) else 0
        b0 = 0
        while b0 < nbj:
            if gp_left:
                bl, gp = gp_left, True
                gp_left = 0
            else:
                bl, gp = min(blmax, nbj - b0), False
                if bl % 2 == 1 and bl > 1 and bl * F2 * L >= _SM_MIN:
                    bl -= 1   # keep the big chunk even (slot-major capable)
            sm = (bl % 2 == 0) and (gp or bl * F2 * L >= _SM_MIN)
            ck = _O()
            ck.j, ck.L, ck.b0, ck.bl, ck.sm, ck.gp = j, L, b0, bl, sm, gp
            ck.off1, ck.off3, ck.off7, ck.agg7 = off1, off3, off7, agg7
            chunks.append(ck)
            off1 += bl * L
            off3 += bl * F1 * L
            off7 += bl * F2 * L
            agg7 += bl * F2
            b0 += bl
    st.chunks = chunks
    st.SF1, st.SF3, st.SF7 = off1, off3, off7
    assert agg7 == (ntot // P) * F2

    # per-class chunk lookup tables (by block index)
    st.cmap = {}
    for j in range(1, jmax + 1):
        nbj = int(nj[j]) // P
        cid = np.zeros(nbj, np.int64)
        for i, ck in enumerate(chunks):
            if ck.j == j:
                cid[ck.b0 : ck.b0 + ck.bl] = i
        st.cmap[j] = cid
    return st


def edge_slots(st, cs):
    """per edge: partition p, and for each tensor the flat offset of the
    (slot k, feature 0) element plus the per-feature stride."""
    nodes = cs.ecol
    j = cs.jcls[nodes]
    q_local = cs.nodepos[nodes] - st.base_node[j]
    nbj = st.nj[j] // P
    p = q_local // nbj
    b = q_local % nbj
    k = cs.kocc
    ne = len(nodes)
    off1 = np.empty(ne, np.int64)
    off3 = np.empty(ne, np.int64)
    off7 = np.empty(ne, np.int64)
    fs3 = np.empty(ne, np.int64)
    fs7 = np.empty(ne, np.int64)
    for jj in range(1, st.jmax + 1):
        sel = j == jj
        if not sel.any():
            continue
        L = G * jj
        ci = st.cmap[jj][b[sel]]
        cb0 = np.array([st.chunks[i].b0 for i in range(len(st.chunks))])
        cbl = np.array([st.chunks[i].bl for i in range(len(st.chunks))])
        csm = np.array([st.chunks[i].sm for i in range(len(st.chunks))])
        c1 = np.array([st.chunks[i].off1 for i in range(len(st.chunks))])
        c3 = np.array([st.chunks[i].off3 for i in range(len(st.chunks))])
        c7 = np.array([st.chunks[i].off7 for i in range(len(st.chunks))])
        bo = b[sel] - cb0[ci]
        bl = cbl[ci]
        sm = csm[ci]
        kk = k[sel]
        # slot-major: base + k*(F*bl) + f*bl + bo ; node-major: bo*(F*L)+f*L+k
        off1[sel] = np.where(sm, c1[ci] + kk * bl + bo,
                             c1[ci] + bo * L + kk)
        off3[sel] = np.where(sm, c3[ci] + kk * (F1 * bl) + bo,
                             c3[ci] + bo * (F1 * L) + kk)
        off7[sel] = np.where(sm, c7[ci] + kk * (F2 * bl) + bo,
                             c7[ci] + bo * (F2 * L) + kk)
        fs3[sel] = np.where(sm, bl, L)
        fs7[sel] = np.where(sm, bl, L)
    return p, off1, off3, fs3, off7, fs7


def node_slots(st, cs):
    """per local node: partition p, global block gb, and the (base, fstride)
    of its features in the chunk-major agg7 layout."""
    j = cs.jcls
    q_local = cs.nodepos - st.base_node[j]
    nbj = st.nj[j] // P
    p = q_local // nbj
    b = q_local % nbj
    gb = st.base_blk[j] + b
    ns = len(j)
    base = np.empty(ns, np.int64)
    fstr = np.empty(ns, np.int64)
    cb0 = np.array([c.b0 for c in st.chunks])
    cbl = np.array([c.bl for c in st.chunks])
    csm = np.array([c.sm for c in st.chunks])
    ca7 = np.array([c.agg7 for c in st.chunks])
    for jj in range(1, st.jmax + 1):
        sel = j == jj
        if not sel.any():
            continue
        ci = st.cmap[jj][b[sel]]
        bo = b[sel] - cb0[ci]
        base[sel] = np.where(csm[ci], ca7[ci] + bo,
                             ca7[ci] + bo * F2)
        fstr[sel] = np.where(csm[ci], cbl[ci], 1)
    return p, gb, base, fstr


# --------------------------------------------------------------------------
# device-program helpers
def _fold_flat(nc, eng, t_in, FB, L, out_ap, fshape=None):
    """tree-fold the leading slot axis of a slot-major chunk (viewed as
    [P, L*FB] with l outermost): contiguous halves, all 2x; the final add
    writes out_ap (free size FB; if fshape=(F, bl) the inputs are viewed
    [P, F, bl] to match a shaped/strided out_ap)."""
    tv = t_in[:]
    cur = L
    while cur > 2:
        if cur % 2 == 1:
            eng.tensor_tensor(
                tv[:, 0:FB], tv[:, 0:FB],
                tv[:, (cur - 1) * FB : cur * FB], ADD,
            )
            cur -= 1
            if cur == 2:
                break
        h = cur // 2
        eng.tensor_tensor(
            tv[:, 0 : h * FB], tv[:, 0 : h * FB],
            tv[:, h * FB : cur * FB], ADD,
        )
        cur = h
    i0, i1 = tv[:, 0:FB], tv[:, FB : 2 * FB]
    if fshape is not None:
        F, bl = fshape
        i0 = i0.rearrange("p (f b) -> p f b", f=F)
        i1 = i1.rearrange("p (f b) -> p f b", f=F)
    eng.tensor_tensor(out_ap, i0, i1, ADD)


def _edge_stream(nc, pool, st, F, xs, t_ew, agg_of, tag, shaped_fold=False,
                 subset=None, ews_dram=None, ew_state=None):
    """Stream per-edge fp16 payload chunks, multiply by the ew slots and
    reduce the slot axis.  agg_of(ck) -> output AP (free size bl*F) in the
    chosen agg layout for that chunk.  If ews_dram is given, the ew slots
    are DMA'd just-in-time in pieces right before the chunks needing them."""
    for ck in (subset if subset is not None else st.chunks):
        L, bl = ck.L, ck.bl
        eng = nc.vector
        if ews_dram is not None:
            need = ck.off1 + bl * L
            if need > ew_state["done"]:
                end = max(need, min(st.SF1, ew_state["done"] + 2048))
                nc.sync.dma_start(
                    out=t_ew[:, ew_state["done"] : end],
                    in_=ews_dram[:, ew_state["done"] : end],
                )
                ew_state["done"] = end
        offF = ck.off1 if F == 1 else (ck.off3 if F == F1 else ck.off7)
        n = bl * F * L
        t_in = pool.tile([P, n], F16, tag=tag)
        nc.sync.dma_start(out=t_in[:], in_=xs[:, offF : offF + n])
        if ck.sm:
            FB = F * bl
            if t_ew is not None:
                eng.tensor_tensor(
                    t_in[:].rearrange("p (l f b) -> p l f b", l=L, f=F),
                    t_in[:].rearrange("p (l f b) -> p l f b", l=L, f=F),
                    t_ew[:, ck.off1 : ck.off1 + bl * L]
                    .rearrange("p (l b) -> p l b", l=L)
                    .unsqueeze(2)
                    .broadcast_to([P, L, F, bl]),
                    MUL,
                )
            _fold_flat(nc, eng, t_in, FB, L, agg_of(ck),
                       fshape=(F, bl) if shaped_fold else None)
        else:
            if t_ew is not None:
                eng.tensor_tensor(
                    t_in[:].rearrange("p (b f l) -> p b f l", f=F, l=L),
                    t_in[:].rearrange("p (b f l) -> p b f l", f=F, l=L),
                    t_ew[:, ck.off1 : ck.off1 + bl * L]
                    .rearrange("p (b l) -> p b l", l=L)
                    .unsqueeze(2)
                    .broadcast_to([P, bl, F, L]),
                    MUL,
                )
            with nc.allow_low_precision(reason="fp16 agg within tolerance"):
                eng.tensor_reduce(
                    out=agg_of(ck),
                    in_=t_in[:].rearrange("p (b f l) -> p b f l", f=F, l=L),
                    axis=AX,
                    op=ADD,
                )


def build_PA(st):
    """ews_pa (node-major, per class) + x_own -> dis (fp16), xp = dis*x."""
    nc = bass.Bass("TRN2", num_devices=S)
    nb = st.ntot // P
    SFA = sum((int(st.nj[j]) // P) * G * j for j in range(1, st.jmax + 1))
    ews = nc.dram_tensor("ews_pa", (P, SFA), F16, kind="ExternalInput")
    x_own = nc.dram_tensor("x_own", (P, nb * F1), F16, kind="ExternalInput")
    dis_o = nc.dram_tensor("dis", (P, nb), F16, kind="ExternalOutput")
    xp_o = nc.dram_tensor("xp", (P, nb * F1), F16, kind="ExternalOutput")
    with tile.TileContext(nc) as tc:
        with tc.tile_pool(name="acc", bufs=1) as apool:
            # whole ews resident; two DMA pieces so reduces start early
            t_ews = apool.tile([P, SFA], F16)
            half = 0
            accf = 0
            for j in range(1, st.jmax + 1):
                if accf >= SFA // 2:
                    half = accf
                    break
                accf += (int(st.nj[j]) // P) * G * j
            if not half:
                half = SFA
            nc.sync.dma_start(out=t_ews[:, :half], in_=ews[:, :half])
            if half < SFA:
                nc.sync.dma_start(out=t_ews[:, half:], in_=ews[:, half:])
            t_xo = apool.tile([P, nb * F1], F16)
            nc.sync.dma_start(out=t_xo[:], in_=x_own[:])
            t_deg = apool.tile([P, nb], F16)
            accf = 0
            accb = 0
            for j in range(1, st.jmax + 1):
                L = G * j
                nbj = int(st.nj[j]) // P
                with nc.allow_low_precision(
                    reason="fp16 deg within tolerance"
                ):
                    nc.vector.tensor_reduce(
                        out=t_deg[:, accb : accb + nbj],
                        in_=t_ews[:, accf : accf + nbj * L].rearrange(
                            "p (b l) -> p b l", l=L
                        ),
                        axis=AX,
                        op=ADD,
                    )
                accf += nbj * L
                accb += nbj
            # sqrt(deg + 1) in one activation: upcast + bias fused
            t_sq = apool.tile([P, nb], FP)
            nc.scalar.activation(
                t_sq[:], t_deg[:], mybir.ActivationFunctionType.Sqrt,
                bias=1.0,
            )
            t_r = apool.tile([P, nb], FP)
            nc.vector.reciprocal(t_r[:], t_sq[:])
            t_d16 = apool.tile([P, nb], F16)
            nc.scalar.copy(t_d16[:], t_r[:])
            nc.sync.dma_start(out=dis_o[:], in_=t_d16[:])
            t_xp = apool.tile([P, nb * F1], F16)
            nc.vector.tensor_tensor(
                t_xp[:].rearrange("p (b f) -> p b f", f=F1),
                t_xo[:].rearrange("p (b f) -> p b f", f=F1),
                t_d16[:].unsqueeze(2).broadcast_to([P, nb, F1]),
                MUL,
            )
            nc.sync.dma_start(out=xp_o[:], in_=t_xp[:])
    legalize_waits(nc)
    return nc


def _pb_geom(st):
    nb = st.ntot // P
    nbp = _ceil(nb, 128) * 128   # v4 node blocks padded (32 nodes/tile, 4-col)
    ntiles = nbp // 32           # 128-col transpose tiles of 4-stride v
    ncc = ntiles // 4            # 512-col W1 column chunks
    ne = ncc * 4                 # W1 matmuls / ht 512-col chunks
    nw2 = ne * 4                 # W2 matmuls
    npy = ncc * 6                # y PSUM tiles (3 outs at 0/32/64, per cc)
    return nb, nbp, ntiles, ncc, ne, nw2, npy


def build_PB(st):
    """xs + ews + xp_own + dis -> yst: packed dis*relu(vW1+b1)@W2 (fp16).

    v is kept 4-strided per node [b*4+f]; PE transposes 128-col tiles; W1
    is a block-diagonal [32,128] stationary (8 nodes per matmul, K row
    groups at partition bases 0/32/64 and a widened K=64 for the base-96
    group); bias+relu fuse into the scalar-engine PSUM eviction; W2 is
    block-diagonal [32,14]; dis scaling fuses into the final eviction."""
    nc = bass.Bass("TRN2", num_devices=S)
    nb, nbp, ntiles, ncc, ne, nw2, npy = _pb_geom(st)
    ews = nc.dram_tensor("ews", (P, st.SF1), F16, kind="ExternalInput")
    xs = nc.dram_tensor("xs", (P, st.SF3), F16, kind="ExternalInput")
    xpo4 = nc.dram_tensor("xpo4", (P, nbp * 4), F16, kind="ExternalInput")
    dis4 = nc.dram_tensor("dis4", (P, nbp * 4), F16, kind="ExternalInput")
    w1bd = nc.dram_tensor("w1bd", (P, 128), F16, kind="ExternalInput")
    w1bdh = nc.dram_tensor("w1bdh", (P, 128), F16, kind="ExternalInput")
    w2bd = nc.dram_tensor("w2bd", (P, 14), F16, kind="ExternalInput")
    w2bdh = nc.dram_tensor("w2bdh", (P, 14), F16, kind="ExternalInput")
    b1v = nc.dram_tensor("b1v", (P, 1), FP, kind="ExternalInput")
    disr = nc.dram_tensor("disr", (P, npy * 512), F16, kind="ExternalInput")
    yst_o = nc.dram_tensor("yst", (P, npy * 512), F16, kind="ExternalOutput")
    with tile.TileContext(nc) as tc:
        with tc.tile_pool(name="sb", bufs=3) as pool, tc.tile_pool(
            name="acc", bufs=1
        ) as apool, tc.tile_pool(
            name="ptr", bufs=2, space="PSUM"
        ) as tpool, tc.tile_pool(
            name="ph", bufs=2, space="PSUM"
        ) as hpool, tc.tile_pool(
            name="py", bufs=2, space="PSUM"
        ) as ypool:
            t_v4 = apool.tile([P, nbp * 4], F16)
            nc.gpsimd.memset(t_v4[:], 0.0)
            t_id = apool.tile([P, P], F16)
            make_identity(nc, t_id[:])
            t_ew = apool.tile([P, st.SF1], F16)
            t_xpo = apool.tile([P, nbp * 4], F16)
            nc.sync.dma_start(out=t_xpo[:], in_=xpo4[:])
            t_dis = apool.tile([P, nbp * 4], F16)
            nc.sync.dma_start(out=t_dis[:], in_=dis4[:])

            def agg_of(ck):
                gb0 = int(st.base_blk[ck.j]) + ck.b0
                view = t_v4[:, gb0 * 4 : (gb0 + ck.bl) * 4].rearrange(
                    "p (b f) -> p b f", f=4
                )[:, :, 0:F1]
                if ck.sm:
                    # fold's final add iterates (f, b)
                    return view.rearrange("p b f -> p f b")
                return view

            t_vt = apool.tile([P, ntiles * 128], F16)
            t_ht = apool.tile([P, ne * 512], F16)
            t_yst = apool.tile([P, npy * 512], F16)
            t_w1 = apool.tile([P, 128], F16)
            t_w1h = apool.tile([P, 128], F16)
            t_w2 = apool.tile([P, 14], F16)
            t_w2h = apool.tile([P, 14], F16)
            t_b1v = apool.tile([P, 1], FP)
            t_disr = apool.tile([P, npy * 512], F16)

            def mlp_part(cc):
                # v-prep for this 512-col slice, then the PE pipeline
                sl = slice(cc * 512, (cc + 1) * 512)
                nc.vector.tensor_tensor(
                    t_v4[:, sl], t_v4[:, sl], t_xpo[:, sl], ADD
                )
                nc.vector.tensor_tensor(
                    t_v4[:, sl], t_v4[:, sl], t_dis[:, sl], MUL
                )
                t_tr = tpool.tile([P, 512], F16, tag="tr")
                for gi in range(4):
                    tt = cc * 4 + gi
                    nc.tensor.transpose(
                        t_tr[:, gi * 128 : (gi + 1) * 128],
                        t_v4[:, tt * 128 : (tt + 1) * 128],
                        t_id[:],
                    )
                nc.scalar.copy(t_vt[:, sl], t_tr[:])
                for Q in range(4):
                    e = cc * 4 + Q
                    t_hp = hpool.tile([P, 512], FP, tag="h")
                    if Q < 3:
                        nc.tensor.matmul(
                            t_hp[:, :],
                            t_w1[32 * Q : 32 * Q + 32, :],
                            t_vt[32 * Q : 32 * Q + 32, sl],
                        )
                    else:
                        nc.tensor.matmul(
                            t_hp[:, :],
                            t_w1h[64:128, :],
                            t_vt[64:128, sl],
                        )
                    nc.scalar.activation(
                        t_ht[:, e * 512 : (e + 1) * 512],
                        t_hp[:, :],
                        RELU,
                        bias=t_b1v[:, 0:1],
                    )
                t_y = None
                for Q in range(4):
                    e = cc * 4 + Q
                    for R in range(4):
                        wl = Q * 4 + R
                        s = wl % 3
                        if s == 0:
                            t_y = ypool.tile([P, 512], FP, tag="y")
                        if R < 3:
                            nc.tensor.matmul(
                                t_y[32 * s : 32 * s + 14, :],
                                t_w2[32 * R : 32 * R + 32, :],
                                t_ht[32 * R : 32 * R + 32,
                                     e * 512 : (e + 1) * 512],
                            )
                        else:
                            nc.tensor.matmul(
                                t_y[32 * s : 32 * s + 14, :],
                                t_w2h[64:128, :],
                                t_ht[64:128, e * 512 : (e + 1) * 512],
                            )
                        if s == 2 or wl == 15:
                            pt = cc * 6 + wl // 3
                            nc.vector.tensor_tensor(
                                t_yst[:, pt * 512 : (pt + 1) * 512],
                                t_y[:, :],
                                t_disr[:, pt * 512 : (pt + 1) * 512],
                                MUL,
                            )

            # interleave: edge chunks needed by column-chunk cc, then its
            # MLP part, so the Tensor/Scalar pipeline overlaps the stream
            gs = [int(st.base_blk[ck.j]) + ck.b0 for ck in st.chunks]
            done = 0
            ew_state = {"done": 0}
            for cc in range(ncc):
                need = 128 * (cc + 1)
                hi = len(st.chunks)
                if cc < ncc - 1:
                    hi = next(
                        (i for i, g in enumerate(gs) if g >= need),
                        len(st.chunks),
                    )
                _edge_stream(nc, pool, st, F1, xs, t_ew, agg_of, "x",
                             shaped_fold=True,
                             subset=st.chunks[done:hi],
                             ews_dram=ews, ew_state=ew_state)
                if cc == 0:
                    # weights arrive while the first folds run
                    nc.sync.dma_start(out=t_w1[:], in_=w1bd[:])
                    nc.sync.dma_start(out=t_w1h[:], in_=w1bdh[:])
                    nc.sync.dma_start(out=t_w2[:], in_=w2bd[:])
                    nc.sync.dma_start(out=t_w2h[:], in_=w2bdh[:])
                    nc.sync.dma_start(out=t_b1v[:], in_=b1v[:])
                    nc.sync.dma_start(out=t_disr[:], in_=disr[:])
                done = hi
                mlp_part(cc)
                # ship this part's packed outputs while the next streams
                nc.sync.dma_start(
                    out=yst_o[:, cc * 6 * 512 : (cc + 1) * 6 * 512],
                    in_=t_yst[:, cc * 6 * 512 : (cc + 1) * 6 * 512],
                )
    legalize_waits(nc)
    return nc


def build_PE(st):
    """yss + ews + yso + dis7 + b2e -> out = dis*(agg2 + ys_own) + b2,
    everything in the chunk-major agg layout (host unscrambles)."""
    nc = bass.Bass("TRN2", num_devices=S)
    nb = st.ntot // P
    ews = nc.dram_tensor("ews", (P, st.SF1), F16, kind="ExternalInput")
    yss = nc.dram_tensor("yss", (P, st.SF7), F16, kind="ExternalInput")
    yso = nc.dram_tensor("yso", (P, nb * F2), F16, kind="ExternalInput")
    dis7 = nc.dram_tensor("dis7", (P, nb * F2), F16, kind="ExternalInput")
    if st.b2_nonzero:
        b2e = nc.dram_tensor("b2e", (P, nb * F2), F16, kind="ExternalInput")
    out_o = nc.dram_tensor("out", (P, nb * F2), F16, kind="ExternalOutput")
    with tile.TileContext(nc) as tc:
        with tc.tile_pool(name="sb", bufs=3) as pool, tc.tile_pool(
            name="acc", bufs=1
        ) as apool:
            t_ew = apool.tile([P, st.SF1], F16)
            t_yso = apool.tile([P, nb * F2], F16)
            t_dis7 = apool.tile([P, nb * F2], F16)
            t_b2e = apool.tile([P, nb * F2], F16)

            t_agg = apool.tile([P, nb * F2], F16)

            def agg_of(ck):
                sl = t_agg[:, ck.agg7 : ck.agg7 + ck.bl * F2]
                if ck.sm:
                    return sl
                return sl.rearrange("p (b f) -> p b f", f=F2)

            ew_state = {"done": 0}
            k1 = min(8, len(st.chunks) - 2)
            _edge_stream(nc, pool, st, F2, yss, t_ew, agg_of, "y",
                         subset=st.chunks[:k1],
                         ews_dram=ews, ew_state=ew_state)
            # node-level operands load while the stream runs (after the
            # big chunks are queued so they don't delay the ramp)
            nc.sync.dma_start(out=t_yso[:], in_=yso[:])
            nc.sync.dma_start(out=t_dis7[:], in_=dis7[:])
            if st.b2_nonzero:
                nc.sync.dma_start(out=t_b2e[:], in_=b2e[:])

            def finals(lo, hi):
                sl = slice(lo, hi)
                nc.vector.tensor_tensor(
                    t_agg[:, sl], t_agg[:, sl], t_yso[:, sl], ADD
                )
                nc.vector.tensor_tensor(
                    t_agg[:, sl], t_agg[:, sl], t_dis7[:, sl], MUL
                )
                if st.b2_nonzero:
                    nc.vector.tensor_tensor(
                        t_agg[:, sl], t_agg[:, sl], t_b2e[:, sl], ADD
                    )
                nc.sync.dma_start(out=out_o[:, sl], in_=t_agg[:, sl])

            # finals + output ship in two halves so the first overlaps the
            # tail of the stream
            half = max(k1, (2 * len(st.chunks)) // 3)
            _edge_stream(nc, pool, st, F2, yss, t_ew, agg_of, "y",
                         subset=st.chunks[k1:half],
                         ews_dram=ews, ew_state=ew_state)
            H = st.chunks[half].agg7 if half < len(st.chunks) else nb * F2
            finals(0, H)
            _edge_stream(nc, pool, st, F2, yss, t_ew, agg_of, "y",
                         subset=st.chunks[half:],
                         ews_dram=ews, ew_state=ew_state)
            if H < nb * F2:
                finals(H, nb * F2)
    legalize_waits(nc)
    return nc


# --------------------------------------------------------------------------
def kernel(x, edge_index, edge_weight, W1, b1, W2, b2):
    x = np.asarray(x, np.float32)
    ei = np.asarray(edge_index)
    ew = np.asarray(edge_weight, np.float32)
    W1 = np.asarray(W1, np.float32)
    b1 = np.asarray(b1, np.float32)
    W2 = np.asarray(W2, np.float32)
    b2 = np.asarray(b2, np.float32)

    PROGRAM_TIMES_NS.clear()
    st = build_structs(ei[0], ei[1], ew)
    st.b2_nonzero = bool(np.any(b2))
    nb = st.ntot // P
    _, nbp, ntiles, ncc, ne, nw2, npy = _pb_geom(st)
    ar = np.arange(NS)

    core_idx = []
    for c in range(S):
        cs = st.cores[c]
        p_e, off1, off3, fs3, off7, fs7 = edge_slots(st, cs)
        p_n, gb, nbase, nfstr = node_slots(st, cs)
        core_idx.append((cs, p_e, off1, off3, fs3, off7, fs7,
                         p_n, gb, nbase, nfstr))

    # node (p,gb) -> (row base before feature, column) in packed yst
    def yst_pos(p_n, gb):
        tc_ = gb // 32
        r32 = gb % 32
        Q = r32 // 8
        g = r32 % 8
        cc = tc_ // 4
        ci = (tc_ % 4) * 128 + p_n
        R = g // 2
        gp = g % 2
        wl = Q * 4 + R
        return 32 * (wl % 3) + 7 * gp, (cc * 6 + wl // 3) * 512 + ci

    # ---------------- P_A ----------------
    nc = build_PA(st)
    SFA = sum((int(st.nj[j]) // P) * G * j for j in range(1, st.jmax + 1))
    in_maps = []
    for c in range(S):
        cs = st.cores[c]
        p_e = core_idx[c][1]
        p_n, gb = core_idx[c][7], core_idx[c][8]
        # node-major per-class layout for PA's degree reduce
        j = cs.jcls[cs.ecol]
        q_local = cs.nodepos[cs.ecol] - st.base_node[j]
        nbj = st.nj[j] // P
        b = q_local % nbj
        base_free = np.zeros(st.jmax + 2, np.int64)
        for jj in range(1, st.jmax + 1):
            base_free[jj + 1] = base_free[jj] + (st.nj[jj] // P) * G * jj
        offa = base_free[j] + b * (G * j) + cs.kocc
        ews_pa = np.zeros((P, SFA), np.float16)
        ews_pa[p_e, offa] = cs.eew.astype(np.float16)
        x_own = np.zeros((P, nb, F1), np.float16)
        x_own[p_n, gb] = x[c * NS + ar].astype(np.float16)
        in_maps.append({"ews_pa": ews_pa, "x_own": x_own.reshape(P, nb * F1)})
    res = _run(nc, in_maps, "PA_deg")
    dis_l = [res[c]["dis"] for c in range(S)]
    xp_l = [res[c]["xp"] for c in range(S)]

    xp_can = np.zeros((N, F1), np.float16)
    dis_can = np.zeros(N, np.float16)
    for c in range(S):
        p_n, gb = core_idx[c][7], core_idx[c][8]
        xp_can[c * NS + ar] = xp_l[c].reshape(P, nb, F1)[p_n, gb]
        dis_can[c * NS + ar] = dis_l[c][p_n, gb]

    # ---------------- P_B (layer 1 + MLP) ----------------
    nc = build_PB(st)
    W1h = W1.astype(np.float16)
    W2h = W2.astype(np.float16)
    w1bdb = np.zeros((P, 128), np.float16)
    for Q in range(3):
        for gl in range(8):
            w1bdb[32 * Q + 4 * gl : 32 * Q + 4 * gl + 3,
                  16 * gl : 16 * gl + 16] = W1h
    w1bdhb = np.zeros((P, 128), np.float16)
    for gl in range(8):
        w1bdhb[96 + 4 * gl : 96 + 4 * gl + 3, 16 * gl : 16 * gl + 16] = W1h
    w2bdb = np.zeros((P, 14), np.float16)
    for R in range(3):
        for gp in range(2):
            w2bdb[32 * R + 16 * gp : 32 * R + 16 * gp + 16,
                  7 * gp : 7 * gp + 7] = W2h
    w2bdhb = np.zeros((P, 14), np.float16)
    for gp in range(2):
        w2bdhb[96 + 16 * gp : 96 + 16 * gp + 16, 7 * gp : 7 * gp + 7] = W2h
    b1vb = b1.astype(np.float32)[np.arange(P) % 16].reshape(P, 1)

    in_maps = []
    ews_l = []
    for c in range(S):
        cs = st.cores[c]
        p_e, off1, off3, fs3 = (core_idx[c][1], core_idx[c][2],
                                core_idx[c][3], core_idx[c][4])
        p_n, gb = core_idx[c][7], core_idx[c][8]
        ews = np.zeros((P, st.SF1), np.float16)
        ews[p_e, off1] = cs.eew.astype(np.float16)
        ews_l.append(ews)
        xs = np.zeros((P, st.SF3), np.float16)
        for fi in range(F1):
            xs[p_e, off3 + fi * fs3] = xp_can[cs.erow, fi]
        xpo4 = np.zeros((P, nbp, 4), np.float16)
        xpo4[p_n, gb, 0:F1] = xp_l[c].reshape(P, nb, F1)[p_n, gb]
        dis4 = np.zeros((P, nbp, 4), np.float16)
        dis4[p_n, gb, 0:F1] = dis_l[c][p_n, gb][:, None]
        rbase, col_ = yst_pos(p_n, gb)
        disr = np.zeros((P, npy * 512), np.float16)
        for r in range(F2):
            disr[rbase + r, col_] = dis_l[c][p_n, gb]
        in_maps.append(
            {
                "ews": ews,
                "xs": xs,
                "xpo4": xpo4.reshape(P, nbp * 4),
                "dis4": dis4.reshape(P, nbp * 4),
                "w1bd": w1bdb,
                "w1bdh": w1bdhb,
                "w2bd": w2bdb,
                "w2bdh": w2bdhb,
                "b1v": b1vb,
                "disr": disr,
            }
        )
    res = _run(nc, in_maps, "PB_layer1")
    yst_l = [res[c]["yst"] for c in range(S)]

    ys_can = np.zeros((N, F2), np.float16)
    for c in range(S):
        p_n, gb = core_idx[c][7], core_idx[c][8]
        rbase, col_ = yst_pos(p_n, gb)
        v = np.empty((NS, F2), np.float16)
        for r in range(F2):
            v[:, r] = yst_l[c][rbase + r, col_]
        ys_can[c * NS + ar] = v

    # ---------------- P_E (layer 2) ----------------
    nc = build_PE(st)
    b2h = b2.astype(np.float16)
    in_maps = []
    for c in range(S):
        cs = st.cores[c]
        p_e, off7, fs7 = core_idx[c][1], core_idx[c][5], core_idx[c][6]
        p_n, nbase, nfstr = core_idx[c][7], core_idx[c][9], core_idx[c][10]
        yss = np.zeros((P, st.SF7), np.float16)
        for fi in range(F2):
            yss[p_e, off7 + fi * fs7] = ys_can[cs.erow, fi]
        yso = np.zeros((P, nb * F2), np.float16)
        dis7 = np.zeros((P, nb * F2), np.float16)
        ysl = ys_can[c * NS + ar]
        disl = dis_can[c * NS + ar]
        for fi in range(F2):
            yso[p_n, nbase + fi * nfstr] = ysl[:, fi]
            dis7[p_n, nbase + fi * nfstr] = disl
        im = {"ews": ews_l[c], "yss": yss, "yso": yso, "dis7": dis7}
        if st.b2_nonzero:
            b2e = np.zeros((P, nb * F2), np.float16)
            for fi in range(F2):
                b2e[p_n, nbase + fi * nfstr] = b2h[fi]
            im["b2e"] = b2e
        in_maps.append(im)
    res = _run(nc, in_maps, "PE_layer2")

    out = np.zeros((N, F2), np.float32)
    for c in range(S):
        o = res[c]["out"]
        p_n, nbase, nfstr = core_idx[c][7], core_idx[c][9], core_idx[c][10]
        for fi in range(F2):
            out[c * NS + ar, fi] = o[p_n, nbase + fi * nfstr].astype(
                np.float32
            )
    return out


# revision 6
# speedup vs baseline: 1.1530x; 1.0015x over previous
"""GCN (2-layer) on 8 Trainium2 NeuronCores — v4 (3 device programs).

Graph/data parallel per the node-range sharding hint: nodes sharded by
range, edges live on the destination core, weights replicated.  All
irregular routing happens on the HOST as pure copies/permutations;
every FP arithmetic op on values runs on device.

- Destination nodes bucketed by in-degree class j=ceil(d/4); each node
  gets exactly 4j slots so segment-sum becomes a slot-axis reduction.
- Big chunks use a SLOT-MAJOR [l, f, b] layout: the ew multiply and a
  tree of tensor_tensor adds are then fully contiguous fp16 APs, which
  is what the DVE 2x packed mode requires on hardware.  Small chunks
  stay node-major with one 1x tensor_reduce (fewer instructions).
- Node values feeding edges are pre-scaled on device (x'=dis*x,
  ys=dis*relu(vW1+b1)W2) so the per-edge device math is one multiply.
- PB's whole MLP runs on the Tensor engine: v is transposed via the PE,
  W1 is applied as a block-diagonal [32,128] stationary (8 nodes per
  matmul), bias+relu ride the scalar-engine PSUM eviction, W2 likewise
  block-diagonal [32,14]; dis scaling fuses into the final eviction.
"""
import sys

sys.path.insert(0, "/opt/trn_rl_repo")

import numpy as np

import bass_rust
from concourse import bass, mybir
from concourse.bass_utils import run_bass_kernel_spmd
from concourse.masks import make_identity
import concourse.tile as tile

import os as _os

PROGRAM_TIMES_NS = []   # (name, exec_time_ns) per device program of last kernel() call


def _enable_tracing():
    import types
    import antenv
    if 'antenv.axon_hooks' in sys.modules:
        return True
    try:
        from trn_agent_boot.trn_boot import _ntff_profile_via_ctypes
        hook = _ntff_profile_via_ctypes('/opt/axon/libaxon_pjrt.so')
    except Exception:
        return False
    mod = types.ModuleType('antenv.axon_hooks')
    mod.get_axon_ntff_profile_hook = lambda: hook
    mod.set_axon_ntff_profile_hook = lambda h: None
    sys.modules['antenv.axon_hooks'] = mod
    antenv.axon_hooks = mod
    import concourse.bass_utils as _bu
    _bu.upload_artifacts = lambda tmpdir: f"local://{tmpdir}"
    return True


def _run(nc, in_maps, name):
    trace = bool(_os.environ.get('GCN_TRACE')) and _enable_tracing()
    r = run_bass_kernel_spmd(nc, in_maps, core_ids=CORE_IDS, trace=trace)
    if trace:
        PROGRAM_TIMES_NS.append((name, r.exec_time_ns))
    return r.results

S = 8
N = 200000
NS = N // S
P = 128
G = 4            # degree-class granularity: slots per node = G*ceil(d/G)
F1 = 3
F2 = 7
CORE_IDS = list(range(S))
FP = mybir.dt.float32
F16 = mybir.dt.float16
MUL = mybir.AluOpType.mult
ADD = mybir.AluOpType.add
AX = mybir.AxisListType.X
RELU = mybir.ActivationFunctionType.Relu

_CHUNK7 = 16384   # chunk budget in F2-elems per partition
_SM_MIN = 1000    # below this (F2-elems), node-major + tensor_reduce


def _ceil(a, b):
    return -(-a // b)


# --------------------------------------------------------------------------
# walrus on this toolchain accepts at most ONE sync-wait per instruction;
# Tile emits several at DAG joins / kernel-tail drain. Hoist excess waits
# onto fresh same-engine NoOps inserted immediately before the violator.
def legalize_waits(nc):
    nop_idx = 0
    for f in nc.m.functions:
        for bb in f.blocks:
            il = bb.instructions
            if not any(
                inst.sync_info is not None
                and len(inst.sync_info.on_wait or []) > 1
                for inst in il
            ):
                continue
            new_il = []
            for inst in il:
                si = inst.sync_info
                w = list(si.on_wait or []) if si is not None else []
                if len(w) > 1:
                    for extra in w[:-1]:
                        nop = mybir.InstNoOp(
                            name=f"I-waitsplit-{nop_idx}", ins=[], outs=[]
                        )
                        nop_idx += 1
                        nop.engine = inst.engine
                        nop.sync_info = bass_rust.SyncInfo(
                            on_wait=[extra], on_update=[]
                        )
                        new_il.append(nop)
                    si.on_wait = [w[-1]]
                new_il.append(inst)
            bb.instructions = new_il


# --------------------------------------------------------------------------
# host-side structure building (integer routing only)
class _O:
    pass


def build_structs(row, col, ew):
    row = row.astype(np.int64)
    col = col.astype(np.int64)
    cores = []
    for c in range(S):
        cs = _O()
        m = (col // NS) == c
        cs.erow = row[m]
        cs.ecol = (col[m] - c * NS).astype(np.int64)
        cs.eew = ew[m].astype(np.float32)
        cores.append(cs)

    for cs in cores:
        d = np.bincount(cs.ecol, minlength=NS)
        cs.jcls = np.maximum(1, _ceil(np.maximum(d, 1), G))
    jmax = max(int(cs.jcls.max()) for cs in cores)
    nj = np.zeros(jmax + 1, np.int64)
    for j in range(1, jmax + 1):
        njc = max(int((cs.jcls == j).sum()) for cs in cores)
        nj[j] = _ceil(max(njc, 1), P) * P
    for cs in cores:
        nodepos = np.full(NS, -1, np.int64)
        pos = 0
        for j in range(1, jmax + 1):
            nodes = np.nonzero(cs.jcls == j)[0]
            nodepos[nodes] = pos + np.arange(len(nodes))
            pos += nj[j]
        cs.nodepos = nodepos
    ntot = int(nj[1:].sum())

    st = _O()
    st.jmax, st.nj, st.ntot = jmax, nj, ntot
    st.cores = cores

    # per-edge occurrence index among edges sharing a destination
    for cs in cores:
        nodes = cs.ecol
        ord_ = np.argsort(nodes, kind="stable")
        ns = nodes[ord_]
        first = np.r_[True, ns[1:] != ns[:-1]]
        idx_of_first = np.maximum.accumulate(
            np.where(first, np.arange(len(ord_)), 0)
        )
        k = np.empty(len(nodes), np.int64)
        k[ord_] = np.arange(len(ord_)) - idx_of_first
        cs.kocc = k

    # class bases
    st.base_node = np.zeros(jmax + 2, np.int64)
    st.base_blk = np.zeros(jmax + 2, np.int64)
    for j in range(1, jmax + 1):
        st.base_node[j + 1] = st.base_node[j] + nj[j]
        st.base_blk[j + 1] = st.base_blk[j] + nj[j] // P

    # shared chunk plan (same block ranges for all three per-edge tensors);
    # carve a small GpSimd-sized chunk off the three biggest classes
    # (GpSimd offload measured net-negative: concurrent GpSimd SBUF traffic
    # halves the DVE 2x packed-mode rate, so no chunks are carved for it)
    gp_classes = set()
    chunks = []
    off1 = off3 = off7 = agg7 = 0
    for j in range(1, jmax + 1):
        L = G * j
        nbj = int(nj[j]) // P
        blmax = max(2, (_CHUNK7 // (F2 * L)) & ~1)
        gp_left = 14 if (j in gp_classes and nbj >= 40) else 0
        b0 = 0
        while b0 < nbj:
            if gp_left:
                bl, gp = gp_left, True
                gp_left = 0
            else:
                bl, gp = min(blmax, nbj - b0), False
                if bl % 2 == 1 and bl > 1 and bl * F2 * L >= _SM_MIN:
                    bl -= 1   # keep the big chunk even (slot-major capable)
            sm = (bl % 2 == 0) and (gp or bl * F2 * L >= _SM_MIN)
            ck = _O()
            ck.j, ck.L, ck.b0, ck.bl, ck.sm, ck.gp = j, L, b0, bl, sm, gp
            ck.off1, ck.off3, ck.off7, ck.agg7 = off1, off3, off7, agg7
            chunks.append(ck)
            off1 += bl * L
            off3 += bl * F1 * L
            off7 += bl * F2 * L
            agg7 += bl * F2
            b0 += bl
    st.chunks = chunks
    st.SF1, st.SF3, st.SF7 = off1, off3, off7
    assert agg7 == (ntot // P) * F2

    # per-class chunk lookup tables (by block index)
    st.cmap = {}
    for j in range(1, jmax + 1):
        nbj = int(nj[j]) // P
        cid = np.zeros(nbj, np.int64)
        for i, ck in enumerate(chunks):
            if ck.j == j:
                cid[ck.b0 : ck.b0 + ck.bl] = i
        st.cmap[j] = cid
    return st


def edge_slots(st, cs):
    """per edge: partition p, and for each tensor the flat offset of the
    (slot k, feature 0) element plus the per-feature stride."""
    nodes = cs.ecol
    j = cs.jcls[nodes]
    q_local = cs.nodepos[nodes] - st.base_node[j]
    nbj = st.nj[j] // P
    p = q_local // nbj
    b = q_local % nbj
    k = cs.kocc
    ne = len(nodes)
    off1 = np.empty(ne, np.int64)
    off3 = np.empty(ne, np.int64)
    off7 = np.empty(ne, np.int64)
    fs3 = np.empty(ne, np.int64)
    fs7 = np.empty(ne, np.int64)
    for jj in range(1, st.jmax + 1):
        sel = j == jj
        if not sel.any():
            continue
        L = G * jj
        ci = st.cmap[jj][b[sel]]
        cb0 = np.array([st.chunks[i].b0 for i in range(len(st.chunks))])
        cbl = np.array([st.chunks[i].bl for i in range(len(st.chunks))])
        csm = np.array([st.chunks[i].sm for i in range(len(st.chunks))])
        c1 = np.array([st.chunks[i].off1 for i in range(len(st.chunks))])
        c3 = np.array([st.chunks[i].off3 for i in range(len(st.chunks))])
        c7 = np.array([st.chunks[i].off7 for i in range(len(st.chunks))])
        bo = b[sel] - cb0[ci]
        bl = cbl[ci]
        sm = csm[ci]
        kk = k[sel]
        # slot-major: base + k*(F*bl) + f*bl + bo ; node-major: bo*(F*L)+f*L+k
        off1[sel] = np.where(sm, c1[ci] + kk * bl + bo,
                             c1[ci] + bo * L + kk)
        off3[sel] = np.where(sm, c3[ci] + kk * (F1 * bl) + bo,
                             c3[ci] + bo * (F1 * L) + kk)
        off7[sel] = np.where(sm, c7[ci] + kk * (F2 * bl) + bo,
                             c7[ci] + bo * (F2 * L) + kk)
        fs3[sel] = np.where(sm, bl, L)
        fs7[sel] = np.where(sm, bl, L)
    return p, off1, off3, fs3, off7, fs7


def node_slots(st, cs):
    """per local node: partition p, global block gb, and the (base, fstride)
    of its features in the chunk-major agg7 layout."""
    j = cs.jcls
    q_local = cs.nodepos - st.base_node[j]
    nbj = st.nj[j] // P
    p = q_local // nbj
    b = q_local % nbj
    gb = st.base_blk[j] + b
    ns = len(j)
    base = np.empty(ns, np.int64)
    fstr = np.empty(ns, np.int64)
    cb0 = np.array([c.b0 for c in st.chunks])
    cbl = np.array([c.bl for c in st.chunks])
    csm = np.array([c.sm for c in st.chunks])
    ca7 = np.array([c.agg7 for c in st.chunks])
    for jj in range(1, st.jmax + 1):
        sel = j == jj
        if not sel.any():
            continue
        ci = st.cmap[jj][b[sel]]
        bo = b[sel] - cb0[ci]
        base[sel] = np.where(csm[ci], ca7[ci] + bo,
                             ca7[ci] + bo * F2)
        fstr[sel] = np.where(csm[ci], cbl[ci], 1)
    return p, gb, base, fstr


# --------------------------------------------------------------------------
# device-program helpers
def _fold_flat(nc, eng, t_in, FB, L, out_ap, fshape=None):
    """tree-fold the leading slot axis of a slot-major chunk (viewed as
    [P, L*FB] with l outermost): contiguous halves, all 2x; the final add
    writes out_ap (free size FB; if fshape=(F, bl) the inputs are viewed
    [P, F, bl] to match a shaped/strided out_ap)."""
    tv = t_in[:]
    cur = L
    while cur > 2:
        if cur % 2 == 1:
            eng.tensor_tensor(
                tv[:, 0:FB], tv[:, 0:FB],
                tv[:, (cur - 1) * FB : cur * FB], ADD,
            )
            cur -= 1
            if cur == 2:
                break
        h = cur // 2
        eng.tensor_tensor(
            tv[:, 0 : h * FB], tv[:, 0 : h * FB],
            tv[:, h * FB : cur * FB], ADD,
        )
        cur = h
    i0, i1 = tv[:, 0:FB], tv[:, FB : 2 * FB]
    if fshape is not None:
        F, bl = fshape
        i0 = i0.rearrange("p (f b) -> p f b", f=F)
        i1 = i1.rearrange("p (f b) -> p f b", f=F)
    eng.tensor_tensor(out_ap, i0, i1, ADD)


def _edge_stream(nc, pool, st, F, xs, t_ew, agg_of, tag, shaped_fold=False,
                 subset=None, ews_dram=None, ew_state=None):
    """Stream per-edge fp16 payload chunks, multiply by the ew slots and
    reduce the slot axis.  agg_of(ck) -> output AP (free size bl*F) in the
    chosen agg layout for that chunk.  If ews_dram is given, the ew slots
    are DMA'd just-in-time in pieces right before the chunks needing them."""
    for ck in (subset if subset is not None else st.chunks):
        L, bl = ck.L, ck.bl
        eng = nc.vector
        if ews_dram is not None:
            need = ck.off1 + bl * L
            if need > ew_state["done"]:
                end = max(need, min(st.SF1, ew_state["done"] + 2048))
                nc.sync.dma_start(
                    out=t_ew[:, ew_state["done"] : end],
                    in_=ews_dram[:, ew_state["done"] : end],
                )
                ew_state["done"] = end
        offF = ck.off1 if F == 1 else (ck.off3 if F == F1 else ck.off7)
        n = bl * F * L
        t_in = pool.tile([P, n], F16, tag=tag)
        nc.sync.dma_start(out=t_in[:], in_=xs[:, offF : offF + n])
        if ck.sm:
            FB = F * bl
            if t_ew is not None:
                eng.tensor_tensor(
                    t_in[:].rearrange("p (l f b) -> p l f b", l=L, f=F),
                    t_in[:].rearrange("p (l f b) -> p l f b", l=L, f=F),
                    t_ew[:, ck.off1 : ck.off1 + bl * L]
                    .rearrange("p (l b) -> p l b", l=L)
                    .unsqueeze(2)
                    .broadcast_to([P, L, F, bl]),
                    MUL,
                )
            _fold_flat(nc, eng, t_in, FB, L, agg_of(ck),
                       fshape=(F, bl) if shaped_fold else None)
        else:
            if t_ew is not None:
                eng.tensor_tensor(
                    t_in[:].rearrange("p (b f l) -> p b f l", f=F, l=L),
                    t_in[:].rearrange("p (b f l) -> p b f l", f=F, l=L),
                    t_ew[:, ck.off1 : ck.off1 + bl * L]
                    .rearrange("p (b l) -> p b l", l=L)
                    .unsqueeze(2)
                    .broadcast_to([P, bl, F, L]),
                    MUL,
                )
            with nc.allow_low_precision(reason="fp16 agg within tolerance"):
                eng.tensor_reduce(
                    out=agg_of(ck),
                    in_=t_in[:].rearrange("p (b f l) -> p b f l", f=F, l=L),
                    axis=AX,
                    op=ADD,
                )


def build_PA(st):
    """ews_pa (node-major, per class) + x_own -> dis (fp16), xp = dis*x."""
    nc = bass.Bass("TRN2", num_devices=S)
    nb = st.ntot // P
    SFA = sum((int(st.nj[j]) // P) * G * j for j in range(1, st.jmax + 1))
    ews = nc.dram_tensor("ews_pa", (P, SFA), F16, kind="ExternalInput")
    x_own = nc.dram_tensor("x_own", (P, nb * F1), F16, kind="ExternalInput")
    dis_o = nc.dram_tensor("dis", (P, nb), F16, kind="ExternalOutput")
    xp_o = nc.dram_tensor("xp", (P, nb * F1), F16, kind="ExternalOutput")
    with tile.TileContext(nc) as tc:
        with tc.tile_pool(name="acc", bufs=1) as apool:
            # whole ews resident; two DMA pieces so reduces start early
            t_ews = apool.tile([P, SFA], F16)
            half = 0
            accf = 0
            for j in range(1, st.jmax + 1):
                if accf >= SFA // 2:
                    half = accf
                    break
                accf += (int(st.nj[j]) // P) * G * j
            if not half:
                half = SFA
            nc.sync.dma_start(out=t_ews[:, :half], in_=ews[:, :half])
            if half < SFA:
                nc.sync.dma_start(out=t_ews[:, half:], in_=ews[:, half:])
            t_xo = apool.tile([P, nb * F1], F16)
            nc.sync.dma_start(out=t_xo[:], in_=x_own[:])
            t_deg = apool.tile([P, nb], F16)
            accf = 0
            accb = 0
            for j in range(1, st.jmax + 1):
                L = G * j
                nbj = int(st.nj[j]) // P
                with nc.allow_low_precision(
                    reason="fp16 deg within tolerance"
                ):
                    nc.vector.tensor_reduce(
                        out=t_deg[:, accb : accb + nbj],
                        in_=t_ews[:, accf : accf + nbj * L].rearrange(
                            "p (b l) -> p b l", l=L
                        ),
                        axis=AX,
                        op=ADD,
                    )
                accf += nbj * L
                accb += nbj
            # sqrt(deg + 1) in one activation: upcast + bias fused
            t_sq = apool.tile([P, nb], FP)
            nc.scalar.activation(
                t_sq[:], t_deg[:], mybir.ActivationFunctionType.Sqrt,
                bias=1.0,
            )
            t_r = apool.tile([P, nb], FP)
            nc.vector.reciprocal(t_r[:], t_sq[:])
            t_d16 = apool.tile([P, nb], F16)
            nc.scalar.copy(t_d16[:], t_r[:])
            nc.sync.dma_start(out=dis_o[:], in_=t_d16[:])
            t_xp = apool.tile([P, nb * F1], F16)
            nc.vector.tensor_tensor(
                t_xp[:].rearrange("p (b f) -> p b f", f=F1),
                t_xo[:].rearrange("p (b f) -> p b f", f=F1),
                t_d16[:].unsqueeze(2).broadcast_to([P, nb, F1]),
                MUL,
            )
            nc.sync.dma_start(out=xp_o[:], in_=t_xp[:])
    legalize_waits(nc)
    return nc


def _pb_geom(st):
    nb = st.ntot // P
    nbp = _ceil(nb, 128) * 128   # v4 node blocks padded (32 nodes/tile, 4-col)
    ntiles = nbp // 32           # 128-col transpose tiles of 4-stride v
    ncc = ntiles // 4            # 512-col W1 column chunks
    ne = ncc * 4                 # W1 matmuls / ht 512-col chunks
    nw2 = ne * 4                 # W2 matmuls
    npy = ncc * 6                # y PSUM tiles (3 outs at 0/32/64, per cc)
    return nb, nbp, ntiles, ncc, ne, nw2, npy


def build_PB(st):
    """xs + ews + xp_own + dis -> yst: packed dis*relu(vW1+b1)@W2 (fp16).

    v is kept 4-strided per node [b*4+f]; PE transposes 128-col tiles; W1
    is a block-diagonal [32,128] stationary (8 nodes per matmul, K row
    groups at partition bases 0/32/64 and a widened K=64 for the base-96
    group); bias+relu fuse into the scalar-engine PSUM eviction; W2 is
    block-diagonal [32,14]; dis scaling fuses into the final eviction."""
    nc = bass.Bass("TRN2", num_devices=S)
    nb, nbp, ntiles, ncc, ne, nw2, npy = _pb_geom(st)
    ews = nc.dram_tensor("ews", (P, st.SF1), F16, kind="ExternalInput")
    xs = nc.dram_tensor("xs", (P, st.SF3), F16, kind="ExternalInput")
    xpo4 = nc.dram_tensor("xpo4", (P, nbp * 4), F16, kind="ExternalInput")
    dis4 = nc.dram_tensor("dis4", (P, nbp * 4), F16, kind="ExternalInput")
    w1bd = nc.dram_tensor("w1bd", (P, 128), F16, kind="ExternalInput")
    w1bdh = nc.dram_tensor("w1bdh", (P, 128), F16, kind="ExternalInput")
    w2bd = nc.dram_tensor("w2bd", (P, 14), F16, kind="ExternalInput")
    w2bdh = nc.dram_tensor("w2bdh", (P, 14), F16, kind="ExternalInput")
    b1v = nc.dram_tensor("b1v", (P, 1), FP, kind="ExternalInput")
    disr = nc.dram_tensor("disr", (P, npy * 512), F16, kind="ExternalInput")
    yst_o = nc.dram_tensor("yst", (P, npy * 512), F16, kind="ExternalOutput")
    with tile.TileContext(nc) as tc:
        with tc.tile_pool(name="sb", bufs=3) as pool, tc.tile_pool(
            name="acc", bufs=1
        ) as apool, tc.tile_pool(
            name="ptr", bufs=2, space="PSUM"
        ) as tpool, tc.tile_pool(
            name="ph", bufs=2, space="PSUM"
        ) as hpool, tc.tile_pool(
            name="py", bufs=2, space="PSUM"
        ) as ypool:
            t_v4 = apool.tile([P, nbp * 4], F16)
            nc.gpsimd.memset(t_v4[:], 0.0)
            t_id = apool.tile([P, P], F16)
            make_identity(nc, t_id[:])
            t_ew = apool.tile([P, st.SF1], F16)
            t_xpo = apool.tile([P, nbp * 4], F16)
            t_dis = apool.tile([P, nbp * 4], F16)

            def agg_of(ck):
                gb0 = int(st.base_blk[ck.j]) + ck.b0
                view = t_v4[:, gb0 * 4 : (gb0 + ck.bl) * 4].rearrange(
                    "p (b f) -> p b f", f=4
                )[:, :, 0:F1]
                if ck.sm:
                    # fold's final add iterates (f, b)
                    return view.rearrange("p b f -> p f b")
                return view

            t_vt = apool.tile([P, ntiles * 128], F16)
            t_ht = apool.tile([P, ne * 512], F16)
            t_yst = apool.tile([P, npy * 512], F16)
            t_w1 = apool.tile([P, 128], F16)
            t_w1h = apool.tile([P, 128], F16)
            t_w2 = apool.tile([P, 14], F16)
            t_w2h = apool.tile([P, 14], F16)
            t_b1v = apool.tile([P, 1], FP)
            t_disr = apool.tile([P, npy * 512], F16)

            def mlp_part(cc):
                # v-prep for this 512-col slice, then the PE pipeline
                sl = slice(cc * 512, (cc + 1) * 512)
                nc.vector.tensor_tensor(
                    t_v4[:, sl], t_v4[:, sl], t_xpo[:, sl], ADD
                )
                nc.vector.tensor_tensor(
                    t_v4[:, sl], t_v4[:, sl], t_dis[:, sl], MUL
                )
                t_tr = tpool.tile([P, 512], F16, tag="tr")
                for gi in range(4):
                    tt = cc * 4 + gi
                    nc.tensor.transpose(
                        t_tr[:, gi * 128 : (gi + 1) * 128],
                        t_v4[:, tt * 128 : (tt + 1) * 128],
                        t_id[:],
                    )
                nc.scalar.copy(t_vt[:, sl], t_tr[:])
                t_y = None
                for Q in range(4):
                    e = cc * 4 + Q
                    t_hp = hpool.tile([P, 512], FP, tag="h")
                    if Q < 3:
                        nc.tensor.matmul(
                            t_hp[:, :],
                            t_w1[32 * Q : 32 * Q + 32, :],
                            t_vt[32 * Q : 32 * Q + 32, sl],
                        )
                    else:
                        nc.tensor.matmul(
                            t_hp[:, :],
                            t_w1h[64:128, :],
                            t_vt[64:128, sl],
                        )
                    nc.scalar.activation(
                        t_ht[:, e * 512 : (e + 1) * 512],
                        t_hp[:, :],
                        RELU,
                        bias=t_b1v[:, 0:1],
                    )
                    # W2 for this group rides right behind its relu so the
                    # per-part chain pipelines across engines
                    for R in range(4):
                        wl = Q * 4 + R
                        s = wl % 3
                        if s == 0:
                            t_y = ypool.tile([P, 512], FP, tag="y")
                        if R < 3:
                            nc.tensor.matmul(
                                t_y[32 * s : 32 * s + 14, :],
                                t_w2[32 * R : 32 * R + 32, :],
                                t_ht[32 * R : 32 * R + 32,
                                     e * 512 : (e + 1) * 512],
                            )
                        else:
                            nc.tensor.matmul(
                                t_y[32 * s : 32 * s + 14, :],
                                t_w2h[64:128, :],
                                t_ht[64:128, e * 512 : (e + 1) * 512],
                            )
                        if s == 2 or wl == 15:
                            pt = cc * 6 + wl // 3
                            nc.vector.tensor_tensor(
                                t_yst[:, pt * 512 : (pt + 1) * 512],
                                t_y[:, :],
                                t_disr[:, pt * 512 : (pt + 1) * 512],
                                MUL,
                            )

            # interleave: edge chunks needed by column-chunk cc, then its
            # MLP part, so the Tensor/Scalar pipeline overlaps the stream
            gs = [int(st.base_blk[ck.j]) + ck.b0 for ck in st.chunks]
            done = 0
            ew_state = {"done": 0}
            for cc in range(ncc):
                need = 128 * (cc + 1)
                hi = len(st.chunks)
                if cc < ncc - 1:
                    hi = next(
                        (i for i, g in enumerate(gs) if g >= need),
                        len(st.chunks),
                    )
                if cc == 0:
                    # node operands + weights arrive mid-stream, after the
                    # first chunks are queued but before the MLP needs them
                    mid = max(done + 1, (done + hi) // 2)
                    _edge_stream(nc, pool, st, F1, xs, t_ew, agg_of, "x",
                                 shaped_fold=True,
                                 subset=st.chunks[done:mid],
                                 ews_dram=ews, ew_state=ew_state)
                    nc.sync.dma_start(out=t_xpo[:], in_=xpo4[:])
                    nc.sync.dma_start(out=t_dis[:], in_=dis4[:])
                    nc.sync.dma_start(out=t_w1[:], in_=w1bd[:])
                    nc.sync.dma_start(out=t_w1h[:], in_=w1bdh[:])
                    nc.sync.dma_start(out=t_w2[:], in_=w2bd[:])
                    nc.sync.dma_start(out=t_w2h[:], in_=w2bdh[:])
                    nc.sync.dma_start(out=t_b1v[:], in_=b1v[:])
                    nc.sync.dma_start(out=t_disr[:], in_=disr[:])
                    done = mid
                _edge_stream(nc, pool, st, F1, xs, t_ew, agg_of, "x",
                             shaped_fold=True,
                             subset=st.chunks[done:hi],
                             ews_dram=ews, ew_state=ew_state)
                done = hi
                mlp_part(cc)
                # ship this part's packed outputs while the next streams
                nc.sync.dma_start(
                    out=yst_o[:, cc * 6 * 512 : (cc + 1) * 6 * 512],
                    in_=t_yst[:, cc * 6 * 512 : (cc + 1) * 6 * 512],
                )
    legalize_waits(nc)
    return nc


def build_PE(st):
    """yss + ews + yso + dis7 + b2e -> out = dis*(agg2 + ys_own) + b2,
    everything in the chunk-major agg layout (host unscrambles)."""
    nc = bass.Bass("TRN2", num_devices=S)
    nb = st.ntot // P
    ews = nc.dram_tensor("ews", (P, st.SF1), F16, kind="ExternalInput")
    yss = nc.dram_tensor("yss", (P, st.SF7), F16, kind="ExternalInput")
    yso = nc.dram_tensor("yso", (P, nb * F2), F16, kind="ExternalInput")
    dis7 = nc.dram_tensor("dis7", (P, nb * F2), F16, kind="ExternalInput")
    if st.b2_nonzero:
        b2e = nc.dram_tensor("b2e", (P, nb * F2), F16, kind="ExternalInput")
    out_o = nc.dram_tensor("out", (P, nb * F2), F16, kind="ExternalOutput")
    with tile.TileContext(nc) as tc:
        with tc.tile_pool(name="sb", bufs=3) as pool, tc.tile_pool(
            name="acc", bufs=1
        ) as apool:
            t_ew = apool.tile([P, st.SF1], F16)
            t_yso = apool.tile([P, nb * F2], F16)
            t_dis7 = apool.tile([P, nb * F2], F16)
            t_b2e = apool.tile([P, nb * F2], F16)

            t_agg = apool.tile([P, nb * F2], F16)

            def agg_of(ck):
                sl = t_agg[:, ck.agg7 : ck.agg7 + ck.bl * F2]
                if ck.sm:
                    return sl
                return sl.rearrange("p (b f) -> p b f", f=F2)

            ew_state = {"done": 0}
            k1 = min(8, len(st.chunks) - 2)
            _edge_stream(nc, pool, st, F2, yss, t_ew, agg_of, "y",
                         subset=st.chunks[:k1],
                         ews_dram=ews, ew_state=ew_state)
            # node-level operands load while the stream runs (after the
            # big chunks are queued so they don't delay the ramp)
            nc.sync.dma_start(out=t_yso[:], in_=yso[:])
            nc.sync.dma_start(out=t_dis7[:], in_=dis7[:])
            if st.b2_nonzero:
                nc.sync.dma_start(out=t_b2e[:], in_=b2e[:])

            def finals(lo, hi):
                sl = slice(lo, hi)
                nc.vector.tensor_tensor(
                    t_agg[:, sl], t_agg[:, sl], t_yso[:, sl], ADD
                )
                nc.vector.tensor_tensor(
                    t_agg[:, sl], t_agg[:, sl], t_dis7[:, sl], MUL
                )
                if st.b2_nonzero:
                    nc.vector.tensor_tensor(
                        t_agg[:, sl], t_agg[:, sl], t_b2e[:, sl], ADD
                    )
                nc.sync.dma_start(out=out_o[:, sl], in_=t_agg[:, sl])

            # finals + output ship in two halves so the first overlaps the
            # tail of the stream
            half = max(k1, (2 * len(st.chunks)) // 3)
            _edge_stream(nc, pool, st, F2, yss, t_ew, agg_of, "y",
                         subset=st.chunks[k1:half],
                         ews_dram=ews, ew_state=ew_state)
            H = st.chunks[half].agg7 if half < len(st.chunks) else nb * F2
            finals(0, H)
            _edge_stream(nc, pool, st, F2, yss, t_ew, agg_of, "y",
                         subset=st.chunks[half:],
                         ews_dram=ews, ew_state=ew_state)
            if H < nb * F2:
                finals(H, nb * F2)
    legalize_waits(nc)
    return nc


# --------------------------------------------------------------------------
def kernel(x, edge_index, edge_weight, W1, b1, W2, b2):
    x = np.asarray(x, np.float32)
    ei = np.asarray(edge_index)
    ew = np.asarray(edge_weight, np.float32)
    W1 = np.asarray(W1, np.float32)
    b1 = np.asarray(b1, np.float32)
    W2 = np.asarray(W2, np.float32)
    b2 = np.asarray(b2, np.float32)

    PROGRAM_TIMES_NS.clear()
    st = build_structs(ei[0], ei[1], ew)
    st.b2_nonzero = bool(np.any(b2))
    nb = st.ntot // P
    _, nbp, ntiles, ncc, ne, nw2, npy = _pb_geom(st)
    ar = np.arange(NS)

    core_idx = []
    for c in range(S):
        cs = st.cores[c]
        p_e, off1, off3, fs3, off7, fs7 = edge_slots(st, cs)
        p_n, gb, nbase, nfstr = node_slots(st, cs)
        core_idx.append((cs, p_e, off1, off3, fs3, off7, fs7,
                         p_n, gb, nbase, nfstr))

    # node (p,gb) -> (row base before feature, column) in packed yst
    def yst_pos(p_n, gb):
        tc_ = gb // 32
        r32 = gb % 32
        Q = r32 // 8
        g = r32 % 8
        cc = tc_ // 4
        ci = (tc_ % 4) * 128 + p_n
        R = g // 2
        gp = g % 2
        wl = Q * 4 + R
        return 32 * (wl % 3) + 7 * gp, (cc * 6 + wl // 3) * 512 + ci

    # ---------------- P_A ----------------
    nc = build_PA(st)
    SFA = sum((int(st.nj[j]) // P) * G * j for j in range(1, st.jmax + 1))
    in_maps = []
    for c in range(S):
        cs = st.cores[c]
        p_e = core_idx[c][1]
        p_n, gb = core_idx[c][7], core_idx[c][8]
        # node-major per-class layout for PA's degree reduce
        j = cs.jcls[cs.ecol]
        q_local = cs.nodepos[cs.ecol] - st.base_node[j]
        nbj = st.nj[j] // P
        b = q_local % nbj
        base_free = np.zeros(st.jmax + 2, np.int64)
        for jj in range(1, st.jmax + 1):
            base_free[jj + 1] = base_free[jj] + (st.nj[jj] // P) * G * jj
        offa = base_free[j] + b * (G * j) + cs.kocc
        ews_pa = np.zeros((P, SFA), np.float16)
        ews_pa[p_e, offa] = cs.eew.astype(np.float16)
        x_own = np.zeros((P, nb, F1), np.float16)
        x_own[p_n, gb] = x[c * NS + ar].astype(np.float16)
        in_maps.append({"ews_pa": ews_pa, "x_own": x_own.reshape(P, nb * F1)})
    res = _run(nc, in_maps, "PA_deg")
    dis_l = [res[c]["dis"] for c in range(S)]
    xp_l = [res[c]["xp"] for c in range(S)]

    xp_can = np.zeros((N, F1), np.float16)
    dis_can = np.zeros(N, np.float16)
    for c in range(S):
        p_n, gb = core_idx[c][7], core_idx[c][8]
        xp_can[c * NS + ar] = xp_l[c].reshape(P, nb, F1)[p_n, gb]
        dis_can[c * NS + ar] = dis_l[c][p_n, gb]

    # ---------------- P_B (layer 1 + MLP) ----------------
    nc = build_PB(st)
    W1h = W1.astype(np.float16)
    W2h = W2.astype(np.float16)
    w1bdb = np.zeros((P, 128), np.float16)
    for Q in range(3):
        for gl in range(8):
            w1bdb[32 * Q + 4 * gl : 32 * Q + 4 * gl + 3,
                  16 * gl : 16 * gl + 16] = W1h
    w1bdhb = np.zeros((P, 128), np.float16)
    for gl in range(8):
        w1bdhb[96 + 4 * gl : 96 + 4 * gl + 3, 16 * gl : 16 * gl + 16] = W1h
    w2bdb = np.zeros((P, 14), np.float16)
    for R in range(3):
        for gp in range(2):
            w2bdb[32 * R + 16 * gp : 32 * R + 16 * gp + 16,
                  7 * gp : 7 * gp + 7] = W2h
    w2bdhb = np.zeros((P, 14), np.float16)
    for gp in range(2):
        w2bdhb[96 + 16 * gp : 96 + 16 * gp + 16, 7 * gp : 7 * gp + 7] = W2h
    b1vb = b1.astype(np.float32)[np.arange(P) % 16].reshape(P, 1)

    in_maps = []
    ews_l = []
    for c in range(S):
        cs = st.cores[c]
        p_e, off1, off3, fs3 = (core_idx[c][1], core_idx[c][2],
                                core_idx[c][3], core_idx[c][4])
        p_n, gb = core_idx[c][7], core_idx[c][8]
        ews = np.zeros((P, st.SF1), np.float16)
        ews[p_e, off1] = cs.eew.astype(np.float16)
        ews_l.append(ews)
        xs = np.zeros((P, st.SF3), np.float16)
        for fi in range(F1):
            xs[p_e, off3 + fi * fs3] = xp_can[cs.erow, fi]
        xpo4 = np.zeros((P, nbp, 4), np.float16)
        xpo4[p_n, gb, 0:F1] = xp_l[c].reshape(P, nb, F1)[p_n, gb]
        dis4 = np.zeros((P, nbp, 4), np.float16)
        dis4[p_n, gb, 0:F1] = dis_l[c][p_n, gb][:, None]
        rbase, col_ = yst_pos(p_n, gb)
        disr = np.zeros((P, npy * 512), np.float16)
        for r in range(F2):
            disr[rbase + r, col_] = dis_l[c][p_n, gb]
        in_maps.append(
            {
                "ews": ews,
                "xs": xs,
                "xpo4": xpo4.reshape(P, nbp * 4),
                "dis4": dis4.reshape(P, nbp * 4),
                "w1bd": w1bdb,
                "w1bdh": w1bdhb,
                "w2bd": w2bdb,
                "w2bdh": w2bdhb,
                "b1v": b1vb,
                "disr": disr,
            }
        )
    res = _run(nc, in_maps, "PB_layer1")
    yst_l = [res[c]["yst"] for c in range(S)]

    ys_can = np.zeros((N, F2), np.float16)
    for c in range(S):
        p_n, gb = core_idx[c][7], core_idx[c][8]
        rbase, col_ = yst_pos(p_n, gb)
        v = np.empty((NS, F2), np.float16)
        for r in range(F2):
            v[:, r] = yst_l[c][rbase + r, col_]
        ys_can[c * NS + ar] = v

    # ---------------- P_E (layer 2) ----------------
    nc = build_PE(st)
    b2h = b2.astype(np.float16)
    in_maps = []
    for c in range(S):
        cs = st.cores[c]
        p_e, off7, fs7 = core_idx[c][1], core_idx[c][5], core_idx[c][6]
        p_n, nbase, nfstr = core_idx[c][7], core_idx[c][9], core_idx[c][10]
        yss = np.zeros((P, st.SF7), np.float16)
        for fi in range(F2):
            yss[p_e, off7 + fi * fs7] = ys_can[cs.erow, fi]
        yso = np.zeros((P, nb * F2), np.float16)
        dis7 = np.zeros((P, nb * F2), np.float16)
        ysl = ys_can[c * NS + ar]
        disl = dis_can[c * NS + ar]
        for fi in range(F2):
            yso[p_n, nbase + fi * nfstr] = ysl[:, fi]
            dis7[p_n, nbase + fi * nfstr] = disl
        im = {"ews": ews_l[c], "yss": yss, "yso": yso, "dis7": dis7}
        if st.b2_nonzero:
            b2e = np.zeros((P, nb * F2), np.float16)
            for fi in range(F2):
                b2e[p_n, nbase + fi * nfstr] = b2h[fi]
            im["b2e"] = b2e
        in_maps.append(im)
    res = _run(nc, in_maps, "PE_layer2")

    out = np.zeros((N, F2), np.float32)
    for c in range(S):
        o = res[c]["out"]
        p_n, nbase, nfstr = core_idx[c][7], core_idx[c][9], core_idx[c][10]
        for fi in range(F2):
            out[c * NS + ar, fi] = o[p_n, nbase + fi * nfstr].astype(
                np.float32
            )
    return out


# revision 7
# speedup vs baseline: 1.1616x; 1.0075x over previous
"""GCN (2-layer) on 8 Trainium2 NeuronCores — v4 (3 device programs).

Graph/data parallel per the node-range sharding hint: nodes sharded by
range, edges live on the destination core, weights replicated.  All
irregular routing happens on the HOST as pure copies/permutations;
every FP arithmetic op on values runs on device.

- Destination nodes bucketed by in-degree class j=ceil(d/4); each node
  gets exactly 4j slots so segment-sum becomes a slot-axis reduction.
- Big chunks use a SLOT-MAJOR [l, f, b] layout: the ew multiply and a
  tree of tensor_tensor adds are then fully contiguous fp16 APs, which
  is what the DVE 2x packed mode requires on hardware.  Small chunks
  stay node-major with one 1x tensor_reduce (fewer instructions).
- Node values feeding edges are pre-scaled on device (x'=dis*x,
  ys=dis*relu(vW1+b1)W2) so the per-edge device math is one multiply.
- PB's whole MLP runs on the Tensor engine: v is transposed via the PE,
  W1 is applied as a block-diagonal [32,128] stationary (8 nodes per
  matmul), bias+relu ride the scalar-engine PSUM eviction, W2 likewise
  block-diagonal [32,14]; dis scaling fuses into the final eviction.
"""
import sys

sys.path.insert(0, "/opt/trn_rl_repo")

import numpy as np

import bass_rust
from concourse import bass, mybir
from concourse.bass_utils import run_bass_kernel_spmd
from concourse.masks import make_identity
import concourse.tile as tile

import os as _os

PROGRAM_TIMES_NS = []   # (name, exec_time_ns) per device program of last kernel() call


def _enable_tracing():
    import types
    import antenv
    if 'antenv.axon_hooks' in sys.modules:
        return True
    try:
        from trn_agent_boot.trn_boot import _ntff_profile_via_ctypes
        hook = _ntff_profile_via_ctypes('/opt/axon/libaxon_pjrt.so')
    except Exception:
        return False
    mod = types.ModuleType('antenv.axon_hooks')
    mod.get_axon_ntff_profile_hook = lambda: hook
    mod.set_axon_ntff_profile_hook = lambda h: None
    sys.modules['antenv.axon_hooks'] = mod
    antenv.axon_hooks = mod
    import concourse.bass_utils as _bu
    _bu.upload_artifacts = lambda tmpdir: f"local://{tmpdir}"
    return True


def _run(nc, in_maps, name):
    trace = bool(_os.environ.get('GCN_TRACE')) and _enable_tracing()
    r = run_bass_kernel_spmd(nc, in_maps, core_ids=CORE_IDS, trace=trace)
    if trace:
        PROGRAM_TIMES_NS.append((name, r.exec_time_ns))
    return r.results

S = 8
N = 200000
NS = N // S
P = 128
G = 4            # degree-class granularity: slots per node = G*ceil(d/G)
F1 = 3
F2 = 7
CORE_IDS = list(range(S))
FP = mybir.dt.float32
F16 = mybir.dt.float16
MUL = mybir.AluOpType.mult
ADD = mybir.AluOpType.add
AX = mybir.AxisListType.X
RELU = mybir.ActivationFunctionType.Relu

_CHUNK7 = 16384   # chunk budget in F2-elems per partition
_SM_MIN = 1000    # below this (F2-elems), node-major + tensor_reduce


def _ceil(a, b):
    return -(-a // b)


# --------------------------------------------------------------------------
# walrus on this toolchain accepts at most ONE sync-wait per instruction;
# Tile emits several at DAG joins / kernel-tail drain. Hoist excess waits
# onto fresh same-engine NoOps inserted immediately before the violator.
def legalize_waits(nc):
    nop_idx = 0
    for f in nc.m.functions:
        for bb in f.blocks:
            il = bb.instructions
            if not any(
                inst.sync_info is not None
                and len(inst.sync_info.on_wait or []) > 1
                for inst in il
            ):
                continue
            new_il = []
            for inst in il:
                si = inst.sync_info
                w = list(si.on_wait or []) if si is not None else []
                if len(w) > 1:
                    for extra in w[:-1]:
                        nop = mybir.InstNoOp(
                            name=f"I-waitsplit-{nop_idx}", ins=[], outs=[]
                        )
                        nop_idx += 1
                        nop.engine = inst.engine
                        nop.sync_info = bass_rust.SyncInfo(
                            on_wait=[extra], on_update=[]
                        )
                        new_il.append(nop)
                    si.on_wait = [w[-1]]
                new_il.append(inst)
            bb.instructions = new_il


# --------------------------------------------------------------------------
# host-side structure building (integer routing only)
class _O:
    pass


def build_structs(row, col, ew):
    row = row.astype(np.int64)
    col = col.astype(np.int64)
    cores = []
    for c in range(S):
        cs = _O()
        m = (col // NS) == c
        cs.erow = row[m]
        cs.ecol = (col[m] - c * NS).astype(np.int64)
        cs.eew = ew[m].astype(np.float32)
        cores.append(cs)

    for cs in cores:
        d = np.bincount(cs.ecol, minlength=NS)
        cs.jcls = np.maximum(1, _ceil(np.maximum(d, 1), G))
    jmax = max(int(cs.jcls.max()) for cs in cores)
    nj = np.zeros(jmax + 1, np.int64)
    for j in range(1, jmax + 1):
        njc = max(int((cs.jcls == j).sum()) for cs in cores)
        nj[j] = _ceil(max(njc, 1), P) * P
    for cs in cores:
        nodepos = np.full(NS, -1, np.int64)
        pos = 0
        for j in range(1, jmax + 1):
            nodes = np.nonzero(cs.jcls == j)[0]
            nodepos[nodes] = pos + np.arange(len(nodes))
            pos += nj[j]
        cs.nodepos = nodepos
    ntot = int(nj[1:].sum())

    st = _O()
    st.jmax, st.nj, st.ntot = jmax, nj, ntot
    st.cores = cores

    # per-edge occurrence index among edges sharing a destination
    for cs in cores:
        nodes = cs.ecol
        ord_ = np.argsort(nodes, kind="stable")
        ns = nodes[ord_]
        first = np.r_[True, ns[1:] != ns[:-1]]
        idx_of_first = np.maximum.accumulate(
            np.where(first, np.arange(len(ord_)), 0)
        )
        k = np.empty(len(nodes), np.int64)
        k[ord_] = np.arange(len(ord_)) - idx_of_first
        cs.kocc = k

    # class bases
    st.base_node = np.zeros(jmax + 2, np.int64)
    st.base_blk = np.zeros(jmax + 2, np.int64)
    for j in range(1, jmax + 1):
        st.base_node[j + 1] = st.base_node[j] + nj[j]
        st.base_blk[j + 1] = st.base_blk[j] + nj[j] // P

    # shared chunk plan (same block ranges for all three per-edge tensors);
    # carve a small GpSimd-sized chunk off the three biggest classes
    # (GpSimd offload measured net-negative: concurrent GpSimd SBUF traffic
    # halves the DVE 2x packed-mode rate, so no chunks are carved for it)
    gp_classes = set()
    chunks = []
    off1 = off3 = off7 = agg7 = 0
    for j in range(1, jmax + 1):
        L = G * j
        nbj = int(nj[j]) // P
        blmax = max(2, (_CHUNK7 // (F2 * L)) & ~1)
        gp_left = 14 if (j in gp_classes and nbj >= 40) else 0
        b0 = 0
        while b0 < nbj:
            if gp_left:
                bl, gp = gp_left, True
                gp_left = 0
            else:
                bl, gp = min(blmax, nbj - b0), False
                if bl % 2 == 1 and bl > 1 and bl * F2 * L >= _SM_MIN:
                    bl -= 1   # keep the big chunk even (slot-major capable)
            sm = (bl % 2 == 0) and (gp or bl * F2 * L >= _SM_MIN)
            ck = _O()
            ck.j, ck.L, ck.b0, ck.bl, ck.sm, ck.gp = j, L, b0, bl, sm, gp
            ck.off1, ck.off3, ck.off7, ck.agg7 = off1, off3, off7, agg7
            chunks.append(ck)
            off1 += bl * L
            off3 += bl * F1 * L
            off7 += bl * F2 * L
            agg7 += bl * F2
            b0 += bl
    st.chunks = chunks
    st.SF1, st.SF3, st.SF7 = off1, off3, off7
    assert agg7 == (ntot // P) * F2

    # per-class chunk lookup tables (by block index)
    st.cmap = {}
    for j in range(1, jmax + 1):
        nbj = int(nj[j]) // P
        cid = np.zeros(nbj, np.int64)
        for i, ck in enumerate(chunks):
            if ck.j == j:
                cid[ck.b0 : ck.b0 + ck.bl] = i
        st.cmap[j] = cid
    return st


def edge_slots(st, cs):
    """per edge: partition p, and for each tensor the flat offset of the
    (slot k, feature 0) element plus the per-feature stride."""
    nodes = cs.ecol
    j = cs.jcls[nodes]
    q_local = cs.nodepos[nodes] - st.base_node[j]
    nbj = st.nj[j] // P
    p = q_local // nbj
    b = q_local % nbj
    k = cs.kocc
    ne = len(nodes)
    off1 = np.empty(ne, np.int64)
    off3 = np.empty(ne, np.int64)
    off7 = np.empty(ne, np.int64)
    fs3 = np.empty(ne, np.int64)
    fs7 = np.empty(ne, np.int64)
    for jj in range(1, st.jmax + 1):
        sel = j == jj
        if not sel.any():
            continue
        L = G * jj
        ci = st.cmap[jj][b[sel]]
        cb0 = np.array([st.chunks[i].b0 for i in range(len(st.chunks))])
        cbl = np.array([st.chunks[i].bl for i in range(len(st.chunks))])
        csm = np.array([st.chunks[i].sm for i in range(len(st.chunks))])
        c1 = np.array([st.chunks[i].off1 for i in range(len(st.chunks))])
        c3 = np.array([st.chunks[i].off3 for i in range(len(st.chunks))])
        c7 = np.array([st.chunks[i].off7 for i in range(len(st.chunks))])
        bo = b[sel] - cb0[ci]
        bl = cbl[ci]
        sm = csm[ci]
        kk = k[sel]
        # slot-major: base + k*(F*bl) + f*bl + bo ; node-major: bo*(F*L)+f*L+k
        off1[sel] = np.where(sm, c1[ci] + kk * bl + bo,
                             c1[ci] + bo * L + kk)
        off3[sel] = np.where(sm, c3[ci] + kk * (F1 * bl) + bo,
                             c3[ci] + bo * (F1 * L) + kk)
        off7[sel] = np.where(sm, c7[ci] + kk * (F2 * bl) + bo,
                             c7[ci] + bo * (F2 * L) + kk)
        fs3[sel] = np.where(sm, bl, L)
        fs7[sel] = np.where(sm, bl, L)
    return p, off1, off3, fs3, off7, fs7


def node_slots(st, cs):
    """per local node: partition p, global block gb, and the (base, fstride)
    of its features in the chunk-major agg7 layout."""
    j = cs.jcls
    q_local = cs.nodepos - st.base_node[j]
    nbj = st.nj[j] // P
    p = q_local // nbj
    b = q_local % nbj
    gb = st.base_blk[j] + b
    ns = len(j)
    base = np.empty(ns, np.int64)
    fstr = np.empty(ns, np.int64)
    cb0 = np.array([c.b0 for c in st.chunks])
    cbl = np.array([c.bl for c in st.chunks])
    csm = np.array([c.sm for c in st.chunks])
    ca7 = np.array([c.agg7 for c in st.chunks])
    for jj in range(1, st.jmax + 1):
        sel = j == jj
        if not sel.any():
            continue
        ci = st.cmap[jj][b[sel]]
        bo = b[sel] - cb0[ci]
        base[sel] = np.where(csm[ci], ca7[ci] + bo,
                             ca7[ci] + bo * F2)
        fstr[sel] = np.where(csm[ci], cbl[ci], 1)
    return p, gb, base, fstr


# --------------------------------------------------------------------------
# device-program helpers
def _fold_flat(nc, eng, t_in, FB, L, out_ap, fshape=None):
    """tree-fold the leading slot axis of a slot-major chunk (viewed as
    [P, L*FB] with l outermost): contiguous halves, all 2x; the final add
    writes out_ap (free size FB; if fshape=(F, bl) the inputs are viewed
    [P, F, bl] to match a shaped/strided out_ap)."""
    tv = t_in[:]
    cur = L
    while cur > 2:
        if cur % 2 == 1:
            eng.tensor_tensor(
                tv[:, 0:FB], tv[:, 0:FB],
                tv[:, (cur - 1) * FB : cur * FB], ADD,
            )
            cur -= 1
            if cur == 2:
                break
        h = cur // 2
        eng.tensor_tensor(
            tv[:, 0 : h * FB], tv[:, 0 : h * FB],
            tv[:, h * FB : cur * FB], ADD,
        )
        cur = h
    i0, i1 = tv[:, 0:FB], tv[:, FB : 2 * FB]
    if fshape is not None:
        F, bl = fshape
        i0 = i0.rearrange("p (f b) -> p f b", f=F)
        i1 = i1.rearrange("p (f b) -> p f b", f=F)
    eng.tensor_tensor(out_ap, i0, i1, ADD)


def _edge_stream(nc, pool, st, F, xs, t_ew, agg_of, tag, shaped_fold=False,
                 subset=None, ews_dram=None, ew_state=None):
    """Stream per-edge fp16 payload chunks, multiply by the ew slots and
    reduce the slot axis.  agg_of(ck) -> output AP (free size bl*F) in the
    chosen agg layout for that chunk.  If ews_dram is given, the ew slots
    are DMA'd just-in-time in pieces right before the chunks needing them."""
    for ck in (subset if subset is not None else st.chunks):
        L, bl = ck.L, ck.bl
        eng = nc.vector
        if ews_dram is not None:
            need = ck.off1 + bl * L
            if need > ew_state["done"]:
                end = max(need, min(st.SF1, ew_state["done"] + 2048))
                nc.sync.dma_start(
                    out=t_ew[:, ew_state["done"] : end],
                    in_=ews_dram[:, ew_state["done"] : end],
                )
                ew_state["done"] = end
        offF = ck.off1 if F == 1 else (ck.off3 if F == F1 else ck.off7)
        n = bl * F * L
        t_in = pool.tile([P, n], F16, tag=tag)
        nc.sync.dma_start(out=t_in[:], in_=xs[:, offF : offF + n])
        if ck.sm:
            FB = F * bl
            if t_ew is not None:
                eng.tensor_tensor(
                    t_in[:].rearrange("p (l f b) -> p l f b", l=L, f=F),
                    t_in[:].rearrange("p (l f b) -> p l f b", l=L, f=F),
                    t_ew[:, ck.off1 : ck.off1 + bl * L]
                    .rearrange("p (l b) -> p l b", l=L)
                    .unsqueeze(2)
                    .broadcast_to([P, L, F, bl]),
                    MUL,
                )
            _fold_flat(nc, eng, t_in, FB, L, agg_of(ck),
                       fshape=(F, bl) if shaped_fold else None)
        else:
            if t_ew is not None:
                eng.tensor_tensor(
                    t_in[:].rearrange("p (b f l) -> p b f l", f=F, l=L),
                    t_in[:].rearrange("p (b f l) -> p b f l", f=F, l=L),
                    t_ew[:, ck.off1 : ck.off1 + bl * L]
                    .rearrange("p (b l) -> p b l", l=L)
                    .unsqueeze(2)
                    .broadcast_to([P, bl, F, L]),
                    MUL,
                )
            with nc.allow_low_precision(reason="fp16 agg within tolerance"):
                eng.tensor_reduce(
                    out=agg_of(ck),
                    in_=t_in[:].rearrange("p (b f l) -> p b f l", f=F, l=L),
                    axis=AX,
                    op=ADD,
                )


def build_PA(st):
    """ews_pa (node-major, per class) + x_own -> dis (fp16), xp = dis*x."""
    nc = bass.Bass("TRN2", num_devices=S)
    nb = st.ntot // P
    SFA = sum((int(st.nj[j]) // P) * G * j for j in range(1, st.jmax + 1))
    ews = nc.dram_tensor("ews_pa", (P, SFA), F16, kind="ExternalInput")
    x_own = nc.dram_tensor("x_own", (P, nb * F1), F16, kind="ExternalInput")
    dis_o = nc.dram_tensor("dis", (P, nb), F16, kind="ExternalOutput")
    xp_o = nc.dram_tensor("xp", (P, nb * F1), F16, kind="ExternalOutput")
    with tile.TileContext(nc) as tc:
        with tc.tile_pool(name="acc", bufs=1) as apool:
            # whole ews resident; two DMA pieces so reduces start early
            t_ews = apool.tile([P, SFA], F16)
            half = 0
            accf = 0
            for j in range(1, st.jmax + 1):
                if accf >= SFA // 2:
                    half = accf
                    break
                accf += (int(st.nj[j]) // P) * G * j
            if not half:
                half = SFA
            nc.sync.dma_start(out=t_ews[:, :half], in_=ews[:, :half])
            if half < SFA:
                nc.sync.dma_start(out=t_ews[:, half:], in_=ews[:, half:])
            t_xo = apool.tile([P, nb * F1], F16)
            nc.sync.dma_start(out=t_xo[:], in_=x_own[:])
            t_deg = apool.tile([P, nb], F16)
            accf = 0
            accb = 0
            for j in range(1, st.jmax + 1):
                L = G * j
                nbj = int(st.nj[j]) // P
                with nc.allow_low_precision(
                    reason="fp16 deg within tolerance"
                ):
                    nc.vector.tensor_reduce(
                        out=t_deg[:, accb : accb + nbj],
                        in_=t_ews[:, accf : accf + nbj * L].rearrange(
                            "p (b l) -> p b l", l=L
                        ),
                        axis=AX,
                        op=ADD,
                    )
                accf += nbj * L
                accb += nbj
            # sqrt(deg + 1) in one activation: upcast + bias fused
            t_sq = apool.tile([P, nb], FP)
            nc.scalar.activation(
                t_sq[:], t_deg[:], mybir.ActivationFunctionType.Sqrt,
                bias=1.0,
            )
            t_r = apool.tile([P, nb], FP)
            nc.vector.reciprocal(t_r[:], t_sq[:])
            t_d16 = apool.tile([P, nb], F16)
            nc.scalar.copy(t_d16[:], t_r[:])
            nc.sync.dma_start(out=dis_o[:], in_=t_d16[:])
            t_xp = apool.tile([P, nb * F1], F16)
            nc.vector.tensor_tensor(
                t_xp[:].rearrange("p (b f) -> p b f", f=F1),
                t_xo[:].rearrange("p (b f) -> p b f", f=F1),
                t_d16[:].unsqueeze(2).broadcast_to([P, nb, F1]),
                MUL,
            )
            nc.sync.dma_start(out=xp_o[:], in_=t_xp[:])
    legalize_waits(nc)
    return nc


def _pb_geom(st):
    nb = st.ntot // P
    nbp = _ceil(nb, 128) * 128   # v4 node blocks padded (32 nodes/tile, 4-col)
    ntiles = nbp // 32           # 128-col transpose tiles of 4-stride v
    ncc = ntiles // 4            # 512-col W1 column chunks
    ne = ncc * 4                 # W1 matmuls / ht 512-col chunks
    nw2 = ne * 4                 # W2 matmuls
    npy = ncc * 6                # y PSUM tiles (3 outs at 0/32/64, per cc)
    return nb, nbp, ntiles, ncc, ne, nw2, npy


def build_PB(st):
    """xs + ews + xp_own + dis -> yst: packed dis*relu(vW1+b1)@W2 (fp16).

    v is kept 4-strided per node [b*4+f]; PE transposes 128-col tiles; W1
    is a block-diagonal [32,128] stationary (8 nodes per matmul, K row
    groups at partition bases 0/32/64 and a widened K=64 for the base-96
    group); bias+relu fuse into the scalar-engine PSUM eviction; W2 is
    block-diagonal [32,14]; dis scaling fuses into the final eviction."""
    nc = bass.Bass("TRN2", num_devices=S)
    nb, nbp, ntiles, ncc, ne, nw2, npy = _pb_geom(st)
    ews = nc.dram_tensor("ews", (P, st.SF1), F16, kind="ExternalInput")
    xs = nc.dram_tensor("xs", (P, st.SF3), F16, kind="ExternalInput")
    xpo4 = nc.dram_tensor("xpo4", (P, nbp * 4), F16, kind="ExternalInput")
    dis4 = nc.dram_tensor("dis4", (P, nbp * 4), F16, kind="ExternalInput")
    w1bd = nc.dram_tensor("w1bd", (P, 128), F16, kind="ExternalInput")
    w1bdh = nc.dram_tensor("w1bdh", (P, 128), F16, kind="ExternalInput")
    w2bd = nc.dram_tensor("w2bd", (P, 14), F16, kind="ExternalInput")
    w2bdh = nc.dram_tensor("w2bdh", (P, 14), F16, kind="ExternalInput")
    b1v = nc.dram_tensor("b1v", (P, 1), FP, kind="ExternalInput")
    disr = nc.dram_tensor("disr", (P, npy * 512), F16, kind="ExternalInput")
    yst_o = nc.dram_tensor("yst", (P, npy * 512), F16, kind="ExternalOutput")
    with tile.TileContext(nc) as tc:
        with tc.tile_pool(name="sb", bufs=3) as pool, tc.tile_pool(
            name="acc", bufs=1
        ) as apool, tc.tile_pool(
            name="ptr", bufs=2, space="PSUM"
        ) as tpool, tc.tile_pool(
            name="ph", bufs=2, space="PSUM"
        ) as hpool, tc.tile_pool(
            name="py", bufs=2, space="PSUM"
        ) as ypool:
            t_v4 = apool.tile([P, nbp * 4], F16)
            nc.gpsimd.memset(t_v4[:], 0.0)
            t_id = apool.tile([P, P], F16)
            make_identity(nc, t_id[:])
            t_ew = apool.tile([P, st.SF1], F16)
            t_xpo = apool.tile([P, nbp * 4], F16)
            t_dis = apool.tile([P, nbp * 4], F16)

            def agg_of(ck):
                gb0 = int(st.base_blk[ck.j]) + ck.b0
                view = t_v4[:, gb0 * 4 : (gb0 + ck.bl) * 4].rearrange(
                    "p (b f) -> p b f", f=4
                )[:, :, 0:F1]
                if ck.sm:
                    # fold's final add iterates (f, b)
                    return view.rearrange("p b f -> p f b")
                return view

            t_vt = apool.tile([P, ntiles * 128], F16)
            t_ht = apool.tile([P, ne * 512], F16)
            t_yst = apool.tile([P, npy * 512], F16)
            t_w1 = apool.tile([P, 128], F16)
            t_w1h = apool.tile([P, 128], F16)
            t_w2 = apool.tile([P, 14], F16)
            t_w2h = apool.tile([P, 14], F16)
            t_b1v = apool.tile([P, 1], FP)
            t_disr = apool.tile([P, npy * 512], F16)

            def mlp_part(cc):
                # v-prep for this 512-col slice, then the PE pipeline
                sl = slice(cc * 512, (cc + 1) * 512)
                nc.vector.tensor_tensor(
                    t_v4[:, sl], t_v4[:, sl], t_xpo[:, sl], ADD
                )
                nc.vector.tensor_tensor(
                    t_v4[:, sl], t_v4[:, sl], t_dis[:, sl], MUL
                )
                t_tr = tpool.tile([P, 512], F16, tag="tr")
                for gi in range(4):
                    tt = cc * 4 + gi
                    nc.tensor.transpose(
                        t_tr[:, gi * 128 : (gi + 1) * 128],
                        t_v4[:, tt * 128 : (tt + 1) * 128],
                        t_id[:],
                    )
                nc.scalar.copy(t_vt[:, sl], t_tr[:])
                t_y = None
                for Q in range(4):
                    e = cc * 4 + Q
                    t_hp = hpool.tile([P, 512], FP, tag="h")
                    if Q < 3:
                        nc.tensor.matmul(
                            t_hp[:, :],
                            t_w1[32 * Q : 32 * Q + 32, :],
                            t_vt[32 * Q : 32 * Q + 32, sl],
                        )
                    else:
                        nc.tensor.matmul(
                            t_hp[:, :],
                            t_w1h[64:128, :],
                            t_vt[64:128, sl],
                        )
                    nc.scalar.activation(
                        t_ht[:, e * 512 : (e + 1) * 512],
                        t_hp[:, :],
                        RELU,
                        bias=t_b1v[:, 0:1],
                    )
                    # W2 for this group rides right behind its relu so the
                    # per-part chain pipelines across engines
                    for R in range(4):
                        wl = Q * 4 + R
                        s = wl % 3
                        if s == 0:
                            t_y = ypool.tile([P, 512], FP, tag="y")
                        if R < 3:
                            nc.tensor.matmul(
                                t_y[32 * s : 32 * s + 14, :],
                                t_w2[32 * R : 32 * R + 32, :],
                                t_ht[32 * R : 32 * R + 32,
                                     e * 512 : (e + 1) * 512],
                            )
                        else:
                            nc.tensor.matmul(
                                t_y[32 * s : 32 * s + 14, :],
                                t_w2h[64:128, :],
                                t_ht[64:128, e * 512 : (e + 1) * 512],
                            )
                        if s == 2 or wl == 15:
                            pt = cc * 6 + wl // 3
                            nc.vector.tensor_tensor(
                                t_yst[:, pt * 512 : (pt + 1) * 512],
                                t_y[:, :],
                                t_disr[:, pt * 512 : (pt + 1) * 512],
                                MUL,
                            )

            # interleave: edge chunks needed by column-chunk cc, then its
            # MLP part, so the Tensor/Scalar pipeline overlaps the stream
            gs = [int(st.base_blk[ck.j]) + ck.b0 for ck in st.chunks]
            done = 0
            ew_state = {"done": 0}
            for cc in range(ncc):
                need = 128 * (cc + 1)
                hi = len(st.chunks)
                if cc < ncc - 1:
                    hi = next(
                        (i for i, g in enumerate(gs) if g >= need),
                        len(st.chunks),
                    )
                if cc == 0:
                    # node operands + weights arrive mid-stream, after the
                    # first chunks are queued but before the MLP needs them
                    mid = max(done + 1, (done + hi) // 2)
                    _edge_stream(nc, pool, st, F1, xs, t_ew, agg_of, "x",
                                 shaped_fold=True,
                                 subset=st.chunks[done:mid],
                                 ews_dram=ews, ew_state=ew_state)
                    nc.sync.dma_start(out=t_xpo[:], in_=xpo4[:])
                    nc.sync.dma_start(out=t_dis[:], in_=dis4[:])
                    nc.sync.dma_start(out=t_w1[:], in_=w1bd[:])
                    nc.sync.dma_start(out=t_w1h[:], in_=w1bdh[:])
                    nc.sync.dma_start(out=t_b1v[:], in_=b1v[:])
                    done = mid
                _edge_stream(nc, pool, st, F1, xs, t_ew, agg_of, "x",
                             shaped_fold=True,
                             subset=st.chunks[done:hi],
                             ews_dram=ews, ew_state=ew_state)
                done = hi
                if cc == 0:
                    # W2/disr are first needed ~10us into the MLP; keep
                    # their 1.8MB out of the part-0 chunk queue
                    nc.sync.dma_start(out=t_w2[:], in_=w2bd[:])
                    nc.sync.dma_start(out=t_w2h[:], in_=w2bdh[:])
                    nc.sync.dma_start(out=t_disr[:], in_=disr[:])
                mlp_part(cc)
                # ship this part's packed outputs while the next streams
                nc.sync.dma_start(
                    out=yst_o[:, cc * 6 * 512 : (cc + 1) * 6 * 512],
                    in_=t_yst[:, cc * 6 * 512 : (cc + 1) * 6 * 512],
                )
    legalize_waits(nc)
    return nc


def build_PE(st):
    """yss + ews + yso + dis7 + b2e -> out = dis*(agg2 + ys_own) + b2,
    everything in the chunk-major agg layout (host unscrambles)."""
    nc = bass.Bass("TRN2", num_devices=S)
    nb = st.ntot // P
    ews = nc.dram_tensor("ews", (P, st.SF1), F16, kind="ExternalInput")
    yss = nc.dram_tensor("yss", (P, st.SF7), F16, kind="ExternalInput")
    yso = nc.dram_tensor("yso", (P, nb * F2), F16, kind="ExternalInput")
    dis7 = nc.dram_tensor("dis7", (P, nb * F2), F16, kind="ExternalInput")
    if st.b2_nonzero:
        b2e = nc.dram_tensor("b2e", (P, nb * F2), F16, kind="ExternalInput")
    out_o = nc.dram_tensor("out", (P, nb * F2), F16, kind="ExternalOutput")
    with tile.TileContext(nc) as tc:
        with tc.tile_pool(name="sb", bufs=3) as pool, tc.tile_pool(
            name="acc", bufs=1
        ) as apool:
            t_ew = apool.tile([P, st.SF1], F16)
            t_yso = apool.tile([P, nb * F2], F16)
            t_dis7 = apool.tile([P, nb * F2], F16)
            t_b2e = apool.tile([P, nb * F2], F16)

            t_agg = apool.tile([P, nb * F2], F16)

            def agg_of(ck):
                sl = t_agg[:, ck.agg7 : ck.agg7 + ck.bl * F2]
                if ck.sm:
                    return sl
                return sl.rearrange("p (b f) -> p b f", f=F2)

            ew_state = {"done": 0}
            k1 = min(8, len(st.chunks) - 2)
            _edge_stream(nc, pool, st, F2, yss, t_ew, agg_of, "y",
                         subset=st.chunks[:k1],
                         ews_dram=ews, ew_state=ew_state)
            # node-level operands load while the stream runs (after the
            # big chunks are queued so they don't delay the ramp)
            nc.sync.dma_start(out=t_yso[:], in_=yso[:])
            nc.sync.dma_start(out=t_dis7[:], in_=dis7[:])
            if st.b2_nonzero:
                nc.sync.dma_start(out=t_b2e[:], in_=b2e[:])

            def finals(lo, hi):
                sl = slice(lo, hi)
                nc.vector.tensor_tensor(
                    t_agg[:, sl], t_agg[:, sl], t_yso[:, sl], ADD
                )
                nc.vector.tensor_tensor(
                    t_agg[:, sl], t_agg[:, sl], t_dis7[:, sl], MUL
                )
                if st.b2_nonzero:
                    nc.vector.tensor_tensor(
                        t_agg[:, sl], t_agg[:, sl], t_b2e[:, sl], ADD
                    )
                nc.sync.dma_start(out=out_o[:, sl], in_=t_agg[:, sl])

            # finals + output ship in two halves so the first overlaps the
            # tail of the stream
            half = max(k1, (2 * len(st.chunks)) // 3)
            _edge_stream(nc, pool, st, F2, yss, t_ew, agg_of, "y",
                         subset=st.chunks[k1:half],
                         ews_dram=ews, ew_state=ew_state)
            H = st.chunks[half].agg7 if half < len(st.chunks) else nb * F2
            finals(0, H)
            _edge_stream(nc, pool, st, F2, yss, t_ew, agg_of, "y",
                         subset=st.chunks[half:],
                         ews_dram=ews, ew_state=ew_state)
            if H < nb * F2:
                finals(H, nb * F2)
    legalize_waits(nc)
    return nc


# --------------------------------------------------------------------------
def kernel(x, edge_index, edge_weight, W1, b1, W2, b2):
    x = np.asarray(x, np.float32)
    ei = np.asarray(edge_index)
    ew = np.asarray(edge_weight, np.float32)
    W1 = np.asarray(W1, np.float32)
    b1 = np.asarray(b1, np.float32)
    W2 = np.asarray(W2, np.float32)
    b2 = np.asarray(b2, np.float32)

    PROGRAM_TIMES_NS.clear()
    st = build_structs(ei[0], ei[1], ew)
    st.b2_nonzero = bool(np.any(b2))
    nb = st.ntot // P
    _, nbp, ntiles, ncc, ne, nw2, npy = _pb_geom(st)
    ar = np.arange(NS)

    core_idx = []
    for c in range(S):
        cs = st.cores[c]
        p_e, off1, off3, fs3, off7, fs7 = edge_slots(st, cs)
        p_n, gb, nbase, nfstr = node_slots(st, cs)
        core_idx.append((cs, p_e, off1, off3, fs3, off7, fs7,
                         p_n, gb, nbase, nfstr))

    # node (p,gb) -> (row base before feature, column) in packed yst
    def yst_pos(p_n, gb):
        tc_ = gb // 32
        r32 = gb % 32
        Q = r32 // 8
        g = r32 % 8
        cc = tc_ // 4
        ci = (tc_ % 4) * 128 + p_n
        R = g // 2
        gp = g % 2
        wl = Q * 4 + R
        return 32 * (wl % 3) + 7 * gp, (cc * 6 + wl // 3) * 512 + ci

    # ---------------- P_A ----------------
    nc = build_PA(st)
    SFA = sum((int(st.nj[j]) // P) * G * j for j in range(1, st.jmax + 1))
    in_maps = []
    for c in range(S):
        cs = st.cores[c]
        p_e = core_idx[c][1]
        p_n, gb = core_idx[c][7], core_idx[c][8]
        # node-major per-class layout for PA's degree reduce
        j = cs.jcls[cs.ecol]
        q_local = cs.nodepos[cs.ecol] - st.base_node[j]
        nbj = st.nj[j] // P
        b = q_local % nbj
        base_free = np.zeros(st.jmax + 2, np.int64)
        for jj in range(1, st.jmax + 1):
            base_free[jj + 1] = base_free[jj] + (st.nj[jj] // P) * G * jj
        offa = base_free[j] + b * (G * j) + cs.kocc
        ews_pa = np.zeros((P, SFA), np.float16)
        ews_pa[p_e, offa] = cs.eew.astype(np.float16)
        x_own = np.zeros((P, nb, F1), np.float16)
        x_own[p_n, gb] = x[c * NS + ar].astype(np.float16)
        in_maps.append({"ews_pa": ews_pa, "x_own": x_own.reshape(P, nb * F1)})
    res = _run(nc, in_maps, "PA_deg")
    dis_l = [res[c]["dis"] for c in range(S)]
    xp_l = [res[c]["xp"] for c in range(S)]

    xp_can = np.zeros((N, F1), np.float16)
    dis_can = np.zeros(N, np.float16)
    for c in range(S):
        p_n, gb = core_idx[c][7], core_idx[c][8]
        xp_can[c * NS + ar] = xp_l[c].reshape(P, nb, F1)[p_n, gb]
        dis_can[c * NS + ar] = dis_l[c][p_n, gb]

    # ---------------- P_B (layer 1 + MLP) ----------------
    nc = build_PB(st)
    W1h = W1.astype(np.float16)
    W2h = W2.astype(np.float16)
    w1bdb = np.zeros((P, 128), np.float16)
    for Q in range(3):
        for gl in range(8):
            w1bdb[32 * Q + 4 * gl : 32 * Q + 4 * gl + 3,
                  16 * gl : 16 * gl + 16] = W1h
    w1bdhb = np.zeros((P, 128), np.float16)
    for gl in range(8):
        w1bdhb[96 + 4 * gl : 96 + 4 * gl + 3, 16 * gl : 16 * gl + 16] = W1h
    w2bdb = np.zeros((P, 14), np.float16)
    for R in range(3):
        for gp in range(2):
            w2bdb[32 * R + 16 * gp : 32 * R + 16 * gp + 16,
                  7 * gp : 7 * gp + 7] = W2h
    w2bdhb = np.zeros((P, 14), np.float16)
    for gp in range(2):
        w2bdhb[96 + 16 * gp : 96 + 16 * gp + 16, 7 * gp : 7 * gp + 7] = W2h
    b1vb = b1.astype(np.float32)[np.arange(P) % 16].reshape(P, 1)

    in_maps = []
    ews_l = []
    for c in range(S):
        cs = st.cores[c]
        p_e, off1, off3, fs3 = (core_idx[c][1], core_idx[c][2],
                                core_idx[c][3], core_idx[c][4])
        p_n, gb = core_idx[c][7], core_idx[c][8]
        ews = np.zeros((P, st.SF1), np.float16)
        ews[p_e, off1] = cs.eew.astype(np.float16)
        ews_l.append(ews)
        xs = np.zeros((P, st.SF3), np.float16)
        for fi in range(F1):
            xs[p_e, off3 + fi * fs3] = xp_can[cs.erow, fi]
        xpo4 = np.zeros((P, nbp, 4), np.float16)
        xpo4[p_n, gb, 0:F1] = xp_l[c].reshape(P, nb, F1)[p_n, gb]
        dis4 = np.zeros((P, nbp, 4), np.float16)
        dis4[p_n, gb, 0:F1] = dis_l[c][p_n, gb][:, None]
        rbase, col_ = yst_pos(p_n, gb)
        disr = np.zeros((P, npy * 512), np.float16)
        for r in range(F2):
            disr[rbase + r, col_] = dis_l[c][p_n, gb]
        in_maps.append(
            {
                "ews": ews,
                "xs": xs,
                "xpo4": xpo4.reshape(P, nbp * 4),
                "dis4": dis4.reshape(P, nbp * 4),
                "w1bd": w1bdb,
                "w1bdh": w1bdhb,
                "w2bd": w2bdb,
                "w2bdh": w2bdhb,
                "b1v": b1vb,
                "disr": disr,
            }
        )
    res = _run(nc, in_maps, "PB_layer1")
    yst_l = [res[c]["yst"] for c in range(S)]

    ys_can = np.zeros((N, F2), np.float16)
    for c in range(S):
        p_n, gb = core_idx[c][7], core_idx[c][8]
        rbase, col_ = yst_pos(p_n, gb)
        v = np.empty((NS, F2), np.float16)
        for r in range(F2):
            v[:, r] = yst_l[c][rbase + r, col_]
        ys_can[c * NS + ar] = v

    # ---------------- P_E (layer 2) ----------------
    nc = build_PE(st)
    b2h = b2.astype(np.float16)
    in_maps = []
    for c in range(S):
        cs = st.cores[c]
        p_e, off7, fs7 = core_idx[c][1], core_idx[c][5], core_idx[c][6]
        p_n, nbase, nfstr = core_idx[c][7], core_idx[c][9], core_idx[c][10]
        yss = np.zeros((P, st.SF7), np.float16)
        for fi in range(F2):
            yss[p_e, off7 + fi * fs7] = ys_can[cs.erow, fi]
        yso = np.zeros((P, nb * F2), np.float16)
        dis7 = np.zeros((P, nb * F2), np.float16)
        ysl = ys_can[c * NS + ar]
        disl = dis_can[c * NS + ar]
        for fi in range(F2):
            yso[p_n, nbase + fi * nfstr] = ysl[:, fi]
            dis7[p_n, nbase + fi * nfstr] = disl
        im = {"ews": ews_l[c], "yss": yss, "yso": yso, "dis7": dis7}
        if st.b2_nonzero:
            b2e = np.zeros((P, nb * F2), np.float16)
            for fi in range(F2):
                b2e[p_n, nbase + fi * nfstr] = b2h[fi]
            im["b2e"] = b2e
        in_maps.append(im)
    res = _run(nc, in_maps, "PE_layer2")

    out = np.zeros((N, F2), np.float32)
    for c in range(S):
        o = res[c]["out"]
        p_n, nbase, nfstr = core_idx[c][7], core_idx[c][9], core_idx[c][10]
        for fi in range(F2):
            out[c * NS + ar, fi] = o[p_n, nbase + fi * nfstr].astype(
                np.float32
            )
    return out
